# revision 18
# baseline (speedup 1.0000x reference)
"""AdaptiveLocalConv Trainium2 kernel — 8-core SPMD.

Sharding: (batch, seq-quarter) -> 8 shards of 1024 tokens (+64 halo each side
for the deformable gather, reach <= +-19).

Per-core pipeline:
  - 4 projections from x via PE (f32r), x passed pre-transposed [C, 1152].
  - per-(token,head): 13 deformable taps; kernel-table interpolation via a
    log2 halving-gather on DVE; taps placed into a banded matrix A
    [token, head, 256-slot J-band] with gpsimd local_scatter (bf16).
  - banded matmul out^T[d, l] = sum_J v[J, d] * A^T[J, l] on PE after
    PE-transposing A blocks; sequence-end clamp handled exactly by a rank-1
    correction matmul.
  - squeeze-excite via a 4-core AllReduce of the per-core partial mean (2KB),
    SE scale folded into out_w columns; final out_w matmul in transposed
    layout; silu; DMA out with a transposing access pattern.
"""
import sys
if "/opt/trn_rl_repo" not in sys.path:
    sys.path.insert(0, "/opt/trn_rl_repo")

import numpy as np
import ml_dtypes

import concourse.bass as bass
import concourse.mybir as mybir
from concourse import bacc
from concourse.tile import TileContext
from concourse.bass_utils import run_bass_kernel_spmd
from concourse.masks import make_identity

F32 = mybir.dt.float32
F32R = mybir.dt.float32r
BF16 = mybir.dt.bfloat16
I32 = mybir.dt.int32
I16 = mybir.dt.int16
OP = mybir.AluOpType
ACT = mybir.ActivationFunctionType

B, L, C, H, K, D = 2, 4096, 512, 8, 64, 64
P = 128
HALO = 64
LSH = 1024          # tokens per core
XROWS = LSH + 2 * HALO   # 1152
LT = LSH // P       # 8 own l-tiles
VT = XROWS // P     # 9 v tiles
NB_HI = np.float32(L - 1.001)
EPS = 1e-6

_GRAPH_CACHE = {}


def _bcast(ap, shape):
    return ap.broadcast_to(shape)


def build_graph(n_cores=8, skip_cc=False):
    nc = bacc.Bacc("TRN2", target_bir_lowering=False, debug=False,
                   enable_asserts=False, num_devices=n_cores)

    # ---------------- DRAM parameters ----------------
    xT_d = nc.dram_tensor("xT", [C, XROWS], F32R, kind="ExternalInput")
    lpos_d = nc.dram_tensor("lpos", [P, LT], F32, kind="ExternalInput")
    woT_d = nc.dram_tensor("woT", [C, 16], F32R, kind="ExternalInput")
    wob_d = nc.dram_tensor("wob", [1, 16], F32R, kind="ExternalInput")
    wog_d = nc.dram_tensor("wog", [P, 16], F32, kind="ExternalInput")
    kwT_d = nc.dram_tensor("kwT", [C, C], F32R, kind="ExternalInput")
    kb_d = nc.dram_tensor("kb", [1, C], F32R, kind="ExternalInput")
    kg_d = nc.dram_tensor("kg", [P, C], F32, kind="ExternalInput")
    vwT_d = nc.dram_tensor("vwT", [C, C], F32R, kind="ExternalInput")
    vb_d = nc.dram_tensor("vb", [1, C], F32R, kind="ExternalInput")
    sw1T_d = nc.dram_tensor("sw1T", [C, P], F32, kind="ExternalInput")   # pre-scaled by 1/L
    sw2T_d = nc.dram_tensor("sw2T", [P, C], BF16, kind="ExternalInput")
    owT_d = nc.dram_tensor("owT", [C, C], F32, kind="ExternalInput")
    ones_d = nc.dram_tensor("ones", [1, P], F32R, kind="ExternalInput")
    out_d = nc.dram_tensor("out", [LSH, C], F32, kind="ExternalOutput")

    with TileContext(nc) as tc:
        _build_body(nc, tc, dict(
            xT=xT_d, lpos=lpos_d, woT=woT_d, wob=wob_d, wog=wog_d,
            kwT=kwT_d, kb=kb_d, kg=kg_d, vwT=vwT_d, vb=vb_d,
            sw1T=sw1T_d, sw2T=sw2T_d, owT=owT_d, out=out_d, ones=ones_d,
        ), skip_cc=skip_cc)
    nc.compile()
    return nc


def _build_body(nc, tc, dd, skip_cc=False):
    import contextlib
    ctx = contextlib.ExitStack()
    with ctx:
        cst = ctx.enter_context(tc.tile_pool(name="cst", bufs=1))
        vsb = ctx.enter_context(tc.tile_pool(name="vsb", bufs=VT))
        wk = ctx.enter_context(tc.tile_pool(name="wk", bufs=2))
        atp = ctx.enter_context(tc.tile_pool(name="atp", bufs=4))
        outp = ctx.enter_context(tc.tile_pool(name="outp", bufs=1))
        ps_proj = ctx.enter_context(tc.tile_pool(name="ps_proj", bufs=2, space="PSUM"))
        ps_sm = ctx.enter_context(tc.tile_pool(name="ps_sm", bufs=2, space="PSUM"))
        ps_tr = ctx.enter_context(tc.tile_pool(name="ps_tr", bufs=2, space="PSUM"))
        ps_mm = ctx.enter_context(tc.tile_pool(name="ps_mm", bufs=2, space="PSUM"))
        dram = ctx.enter_context(tc.tile_pool(name="dram", bufs=1, space="DRAM"))

        # ---------------- constants & weights to SBUF ----------------
        xT = [cst.tile([P, XROWS], F32R, tag=f"xT{i}", name=f"xT{i}") for i in range(4)]
        for i in range(4):
            nc.sync.dma_start(xT[i][:], dd["xT"].ap()[i * P:(i + 1) * P, :])
        vwT = [cst.tile([P, C], F32R, tag=f"vwT{i}", name=f"vwT{i}") for i in range(4)]
        kwT = [cst.tile([P, C], F32R, tag=f"kwT{i}", name=f"kwT{i}") for i in range(4)]
        owT = [cst.tile([P, C], F32, tag=f"owT{i}", name=f"owT{i}") for i in range(4)]
        woT = [cst.tile([P, 16], F32R, tag=f"woT{i}", name=f"woT{i}") for i in range(4)]
        sw1T = [cst.tile([P, P], F32, tag=f"sw1T{i}", name=f"sw1T{i}") for i in range(4)]
        for i in range(4):
            sl = slice(i * P, (i + 1) * P)
            nc.sync.dma_start(vwT[i][:], dd["vwT"].ap()[sl, :])
            nc.sync.dma_start(kwT[i][:], dd["kwT"].ap()[sl, :])
            nc.sync.dma_start(owT[i][:], dd["owT"].ap()[sl, :])
            nc.sync.dma_start(woT[i][:], dd["woT"].ap()[sl, :])
            nc.sync.dma_start(sw1T[i][:], dd["sw1T"].ap()[sl, :])
        sw2T = cst.tile([P, C], BF16)
        nc.sync.dma_start(sw2T[:], dd["sw2T"].ap())
        wob = cst.tile([1, 16], F32R)
        kb = cst.tile([1, C], F32R)
        vb = cst.tile([1, C], F32R)
        wog = cst.tile([P, 16], F32)
        kg = cst.tile([P, C], F32)
        lpos = cst.tile([P, LT], F32)
        nc.sync.dma_start(wob[:], dd["wob"].ap())
        nc.sync.dma_start(kb[:], dd["kb"].ap())
        nc.sync.dma_start(vb[:], dd["vb"].ap())
        nc.sync.dma_start(wog[:], dd["wog"].ap())
        nc.sync.dma_start(kg[:], dd["kg"].ap())
        nc.sync.dma_start(lpos[:], dd["lpos"].ap())

        eps_t = cst.tile([P, 1], F32)
        nc.vector.memset(eps_t[:], EPS)
        ones1 = cst.tile([1, P], F32R)
        nc.sync.dma_start(ones1[:], dd["ones"].ap())
        identb = cst.tile([P, P], BF16)
        make_identity(nc, identb[:])

        # iotas
        iotaS = cst.tile([P, H, 13], F32)       # s value -6..6 per head
        it0 = cst.tile([P, H, 13], I32)
        nc.gpsimd.iota(it0[:], pattern=[[0, H], [1, 13]], base=-6, channel_multiplier=0)
        nc.vector.tensor_copy(iotaS[:], it0[:])
        iotaA6 = cst.tile([P, H, 6], F32)       # a = 1..6 per head
        it1 = cst.tile([P, H, 6], I32)
        nc.gpsimd.iota(it1[:], pattern=[[0, H], [1, 6]], base=1, channel_multiplier=0)
        nc.vector.tensor_copy(iotaA6[:], it1[:])
        iotaA7 = cst.tile([P, H, 7], F32)       # a = 0..6 per head
        it2 = cst.tile([P, H, 7], I32)
        nc.gpsimd.iota(it2[:], pattern=[[0, H], [1, 7]], base=0, channel_multiplier=0)
        nc.vector.tensor_copy(iotaA7[:], it2[:])
        # scatter index base: h*256 + lam + 46 + a  (a = s+6: 0..12)
        iotaIDX = cst.tile([P, H, 13], F32)
        it3 = cst.tile([P, H, 13], I32)
        nc.gpsimd.iota(it3[:], pattern=[[0, 2], [256, 4], [1, 13]], base=46, channel_multiplier=1)
        nc.vector.tensor_copy(iotaIDX[:], it3[:])

        # ---------------- v projection over halo (9 tiles) ----------------
        v_sb = []
        for t in range(VT):
            vp = ps_proj.tile([P, C], F32, tag="proj", name="vp")
            col = slice(t * P, (t + 1) * P)
            for kc in range(4):
                nc.tensor.matmul(vp[:], xT[kc][:, col],
                                 vwT[kc][:], start=(kc == 0), stop=False)
            nc.tensor.matmul(vp[:], ones1[:], vb[:],
                             start=False, stop=True)
            vt = vsb.tile([P, C], BF16, tag="v", name=f"v{t}")
            nc.scalar.copy(vt[:], vp[:])
            v_sb.append(vt)

        # vv = 0.001*v[L-2] + 0.999*v[L-1]  (local rows 1086, 1087 = tile 8, parts 62, 63)
        vv = cst.tile([1, C], BF16)
        vvf = cst.tile([1, C], F32)
        vt2 = cst.tile([1, C], BF16)
        vt3 = cst.tile([1, C], BF16)
        nc.sync.dma_start(vt2[:], v_sb[8][62:63, :])
        nc.sync.dma_start(vt3[:], v_sb[8][63:64, :])
        nc.vector.tensor_scalar(out=vvf[:], in0=vt2[:], scalar1=0.001,
                                scalar2=None, op0=OP.mult)
        nc.vector.scalar_tensor_tensor(out=vv[:], in0=vt3[:], scalar=0.999,
                                       in1=vvf[:], op0=OP.mult, op1=OP.add)

        # squeeze-excite accumulator
        seacc = cst.tile([P, 4], F32)
        nc.vector.memset(seacc[:], 0.0)

        outT_sb = [outp.tile([P, LSH], BF16, tag=f"outT{cc}", name=f"outT{cc}") for cc in range(4)]

        # ---------------- main per-l-tile loop ----------------
        for lt in range(LT):
            xcol = slice(HALO + lt * P, HALO + (lt + 1) * P)

            # window/offset projection [P, 16]
            wop = ps_sm.tile([P, 16], F32, tag="tiny", name="wop")
            for kc in range(4):
                nc.tensor.matmul(wop[:], xT[kc][:, xcol],
                                 woT[kc][:], start=(kc == 0), stop=False)
            nc.tensor.matmul(wop[:], ones1[:], wob[:],
                             start=False, stop=True)
            # kernel projection [P, 512]
            kp = ps_proj.tile([P, C], F32, tag="proj", name="kp")
            for kc in range(4):
                nc.tensor.matmul(kp[:], xT[kc][:, xcol],
                                 kwT[kc][:], start=(kc == 0), stop=False)
            nc.tensor.matmul(kp[:], ones1[:], kb[:],
                             start=False, stop=True)

            # --- rmsnorm window/offset, sigmoid/tanh ---
            wsq = wk.tile([P, 16], F32, tag="wsq")
            nc.scalar.activation(wsq[:], wop[:], ACT.Square)
            wss = wk.tile([P, 2], F32, tag="wss")
            nc.vector.tensor_reduce(out=wss[:], in_=wsq[:].rearrange("p (g h) -> p g h", g=2),
                                    axis=mybir.AxisListType.X, op=OP.add)
            wrstd = wk.tile([P, 2], F32, tag="wrstd")
            nc.scalar.activation(wrstd[:], wss[:], ACT.Sqrt, bias=eps_t[:], scale=1.0 / 8)
            nc.vector.reciprocal(wrstd[:], wrstd[:])
            won = wk.tile([P, 16], F32, tag="won")
            nc.vector.tensor_tensor(
                out=won[:].rearrange("p (g h) -> p g h", g=2),
                in0=wop[:].rearrange("p (g h) -> p g h", g=2),
                in1=_bcast(wrstd[:][:, :, None], [P, 2, 8]), op=OP.mult)
            nc.vector.tensor_tensor(out=won[:], in0=won[:], in1=wog[:], op=OP.mult)
            win_raw = wk.tile([P, H], F32, tag="win_raw")
            nc.scalar.activation(win_raw[:], won[:, 0:8], ACT.Sigmoid)
            cth = wk.tile([P, H], F32, tag="cth")
            nc.scalar.activation(cth[:], won[:, 8:16], ACT.Tanh)
            cc_ = wk.tile([P, H], F32, tag="cc_")
            nc.vector.tensor_scalar(out=cc_[:], in0=cth[:], scalar1=12.0, scalar2=None,
                                    op0=OP.mult)
            hwv = wk.tile([P, H], F32, tag="hwv")
            nc.vector.tensor_scalar(out=hwv[:], in0=win_raw[:], scalar1=5.5, scalar2=0.5,
                                    op0=OP.mult, op1=OP.add)
            tinv = wk.tile([P, H], F32, tag="tinv")
            nc.vector.reciprocal(tinv[:], hwv[:])

            # c0 = floor(c), phi = c - c0  (via +16 trunc with round-fix)
            cp16 = wk.tile([P, H], F32, tag="cp16")
            nc.vector.tensor_scalar(out=cp16[:], in0=cc_[:], scalar1=16.0, scalar2=None,
                                    op0=OP.add)
            ci = wk.tile([P, H], I32, tag="ci")
            nc.vector.tensor_copy(ci[:], cp16[:])
            cf = wk.tile([P, H], F32, tag="cf")
            nc.vector.tensor_copy(cf[:], ci[:])
            cgt = wk.tile([P, H], F32, tag="cgt")
            nc.vector.tensor_tensor(out=cgt[:], in0=cf[:], in1=cp16[:], op=OP.is_gt)
            c0p16 = wk.tile([P, H], F32, tag="c0p16")
            nc.vector.tensor_tensor(out=c0p16[:], in0=cf[:], in1=cgt[:], op=OP.subtract)
            phi = wk.tile([P, H], F32, tag="phi")
            nc.vector.tensor_tensor(out=phi[:], in0=cp16[:], in1=c0p16[:], op=OP.subtract)
            c012 = wk.tile([P, H], F32, tag="c012")   # c0 + 12
            nc.vector.tensor_scalar(out=c012[:], in0=c0p16[:], scalar1=4.0, scalar2=None,
                                    op0=OP.subtract)

            # --- rmsnorm kernel + silu ---
            ksq = wk.tile([P, C], F32, tag="ksq")
            nc.scalar.activation(ksq[:], kp[:], ACT.Square)
            kss = wk.tile([P, 1], F32, tag="kss")
            nc.vector.tensor_reduce(out=kss[:], in_=ksq[:], axis=mybir.AxisListType.X,
                                    op=OP.add)
            krstd = wk.tile([P, 1], F32, tag="krstd")
            nc.scalar.activation(krstd[:], kss[:], ACT.Sqrt, bias=eps_t[:], scale=1.0 / C)
            nc.vector.reciprocal(krstd[:], krstd[:])
            kn = wk.tile([P, C], F32, tag="kn")
            nc.vector.tensor_scalar(out=kn[:], in0=kp[:], scalar1=krstd[:], scalar2=None,
                                    op0=OP.mult)
            nc.vector.tensor_tensor(out=kn[:], in0=kn[:], in1=kg[:], op=OP.mult)
            kern = wk.tile([P, H, K], BF16, tag="kern")
            ksg = wk.tile([P, C], F32, tag="ksg")
            nc.scalar.activation(ksg[:], kn[:], ACT.Sigmoid)
            nc.vector.tensor_tensor(out=kern[:].rearrange("p h k -> p (h k)"), in0=kn[:],
                                    in1=ksg[:], op=OP.mult)

            # D table: Dt[k] = kern[k+1] - kern[k], Dt[63] = 0
            Dt = wk.tile([P, H, K], BF16, tag="Dt")
            nc.vector.memset(Dt[:, :, 63:64], 0.0)
            nc.vector.tensor_tensor(out=Dt[:, :, 0:63], in0=kern[:, :, 1:64],
                                    in1=kern[:, :, 0:63], op=OP.subtract)

            # --- interpolation indices ---
            rel6 = wk.tile([P, H, 6], F32, tag="rel6")
            nc.vector.tensor_tensor(out=rel6[:], in0=iotaA6[:],
                                    in1=_bcast(tinv[:][:, :, None], [P, H, 6]), op=OP.mult)
            npos = wk.tile([P, H, 6], F32, tag="npos")
            nc.vector.tensor_scalar(out=npos[:], in0=rel6[:], scalar1=1.0, scalar2=float(K - 1),
                                    op0=OP.min, op1=OP.mult)
            ii = wk.tile([P, H, 6], I32, tag="ii")
            nc.vector.tensor_copy(ii[:], npos[:])
            tf = wk.tile([P, H, 6], F32, tag="tf")
            nc.vector.tensor_copy(tf[:], ii[:])
            tgt = wk.tile([P, H, 6], F32, tag="tgt")
            nc.vector.tensor_tensor(out=tgt[:], in0=tf[:], in1=npos[:], op=OP.is_gt)
            idxf = wk.tile([P, H, 6], F32, tag="idxf")
            nc.vector.tensor_tensor(out=idxf[:], in0=tf[:], in1=tgt[:], op=OP.subtract)
            nc.vector.tensor_scalar(out=idxf[:], in0=idxf[:], scalar1=float(K - 2),
                                    scalar2=None, op0=OP.min)
            w_c = wk.tile([P, H, 6], F32, tag="w_c")
            nc.vector.tensor_tensor(out=w_c[:], in0=npos[:], in1=idxf[:], op=OP.subtract)

            # bits of idxf (f32 0/1 + int16 mask copies), msb first
            bits = []
            rcur = idxf
            for j, bv in enumerate([32.0, 16.0, 8.0, 4.0, 2.0, 1.0]):
                bj = wk.tile([P, H, 6], F32, tag=f"bit{j}")
                nc.vector.tensor_scalar(out=bj[:], in0=rcur[:], scalar1=bv, scalar2=None,
                                        op0=OP.is_ge)
                bi = wk.tile([P, H, 6], I16, tag=f"biti{j}")
                nc.vector.tensor_copy(bi[:], bj[:])
                bits.append(bi)
                if j < 5:
                    rnew = wk.tile([P, H, 6], F32, tag=f"rem{j}")
                    nc.vector.scalar_tensor_tensor(out=rnew[:], in0=bj[:], scalar=-bv,
                                                   in1=rcur[:], op0=OP.mult, op1=OP.add)
                    rcur = rnew

            # --- halving gather of (kern, Dt) pairs at idxf ---
            st = wk.tile([P, H, 6, 2, 32], BF16, tag="st")
            nc.vector.tensor_copy(st[:, :, :, 0, :],
                                  _bcast(kern[:][:, :, None, 0:32], [P, H, 6, 32]))
            nc.vector.tensor_copy(st[:, :, :, 1, :],
                                  _bcast(Dt[:][:, :, None, 0:32], [P, H, 6, 32]))
            nc.vector.copy_predicated(st[:, :, :, 0, :],
                                      _bcast(bits[0][:][:, :, :, None], [P, H, 6, 32]),
                                      _bcast(kern[:][:, :, None, 32:64], [P, H, 6, 32]))
            nc.vector.copy_predicated(st[:, :, :, 1, :],
                                      _bcast(bits[0][:][:, :, :, None], [P, H, 6, 32]),
                                      _bcast(Dt[:][:, :, None, 32:64], [P, H, 6, 32]))
            w = 16
            for j in range(1, 6):
                nc.vector.copy_predicated(
                    st[:, :, :, :, 0:w],
                    _bcast(bits[j][:][:, :, :, None, None], [P, H, 6, 2, w]),
                    st[:, :, :, :, w:2 * w])
                w //= 2
            # (sim note: copy_predicated views need the ravel shim in test_sim)
            # g0 = st[...,0,0], g1 = st[...,1,0]
            lerp = wk.tile([P, H, 6], F32, tag="lerp")
            nc.vector.tensor_tensor(out=lerp[:], in0=w_c[:], in1=st[:, :, :, 1, 0],
                                    op=OP.mult)
            nc.vector.tensor_tensor(out=lerp[:], in0=lerp[:], in1=st[:, :, :, 0, 0],
                                    op=OP.add)
            # ker7 = 1 + max(lerp, 0); col 0 from kern[...,0]
            ker7 = wk.tile([P, H, 7], F32, tag="ker7")
            nc.vector.tensor_scalar(out=ker7[:, :, 1:7], in0=lerp[:], scalar1=0.0,
                                    scalar2=1.0, op0=OP.max, op1=OP.add)
            nc.vector.tensor_scalar(out=ker7[:, :, 0:1], in0=kern[:, :, 0:1], scalar1=0.0,
                                    scalar2=1.0, op0=OP.max, op1=OP.add)

            # win7 = exp(-(a * tinv)^2)
            rel7 = wk.tile([P, H, 7], F32, tag="rel7")
            nc.vector.tensor_tensor(out=rel7[:], in0=iotaA7[:],
                                    in1=_bcast(tinv[:][:, :, None], [P, H, 7]), op=OP.mult)
            nc.vector.tensor_tensor(out=rel7[:], in0=rel7[:], in1=rel7[:], op=OP.mult)
            win7 = wk.tile([P, H, 7], F32, tag="win7")
            nc.scalar.activation(win7[:], rel7[:], ACT.Exp, scale=-1.0)
            wt7 = wk.tile([P, H, 7], F32, tag="wt7")
            nc.vector.tensor_tensor(out=wt7[:], in0=ker7[:], in1=win7[:], op=OP.mult)

            # expand to 13 taps (s order -6..6)
            w13 = wk.tile([P, H, 13], F32, tag="w13")
            for a in range(6):
                nc.vector.tensor_copy(w13[:, :, a:a + 1], wt7[:, :, 6 - a:7 - a])
            nc.vector.tensor_copy(w13[:, :, 6:13], wt7[:, :, 0:7])

            # validity / special masks
            nb13 = wk.tile([P, H, 13], F32, tag="nb13")
            lc = wk.tile([P, H], F32, tag="lc")
            nc.vector.tensor_tensor(out=lc[:], in0=cc_[:],
                                    in1=_bcast(lpos[:, lt:lt + 1], [P, H]), op=OP.add)
            nc.vector.tensor_tensor(out=nb13[:], in0=iotaS[:],
                                    in1=_bcast(lc[:][:, :, None], [P, H, 13]), op=OP.add)
            vlo = wk.tile([P, H, 13], F32, tag="vlo")
            nc.vector.tensor_scalar(out=vlo[:], in0=nb13[:], scalar1=0.0, scalar2=None,
                                    op0=OP.is_ge)
            vhi = wk.tile([P, H, 13], F32, tag="vhi")
            nc.vector.tensor_scalar(out=vhi[:], in0=nb13[:], scalar1=float(L), scalar2=None,
                                    op0=OP.is_lt)
            valid = wk.tile([P, H, 13], F32, tag="valid")
            nc.vector.tensor_tensor(out=valid[:], in0=vlo[:], in1=vhi[:], op=OP.mult)
            sp1 = wk.tile([P, H, 13], F32, tag="sp1")
            nc.vector.tensor_scalar(out=sp1[:], in0=nb13[:], scalar1=float(NB_HI),
                                    scalar2=None, op0=OP.is_gt)
            spec = wk.tile([P, H, 13], F32, tag="spec")
            nc.vector.tensor_tensor(out=spec[:], in0=sp1[:], in1=vhi[:], op=OP.mult)

            wv = wk.tile([P, H, 13], F32, tag="wv")
            nc.vector.tensor_tensor(out=wv[:], in0=w13[:], in1=valid[:], op=OP.mult)
            wsum = wk.tile([P, H], F32, tag="wsum")
            nc.vector.tensor_reduce(out=wsum[:], in_=wv[:], axis=mybir.AxisListType.X,
                                    op=OP.add)
            rw = wk.tile([P, H], F32, tag="rw")
            nc.vector.tensor_scalar(out=rw[:], in0=wsum[:], scalar1=1.0, scalar2=None,
                                    op0=OP.max)
            nc.vector.reciprocal(rw[:], rw[:])

            wsp = wk.tile([P, H, 13], F32, tag="wsp")
            nc.vector.tensor_tensor(out=wsp[:], in0=wv[:], in1=spec[:], op=OP.mult)
            wint = wk.tile([P, H, 13], F32, tag="wint")
            nc.vector.tensor_tensor(out=wint[:], in0=wv[:], in1=wsp[:], op=OP.subtract)
            wspec = wk.tile([P, H], F32, tag="wspec")
            nc.vector.tensor_reduce(out=wspec[:], in_=wsp[:], axis=mybir.AxisListType.X,
                                    op=OP.add)
            wspec_s = wk.tile([P, H], F32, tag="wspec_s")
            nc.vector.tensor_tensor(out=wspec_s[:], in0=wspec[:], in1=rw[:], op=OP.mult)

            om = wk.tile([P, H], F32, tag="om")
            nc.vector.tensor_scalar(out=om[:], in0=phi[:], scalar1=-1.0, scalar2=1.0,
                                    op0=OP.mult, op1=OP.add)
            uf = wk.tile([P, H], F32, tag="uf")
            nc.vector.tensor_tensor(out=uf[:], in0=om[:], in1=rw[:], op=OP.mult)
            uc = wk.tile([P, H], F32, tag="uc")
            nc.vector.tensor_tensor(out=uc[:], in0=phi[:], in1=rw[:], op=OP.mult)

            wf = wk.tile([P, H, 13], BF16, tag="wf")
            nc.vector.tensor_tensor(out=wf[:], in0=wint[:],
                                    in1=_bcast(uf[:][:, :, None], [P, H, 13]), op=OP.mult)
            wcc = wk.tile([P, H, 13], BF16, tag="wcc")
            nc.vector.tensor_tensor(out=wcc[:], in0=wint[:],
                                    in1=_bcast(uc[:][:, :, None], [P, H, 13]), op=OP.mult)

            # scatter indices
            idxf32 = wk.tile([P, H, 13], F32, tag="idxf32")
            nc.vector.tensor_tensor(out=idxf32[:], in0=iotaIDX[:],
                                    in1=_bcast(c012[:][:, :, None], [P, H, 13]), op=OP.add)
            i16 = wk.tile([P, H, 13], I16, tag="i16")
            nc.vector.tensor_copy(i16[:], idxf32[:])
            i16b = wk.tile([P, H, 13], I16, tag="i16b")
            nc.vector.tensor_scalar(out=i16b[:], in0=i16[:], scalar1=1, scalar2=None,
                                    op0=OP.add)

            A0 = wk.tile([P, H, 256], BF16, tag="A0")
            A1 = wk.tile([P, H, 256], BF16, tag="A1")
            for hb in range(2):
                hs = slice(hb * 4, hb * 4 + 4)
                nc.gpsimd.local_scatter(
                    A0[:, hs, :].rearrange("p h w -> p (h w)"),
                    wf[:, hs, :].rearrange("p h a -> p (h a)"),
                    i16[:, hs, :].rearrange("p h a -> p (h a)"),
                    channels=P, num_elems=4 * 256, num_idxs=4 * 13)
                nc.gpsimd.local_scatter(
                    A1[:, hs, :].rearrange("p h w -> p (h w)"),
                    wcc[:, hs, :].rearrange("p h a -> p (h a)"),
                    i16b[:, hs, :].rearrange("p h a -> p (h a)"),
                    channels=P, num_elems=4 * 256, num_idxs=4 * 13)
            A = wk.tile([P, H, 256], BF16, tag="A")
            nc.vector.tensor_tensor(out=A[:], in0=A0[:], in1=A1[:], op=OP.add)

            # wspecT rows (for the rank-1 edge correction): 8 x [1, P]
            wspb = wk.tile([P, H], BF16, tag="wspb")
            nc.vector.tensor_copy(wspb[:], wspec_s[:])
            wspT = wk.tile([1, H, P], BF16, tag="wspT_sb")
            for h_ in range(H):
                wsp_ps = ps_sm.tile([1, P], BF16, tag="tiny", name="wsp_ps")
                nc.tensor.transpose(wsp_ps[:], wspb[:, h_:h_ + 1], identb[:])
                nc.scalar.copy(wspT[0:1, h_, :], wsp_ps[:])

            # --- transpose A blocks and banded matmul ---
            for cci in range(4):
                po = ps_mm.tile([P, P], F32, tag="mm", name="po")
                for hh in range(2):
                    h = 2 * cci + hh
                    prange = slice(hh * 64, hh * 64 + 64)
                    for blk in range(2):
                        tp = ps_tr.tile([P, P], BF16, tag="tp")
                        nc.tensor.transpose(tp[:], A[:, h, blk * P:(blk + 1) * P], identb[:])
                        at = atp.tile([P, P], BF16, tag="at")
                        if (h + blk) % 2 == 0:
                            nc.scalar.copy(at[:], tp[:])
                        else:
                            nc.vector.tensor_copy(at[:], tp[:])
                        nc.tensor.matmul(po[prange, :],
                                         v_sb[lt + blk][:, h * D:(h + 1) * D],
                                         at[:], start=(blk == 0), stop=False)
                    nc.tensor.matmul(po[prange, :], vv[:, h * D:(h + 1) * D],
                                     wspT[0:1, h, :], start=False, stop=True)
                # SE partial sum + copy out
                red = wk.tile([P, 1], F32, tag="red")
                nc.vector.tensor_reduce(out=red[:], in_=po[:], axis=mybir.AxisListType.X,
                                        op=OP.add)
                nc.vector.tensor_tensor(out=seacc[:, cci:cci + 1], in0=seacc[:, cci:cci + 1],
                                        in1=red[:], op=OP.add)
                nc.scalar.copy(outT_sb[cci][:, lt * P:(lt + 1) * P], po[:])

        # ---------------- AllReduce of SE partial means ----------------
        if skip_cc:
            armean = seacc
        else:
            cci_d = dram.tile([P, 4], F32)
            cco_d = dram.tile([P, 4], F32)
            nc.gpsimd.dma_start(cci_d[:], seacc[:])
            nc.gpsimd.collective_compute(
                "AllReduce", OP.add,
                replica_groups=[[0, 1, 2, 3], [4, 5, 6, 7]],
                ins=[cci_d[:].opt()], outs=[cco_d[:].opt()])
            armean = cst.tile([P, 4], F32)
            nc.gpsimd.dma_start(armean[:], cco_d[:])

        # ---------------- SE MLP ----------------
        hid_ps = ps_sm.tile([1, P], F32, tag="tiny", name="hid_ps")
        for cci in range(4):
            nc.tensor.matmul(hid_ps[:], armean[:, cci:cci + 1], sw1T[cci][:],
                             start=(cci == 0), stop=(cci == 3))
        hid = cst.tile([1, P], BF16)
        hsg = cst.tile([1, P], F32)
        nc.scalar.activation(hsg[:], hid_ps[:], ACT.Sigmoid)
        nc.vector.tensor_tensor(out=hid[:], in0=hid_ps[:], in1=hsg[:], op=OP.mult)
        hidT_ps = ps_sm.tile([P, 1], BF16, tag="tiny", name="hidT_ps")
        nc.tensor.transpose(hidT_ps[:], hid[:], identb[0:1, 0:1])
        hidT = cst.tile([P, 1], BF16)
        nc.scalar.copy(hidT[:], hidT_ps[:])
        sc_ps = ps_sm.tile([1, C], F32, tag="tiny", name="sc_ps")
        nc.tensor.matmul(sc_ps[:], hidT[:], sw2T[:], start=True, stop=True)
        scrow = cst.tile([1, C], BF16)
        nc.scalar.activation(scrow[:], sc_ps[:], ACT.Sigmoid)
        owb = []
        for cci in range(4):
            scT_ps = ps_sm.tile([P, 1], BF16, tag="tiny", name="scT_ps")
            nc.tensor.transpose(scT_ps[:], scrow[:, cci * P:(cci + 1) * P], identb[0:1, 0:1])
            scT = cst.tile([P, 1], F32, tag=f"scT{cci}", name=f"scT{cci}")
            nc.scalar.copy(scT[:], scT_ps[:])
            ow = cst.tile([P, C], BF16, tag=f"owb{cci}", name=f"owb{cci}")
            nc.vector.tensor_scalar(out=ow[:], in0=owT[cci][:], scalar1=scT[:],
                                    scalar2=None, op0=OP.mult)
            owb.append(ow)

        # ---------------- final out_w matmul + silu + DMA out ----------------
        for lt in range(LT):
            lsl = slice(lt * P, (lt + 1) * P)
            for co in range(4):
                pf_ = ps_mm.tile([P, P], F32, tag="mm", name="pf_")
                for cci in range(4):
                    nc.tensor.matmul(pf_[:], owb[cci][:, co * P:(co + 1) * P],
                                     outT_sb[cci][:, lsl], start=(cci == 0), stop=(cci == 3))
                fo = wk.tile([P, P], F32, tag="fo")
                nc.scalar.activation(fo[:], pf_[:], ACT.Sigmoid)
                nc.vector.tensor_tensor(out=fo[:], in0=fo[:], in1=pf_[:], op=OP.mult)
                nc.sync.dma_start(
                    dd["out"].ap()[lsl, co * P:(co + 1) * P].rearrange("l c -> c l"),
                    fo[:])


def make_in_maps(inputs, n_cores=8):
    x = np.ascontiguousarray(inputs["x"], dtype=np.float32)
    window_w = inputs["window_w"]; window_b = inputs["window_b"]
    window_gamma = inputs["window_gamma"]
    offset_w = inputs["offset_w"]; offset_b = inputs["offset_b"]
    offset_gamma = inputs["offset_gamma"]
    kernel_w = inputs["kernel_w"]; kernel_b = inputs["kernel_b"]
    kernel_gamma = inputs["kernel_gamma"]
    v_w = inputs["v_w"]; v_b = inputs["v_b"]
    se_w1 = inputs["se_w1"]; se_w2 = inputs["se_w2"]; out_w = inputs["out_w"]

    woT = np.concatenate([window_w, offset_w], 0).T.astype(np.float32)      # (512,16)
    wob = np.concatenate([window_b, offset_b])[None].astype(np.float32)     # (1,16)
    wog = np.tile(np.concatenate([window_gamma, offset_gamma])[None], (P, 1)).astype(np.float32)
    kwT = np.ascontiguousarray(kernel_w.T, np.float32)
    kb = kernel_b[None].astype(np.float32)
    kgm = np.tile(kernel_gamma[None], (P, 1)).astype(np.float32)
    vwT = np.ascontiguousarray(v_w.T, np.float32)
    vbm = v_b[None].astype(np.float32)
    sw1T = np.ascontiguousarray(se_w1.T, np.float32) / np.float32(L)
    sw2T = np.ascontiguousarray(se_w2.T).astype(ml_dtypes.bfloat16)
    owT = np.ascontiguousarray(out_w.T, np.float32)

    in_maps = []
    for i in range(n_cores):
        b, q = divmod(i, 4)
        lo = q * LSH - HALO
        xpad = np.zeros((XROWS, C), np.float32)
        s0, s1 = max(lo, 0), min(lo + XROWS, L)
        xpad[s0 - lo:s1 - lo] = x[b, s0:s1]
        xT = np.ascontiguousarray(xpad.T)
        lpos = (q * LSH + np.arange(LSH, dtype=np.float32)).reshape(LT, P).T.copy()
        in_maps.append(dict(
            xT=xT, lpos=lpos, woT=woT, wob=wob, wog=wog, kwT=kwT, kb=kb,
            kg=kgm, vwT=vwT, vb=vbm, sw1T=sw1T, sw2T=sw2T, owT=owT,
            ones=np.ones((1, P), np.float32),
        ))
    return in_maps


def kernel(**inputs) -> np.ndarray:
    if "graph" not in _GRAPH_CACHE:
        _GRAPH_CACHE["graph"] = build_graph(8)
    nc = _GRAPH_CACHE["graph"]
    in_maps = make_in_maps(inputs, 8)
    res = run_bass_kernel_spmd(nc, in_maps, core_ids=list(range(8)))
    out = np.zeros((B, L, C), np.float32)
    for i in range(8):
        b, q = divmod(i, 4)
        out[b, q * LSH:(q + 1) * LSH] = res.results[i]["out"]
    return out


if __name__ == "__main__":
    import reference
    inputs = {k: np.asarray(v) for k, v in reference.setup_inputs().items()}
    got = kernel(**inputs)
    import jax.numpy as jnp
    exp = np.asarray(reference.reference(**{k: jnp.asarray(v) for k, v in inputs.items()}))
    rel = np.linalg.norm(got - exp) / np.linalg.norm(exp)
    print("Relative error:", rel)


# revision 22
# speedup vs baseline: 16.8686x; 16.8686x over previous
"""AdaptiveLocalConv Trainium2 kernel — 8-core SPMD.

Sharding: (batch, seq-quarter) -> 8 shards of 1024 tokens (+64 halo each side
for the deformable gather, reach <= +-19).

Per-core pipeline:
  - 4 projections from x via PE (f32r), x passed pre-transposed [C, 1152].
  - per-(token,head): 13 deformable taps; kernel-table interpolation via a
    log2 halving-gather on DVE; taps placed into a banded matrix A
    [token, head, 256-slot J-band] with gpsimd local_scatter (bf16).
  - banded matmul out^T[d, l] = sum_J v[J, d] * A^T[J, l] on PE after
    PE-transposing A blocks; sequence-end clamp handled exactly by a rank-1
    correction matmul.
  - squeeze-excite via a 4-core AllReduce of the per-core partial mean (2KB),
    SE scale folded into out_w columns; final out_w matmul in transposed
    layout; silu; DMA out with a transposing access pattern.
"""
import sys
if "/opt/trn_rl_repo" not in sys.path:
    sys.path.insert(0, "/opt/trn_rl_repo")

import numpy as np
import ml_dtypes

import concourse.bass as bass
import concourse.mybir as mybir
from concourse import bacc
from concourse.tile import TileContext
from concourse.bass_utils import run_bass_kernel_spmd
from concourse.masks import make_identity

F32 = mybir.dt.float32
F32R = mybir.dt.float32r
BF16 = mybir.dt.bfloat16
I32 = mybir.dt.int32
I16 = mybir.dt.int16
OP = mybir.AluOpType
ACT = mybir.ActivationFunctionType

B, L, C, H, K, D = 2, 4096, 512, 8, 64, 64
P = 128
HALO = 64
LSH = 1024          # tokens per core
XROWS = LSH + 2 * HALO   # 1152
LT = LSH // P       # 8 own l-tiles
VT = XROWS // P     # 9 v tiles
NB_HI = np.float32(L - 1.001)
EPS = 1e-6

_GRAPH_CACHE = {}


def _bcast(ap, shape):
    return ap.broadcast_to(shape)


def build_graph(n_cores=8, skip_cc=False):
    nc = bacc.Bacc("TRN2", target_bir_lowering=False, debug=False,
                   enable_asserts=False, num_devices=n_cores)

    # ---------------- DRAM parameters ----------------
    xT_d = nc.dram_tensor("xT", [C, XROWS], F32R, kind="ExternalInput")
    lpos_d = nc.dram_tensor("lpos", [P, LT], F32, kind="ExternalInput")
    woT_d = nc.dram_tensor("woT", [C, 16], F32R, kind="ExternalInput")
    wob_d = nc.dram_tensor("wob", [1, 16], F32R, kind="ExternalInput")
    wog_d = nc.dram_tensor("wog", [P, 16], F32, kind="ExternalInput")
    kwT_d = nc.dram_tensor("kwT", [C, C], F32R, kind="ExternalInput")
    kb_d = nc.dram_tensor("kb", [1, C], F32R, kind="ExternalInput")
    kg_d = nc.dram_tensor("kg", [P, C], F32, kind="ExternalInput")
    vwT_d = nc.dram_tensor("vwT", [C, C], F32R, kind="ExternalInput")
    vb_d = nc.dram_tensor("vb", [1, C], F32R, kind="ExternalInput")
    sw1T_d = nc.dram_tensor("sw1T", [C, P], F32, kind="ExternalInput")   # pre-scaled by 1/L
    sw2T_d = nc.dram_tensor("sw2T", [P, C], BF16, kind="ExternalInput")
    owT_d = nc.dram_tensor("owT", [C, C], F32, kind="ExternalInput")
    ones_d = nc.dram_tensor("ones", [1, P], F32R, kind="ExternalInput")
    out_d = nc.dram_tensor("out", [LSH, C], F32, kind="ExternalOutput")

    with TileContext(nc) as tc:
        _build_body(nc, tc, dict(
            xT=xT_d, lpos=lpos_d, woT=woT_d, wob=wob_d, wog=wog_d,
            kwT=kwT_d, kb=kb_d, kg=kg_d, vwT=vwT_d, vb=vb_d,
            sw1T=sw1T_d, sw2T=sw2T_d, owT=owT_d, out=out_d, ones=ones_d,
        ), skip_cc=skip_cc)
    nc.compile()
    return nc


def _build_body(nc, tc, dd, skip_cc=False):
    import contextlib
    ctx = contextlib.ExitStack()
    with ctx:
        cst = ctx.enter_context(tc.tile_pool(name="cst", bufs=1))
        vsb = ctx.enter_context(tc.tile_pool(name="vsb", bufs=VT))
        wk = ctx.enter_context(tc.tile_pool(name="wk", bufs=2))
        atp = ctx.enter_context(tc.tile_pool(name="atp", bufs=4))
        outp = ctx.enter_context(tc.tile_pool(name="outp", bufs=1))
        ps_proj = ctx.enter_context(tc.tile_pool(name="ps_proj", bufs=2, space="PSUM"))
        ps_sm = ctx.enter_context(tc.tile_pool(name="ps_sm", bufs=2, space="PSUM"))
        ps_tr = ctx.enter_context(tc.tile_pool(name="ps_tr", bufs=2, space="PSUM"))
        ps_mm = ctx.enter_context(tc.tile_pool(name="ps_mm", bufs=2, space="PSUM"))
        dram = ctx.enter_context(tc.tile_pool(name="dram", bufs=1, space="DRAM"))

        # ---------------- constants & weights to SBUF ----------------
        xT = [cst.tile([P, XROWS], F32R, tag=f"xT{i}", name=f"xT{i}") for i in range(4)]
        for i in range(4):
            nc.sync.dma_start(xT[i][:], dd["xT"].ap()[i * P:(i + 1) * P, :])
        vwT = [cst.tile([P, C], F32R, tag=f"vwT{i}", name=f"vwT{i}") for i in range(4)]
        kwT = [cst.tile([P, C], F32R, tag=f"kwT{i}", name=f"kwT{i}") for i in range(4)]
        owT = [cst.tile([P, C], F32, tag=f"owT{i}", name=f"owT{i}") for i in range(4)]
        woT = [cst.tile([P, 16], F32R, tag=f"woT{i}", name=f"woT{i}") for i in range(4)]
        sw1T = [cst.tile([P, P], F32, tag=f"sw1T{i}", name=f"sw1T{i}") for i in range(4)]
        for i in range(4):
            sl = slice(i * P, (i + 1) * P)
            nc.sync.dma_start(vwT[i][:], dd["vwT"].ap()[sl, :])
            nc.sync.dma_start(kwT[i][:], dd["kwT"].ap()[sl, :])
            nc.sync.dma_start(owT[i][:], dd["owT"].ap()[sl, :])
            nc.sync.dma_start(woT[i][:], dd["woT"].ap()[sl, :])
            nc.sync.dma_start(sw1T[i][:], dd["sw1T"].ap()[sl, :])
        sw2T = cst.tile([P, C], BF16)
        nc.sync.dma_start(sw2T[:], dd["sw2T"].ap())
        wob = cst.tile([1, 16], F32R)
        kb = cst.tile([1, C], F32R)
        vb = cst.tile([1, C], F32R)
        wog = cst.tile([P, 16], F32)
        kg = cst.tile([P, C], F32)
        lpos = cst.tile([P, LT], F32)
        nc.sync.dma_start(wob[:], dd["wob"].ap())
        nc.sync.dma_start(kb[:], dd["kb"].ap())
        nc.sync.dma_start(vb[:], dd["vb"].ap())
        nc.sync.dma_start(wog[:], dd["wog"].ap())
        nc.sync.dma_start(kg[:], dd["kg"].ap())
        nc.sync.dma_start(lpos[:], dd["lpos"].ap())

        eps_t = cst.tile([P, 1], F32)
        nc.vector.memset(eps_t[:], EPS)
        ones1 = cst.tile([1, P], F32R)
        nc.sync.dma_start(ones1[:], dd["ones"].ap())
        identb = cst.tile([P, P], BF16)
        make_identity(nc, identb[:])

        # iotas
        iotaS = cst.tile([P, H, 13], F32)       # s value -6..6 per head
        it0 = cst.tile([P, H, 13], I32)
        nc.gpsimd.iota(it0[:], pattern=[[0, H], [1, 13]], base=-6, channel_multiplier=0)
        nc.vector.tensor_copy(iotaS[:], it0[:])
        iotaA6 = cst.tile([P, H, 6], F32)       # a = 1..6 per head
        it1 = cst.tile([P, H, 6], I32)
        nc.gpsimd.iota(it1[:], pattern=[[0, H], [1, 6]], base=1, channel_multiplier=0)
        nc.vector.tensor_copy(iotaA6[:], it1[:])
        iotaA7 = cst.tile([P, H, 7], F32)       # a = 0..6 per head
        it2 = cst.tile([P, H, 7], I32)
        nc.gpsimd.iota(it2[:], pattern=[[0, H], [1, 7]], base=0, channel_multiplier=0)
        nc.vector.tensor_copy(iotaA7[:], it2[:])
        # scatter index base: h*256 + lam + 46 + a  (a = s+6: 0..12)
        iotaIDX = cst.tile([P, H, 13], F32)
        it3 = cst.tile([P, H, 13], I32)
        nc.gpsimd.iota(it3[:], pattern=[[0, 2], [256, 4], [1, 13]], base=46, channel_multiplier=1)
        nc.vector.tensor_copy(iotaIDX[:], it3[:])

        # ---------------- v projection over halo (9 tiles) ----------------
        v_sb = []
        for t in range(VT):
            vp = ps_proj.tile([P, C], F32, tag="proj", name="vp")
            col = slice(t * P, (t + 1) * P)
            for kc in range(4):
                nc.tensor.matmul(vp[:], xT[kc][:, col],
                                 vwT[kc][:], start=(kc == 0), stop=False)
            nc.tensor.matmul(vp[:], ones1[:], vb[:],
                             start=False, stop=True)
            vt = vsb.tile([P, C], BF16, tag="v", name=f"v{t}")
            nc.scalar.copy(vt[:], vp[:])
            v_sb.append(vt)

        # vv = 0.001*v[L-2] + 0.999*v[L-1]  (local rows 1086, 1087 = tile 8, parts 62, 63)
        vv = cst.tile([1, C], BF16)
        vvf = cst.tile([1, C], F32)
        vt2 = cst.tile([1, C], BF16)
        vt3 = cst.tile([1, C], BF16)
        nc.sync.dma_start(vt2[:], v_sb[8][62:63, :])
        nc.sync.dma_start(vt3[:], v_sb[8][63:64, :])
        nc.vector.tensor_scalar(out=vvf[:], in0=vt2[:], scalar1=0.001,
                                scalar2=None, op0=OP.mult)
        nc.vector.scalar_tensor_tensor(out=vv[:], in0=vt3[:], scalar=0.999,
                                       in1=vvf[:], op0=OP.mult, op1=OP.add)

        # squeeze-excite accumulator
        seacc = cst.tile([P, 4], F32)
        nc.vector.memset(seacc[:], 0.0)

        outT_sb = [outp.tile([P, LSH], BF16, tag=f"outT{cc}", name=f"outT{cc}") for cc in range(4)]

        # ---------------- main per-l-tile loop ----------------
        for lt in range(LT):
            xcol = slice(HALO + lt * P, HALO + (lt + 1) * P)

            # window/offset projection [P, 16]
            wop = ps_sm.tile([P, 16], F32, tag="tiny", name="wop")
            for kc in range(4):
                nc.tensor.matmul(wop[:], xT[kc][:, xcol],
                                 woT[kc][:], start=(kc == 0), stop=False)
            nc.tensor.matmul(wop[:], ones1[:], wob[:],
                             start=False, stop=True)
            # kernel projection [P, 512]
            kp = ps_proj.tile([P, C], F32, tag="proj", name="kp")
            for kc in range(4):
                nc.tensor.matmul(kp[:], xT[kc][:, xcol],
                                 kwT[kc][:], start=(kc == 0), stop=False)
            nc.tensor.matmul(kp[:], ones1[:], kb[:],
                             start=False, stop=True)

            # --- rmsnorm window/offset, sigmoid/tanh ---
            wsq = wk.tile([P, 16], F32, tag="wsq")
            nc.scalar.activation(wsq[:], wop[:], ACT.Square)
            wss = wk.tile([P, 2], F32, tag="wss")
            nc.vector.tensor_reduce(out=wss[:], in_=wsq[:].rearrange("p (g h) -> p g h", g=2),
                                    axis=mybir.AxisListType.X, op=OP.add)
            wrstd = wk.tile([P, 2], F32, tag="wrstd")
            nc.scalar.activation(wrstd[:], wss[:], ACT.Sqrt, bias=eps_t[:], scale=1.0 / 8)
            nc.vector.reciprocal(wrstd[:], wrstd[:])
            won = wk.tile([P, 16], F32, tag="won")
            nc.vector.tensor_tensor(
                out=won[:].rearrange("p (g h) -> p g h", g=2),
                in0=wop[:].rearrange("p (g h) -> p g h", g=2),
                in1=_bcast(wrstd[:][:, :, None], [P, 2, 8]), op=OP.mult)
            nc.vector.tensor_tensor(out=won[:], in0=won[:], in1=wog[:], op=OP.mult)
            win_raw = wk.tile([P, H], F32, tag="win_raw")
            nc.scalar.activation(win_raw[:], won[:, 0:8], ACT.Sigmoid)
            cth = wk.tile([P, H], F32, tag="cth")
            nc.scalar.activation(cth[:], won[:, 8:16], ACT.Tanh)
            cc_ = wk.tile([P, H], F32, tag="cc_")
            nc.vector.tensor_scalar(out=cc_[:], in0=cth[:], scalar1=12.0, scalar2=None,
                                    op0=OP.mult)
            hwv = wk.tile([P, H], F32, tag="hwv")
            nc.vector.tensor_scalar(out=hwv[:], in0=win_raw[:], scalar1=5.5, scalar2=0.5,
                                    op0=OP.mult, op1=OP.add)
            tinv = wk.tile([P, H], F32, tag="tinv")
            nc.vector.reciprocal(tinv[:], hwv[:])

            # c0 = floor(c), phi = c - c0  (via +16 trunc with round-fix)
            cp16 = wk.tile([P, H], F32, tag="cp16")
            nc.vector.tensor_scalar(out=cp16[:], in0=cc_[:], scalar1=16.0, scalar2=None,
                                    op0=OP.add)
            ci = wk.tile([P, H], I32, tag="ci")
            nc.vector.tensor_copy(ci[:], cp16[:])
            cf = wk.tile([P, H], F32, tag="cf")
            nc.vector.tensor_copy(cf[:], ci[:])
            cgt = wk.tile([P, H], F32, tag="cgt")
            nc.vector.tensor_tensor(out=cgt[:], in0=cf[:], in1=cp16[:], op=OP.is_gt)
            c0p16 = wk.tile([P, H], F32, tag="c0p16")
            nc.vector.tensor_tensor(out=c0p16[:], in0=cf[:], in1=cgt[:], op=OP.subtract)
            phi = wk.tile([P, H], F32, tag="phi")
            nc.vector.tensor_tensor(out=phi[:], in0=cp16[:], in1=c0p16[:], op=OP.subtract)
            c012 = wk.tile([P, H], F32, tag="c012")   # c0 + 12
            nc.vector.tensor_scalar(out=c012[:], in0=c0p16[:], scalar1=4.0, scalar2=None,
                                    op0=OP.subtract)

            # --- rmsnorm kernel + silu ---
            ksq = wk.tile([P, C], F32, tag="ksq")
            nc.scalar.activation(ksq[:], kp[:], ACT.Square)
            kss = wk.tile([P, 1], F32, tag="kss")
            nc.vector.tensor_reduce(out=kss[:], in_=ksq[:], axis=mybir.AxisListType.X,
                                    op=OP.add)
            krstd = wk.tile([P, 1], F32, tag="krstd")
            nc.scalar.activation(krstd[:], kss[:], ACT.Sqrt, bias=eps_t[:], scale=1.0 / C)
            nc.vector.reciprocal(krstd[:], krstd[:])
            kn = wk.tile([P, C], F32, tag="kn")
            nc.vector.tensor_scalar(out=kn[:], in0=kp[:], scalar1=krstd[:], scalar2=None,
                                    op0=OP.mult)
            nc.vector.tensor_tensor(out=kn[:], in0=kn[:], in1=kg[:], op=OP.mult)
            kern = wk.tile([P, H, K], BF16, tag="kern")
            ksg = wk.tile([P, C], F32, tag="ksg")
            nc.scalar.activation(ksg[:], kn[:], ACT.Sigmoid)
            nc.vector.tensor_tensor(out=kern[:].rearrange("p h k -> p (h k)"), in0=kn[:],
                                    in1=ksg[:], op=OP.mult)

            # D table: Dt[k] = kern[k+1] - kern[k], Dt[63] = 0
            Dt = wk.tile([P, H, K], BF16, tag="Dt")
            nc.vector.memset(Dt[:, :, 63:64], 0.0)
            nc.vector.tensor_tensor(out=Dt[:, :, 0:63], in0=kern[:, :, 1:64],
                                    in1=kern[:, :, 0:63], op=OP.subtract)

            # --- interpolation indices ---
            rel6 = wk.tile([P, H, 6], F32, tag="rel6")
            nc.vector.tensor_tensor(out=rel6[:], in0=iotaA6[:],
                                    in1=_bcast(tinv[:][:, :, None], [P, H, 6]), op=OP.mult)
            npos = wk.tile([P, H, 6], F32, tag="npos")
            nc.vector.tensor_scalar(out=npos[:], in0=rel6[:], scalar1=1.0, scalar2=float(K - 1),
                                    op0=OP.min, op1=OP.mult)
            ii = wk.tile([P, H, 6], I32, tag="ii")
            nc.vector.tensor_copy(ii[:], npos[:])
            tf = wk.tile([P, H, 6], F32, tag="tf")
            nc.vector.tensor_copy(tf[:], ii[:])
            tgt = wk.tile([P, H, 6], F32, tag="tgt")
            nc.vector.tensor_tensor(out=tgt[:], in0=tf[:], in1=npos[:], op=OP.is_gt)
            idxf = wk.tile([P, H, 6], F32, tag="idxf")
            nc.vector.tensor_tensor(out=idxf[:], in0=tf[:], in1=tgt[:], op=OP.subtract)
            nc.vector.tensor_scalar(out=idxf[:], in0=idxf[:], scalar1=float(K - 2),
                                    scalar2=None, op0=OP.min)
            w_c = wk.tile([P, H, 6], F32, tag="w_c")
            nc.vector.tensor_tensor(out=w_c[:], in0=npos[:], in1=idxf[:], op=OP.subtract)

            # bits of idxf (f32 0/1 + int16 mask copies), msb first
            bits = []
            rcur = idxf
            for j, bv in enumerate([32.0, 16.0, 8.0, 4.0, 2.0, 1.0]):
                bj = wk.tile([P, H, 6], F32, tag=f"bit{j}")
                nc.vector.tensor_scalar(out=bj[:], in0=rcur[:], scalar1=bv, scalar2=None,
                                        op0=OP.is_ge)
                bi = wk.tile([P, H, 6], I16, tag=f"biti{j}")
                nc.vector.tensor_copy(bi[:], bj[:])
                bits.append(bi)
                if j < 5:
                    rnew = wk.tile([P, H, 6], F32, tag=f"rem{j}")
                    nc.vector.scalar_tensor_tensor(out=rnew[:], in0=bj[:], scalar=-bv,
                                                   in1=rcur[:], op0=OP.mult, op1=OP.add)
                    rcur = rnew

            # --- halving gather of (kern, Dt) pairs at idxf ---
            st = wk.tile([P, H, 6, 2, 32], BF16, tag="st")
            nc.vector.tensor_copy(st[:, :, :, 0, :],
                                  _bcast(kern[:][:, :, None, 0:32], [P, H, 6, 32]))
            nc.vector.tensor_copy(st[:, :, :, 1, :],
                                  _bcast(Dt[:][:, :, None, 0:32], [P, H, 6, 32]))
            nc.vector.copy_predicated(st[:, :, :, 0, :],
                                      _bcast(bits[0][:][:, :, :, None], [P, H, 6, 32]),
                                      _bcast(kern[:][:, :, None, 32:64], [P, H, 6, 32]))
            nc.vector.copy_predicated(st[:, :, :, 1, :],
                                      _bcast(bits[0][:][:, :, :, None], [P, H, 6, 32]),
                                      _bcast(Dt[:][:, :, None, 32:64], [P, H, 6, 32]))
            w = 16
            for j in range(1, 6):
                nc.vector.copy_predicated(
                    st[:, :, :, :, 0:w],
                    _bcast(bits[j][:][:, :, :, None, None], [P, H, 6, 2, w]),
                    st[:, :, :, :, w:2 * w])
                w //= 2
            # (sim note: copy_predicated views need the ravel shim in test_sim)
            # g0 = st[...,0,0], g1 = st[...,1,0]
            lerp = wk.tile([P, H, 6], F32, tag="lerp")
            nc.vector.tensor_tensor(out=lerp[:], in0=w_c[:], in1=st[:, :, :, 1, 0],
                                    op=OP.mult)
            nc.vector.tensor_tensor(out=lerp[:], in0=lerp[:], in1=st[:, :, :, 0, 0],
                                    op=OP.add)
            # ker7 = 1 + max(lerp, 0); col 0 from kern[...,0]
            ker7 = wk.tile([P, H, 7], F32, tag="ker7")
            nc.vector.tensor_scalar(out=ker7[:, :, 1:7], in0=lerp[:], scalar1=0.0,
                                    scalar2=1.0, op0=OP.max, op1=OP.add)
            nc.vector.tensor_scalar(out=ker7[:, :, 0:1], in0=kern[:, :, 0:1], scalar1=0.0,
                                    scalar2=1.0, op0=OP.max, op1=OP.add)

            # win7 = exp(-(a * tinv)^2)
            rel7 = wk.tile([P, H, 7], F32, tag="rel7")
            nc.vector.tensor_tensor(out=rel7[:], in0=iotaA7[:],
                                    in1=_bcast(tinv[:][:, :, None], [P, H, 7]), op=OP.mult)
            nc.vector.tensor_tensor(out=rel7[:], in0=rel7[:], in1=rel7[:], op=OP.mult)
            win7 = wk.tile([P, H, 7], F32, tag="win7")
            nc.scalar.activation(win7[:], rel7[:], ACT.Exp, scale=-1.0)
            wt7 = wk.tile([P, H, 7], F32, tag="wt7")
            nc.vector.tensor_tensor(out=wt7[:], in0=ker7[:], in1=win7[:], op=OP.mult)

            # expand to 13 taps (s order -6..6)
            w13 = wk.tile([P, H, 13], F32, tag="w13")
            for a in range(6):
                nc.vector.tensor_copy(w13[:, :, a:a + 1], wt7[:, :, 6 - a:7 - a])
            nc.vector.tensor_copy(w13[:, :, 6:13], wt7[:, :, 0:7])

            # validity / special masks
            nb13 = wk.tile([P, H, 13], F32, tag="nb13")
            lc = wk.tile([P, H], F32, tag="lc")
            nc.vector.tensor_tensor(out=lc[:], in0=cc_[:],
                                    in1=_bcast(lpos[:, lt:lt + 1], [P, H]), op=OP.add)
            nc.vector.tensor_tensor(out=nb13[:], in0=iotaS[:],
                                    in1=_bcast(lc[:][:, :, None], [P, H, 13]), op=OP.add)
            vlo = wk.tile([P, H, 13], F32, tag="vlo")
            nc.vector.tensor_scalar(out=vlo[:], in0=nb13[:], scalar1=0.0, scalar2=None,
                                    op0=OP.is_ge)
            vhi = wk.tile([P, H, 13], F32, tag="vhi")
            nc.vector.tensor_scalar(out=vhi[:], in0=nb13[:], scalar1=float(L), scalar2=None,
                                    op0=OP.is_lt)
            valid = wk.tile([P, H, 13], F32, tag="valid")
            nc.vector.tensor_tensor(out=valid[:], in0=vlo[:], in1=vhi[:], op=OP.mult)
            sp1 = wk.tile([P, H, 13], F32, tag="sp1")
            nc.vector.tensor_scalar(out=sp1[:], in0=nb13[:], scalar1=float(NB_HI),
                                    scalar2=None, op0=OP.is_gt)
            spec = wk.tile([P, H, 13], F32, tag="spec")
            nc.vector.tensor_tensor(out=spec[:], in0=sp1[:], in1=vhi[:], op=OP.mult)

            wv = wk.tile([P, H, 13], F32, tag="wv")
            nc.vector.tensor_tensor(out=wv[:], in0=w13[:], in1=valid[:], op=OP.mult)
            wsum = wk.tile([P, H], F32, tag="wsum")
            nc.vector.tensor_reduce(out=wsum[:], in_=wv[:], axis=mybir.AxisListType.X,
                                    op=OP.add)
            rw = wk.tile([P, H], F32, tag="rw")
            nc.vector.tensor_scalar(out=rw[:], in0=wsum[:], scalar1=1.0, scalar2=None,
                                    op0=OP.max)
            nc.vector.reciprocal(rw[:], rw[:])

            wsp = wk.tile([P, H, 13], F32, tag="wsp")
            nc.vector.tensor_tensor(out=wsp[:], in0=wv[:], in1=spec[:], op=OP.mult)
            wint = wk.tile([P, H, 13], F32, tag="wint")
            nc.vector.tensor_tensor(out=wint[:], in0=wv[:], in1=wsp[:], op=OP.subtract)
            wspec = wk.tile([P, H], F32, tag="wspec")
            nc.vector.tensor_reduce(out=wspec[:], in_=wsp[:], axis=mybir.AxisListType.X,
                                    op=OP.add)
            wspec_s = wk.tile([P, H], F32, tag="wspec_s")
            nc.vector.tensor_tensor(out=wspec_s[:], in0=wspec[:], in1=rw[:], op=OP.mult)

            om = wk.tile([P, H], F32, tag="om")
            nc.vector.tensor_scalar(out=om[:], in0=phi[:], scalar1=-1.0, scalar2=1.0,
                                    op0=OP.mult, op1=OP.add)
            uf = wk.tile([P, H], F32, tag="uf")
            nc.vector.tensor_tensor(out=uf[:], in0=om[:], in1=rw[:], op=OP.mult)
            uc = wk.tile([P, H], F32, tag="uc")
            nc.vector.tensor_tensor(out=uc[:], in0=phi[:], in1=rw[:], op=OP.mult)

            wf = wk.tile([P, H, 13], BF16, tag="wf")
            nc.vector.tensor_tensor(out=wf[:], in0=wint[:],
                                    in1=_bcast(uf[:][:, :, None], [P, H, 13]), op=OP.mult)
            wcc = wk.tile([P, H, 13], BF16, tag="wcc")
            nc.vector.tensor_tensor(out=wcc[:], in0=wint[:],
                                    in1=_bcast(uc[:][:, :, None], [P, H, 13]), op=OP.mult)

            # scatter indices
            idxf32 = wk.tile([P, H, 13], F32, tag="idxf32")
            nc.vector.tensor_tensor(out=idxf32[:], in0=iotaIDX[:],
                                    in1=_bcast(c012[:][:, :, None], [P, H, 13]), op=OP.add)
            i16 = wk.tile([P, H, 13], I16, tag="i16")
            nc.vector.tensor_copy(i16[:], idxf32[:])
            i16b = wk.tile([P, H, 13], I16, tag="i16b")
            nc.vector.tensor_scalar(out=i16b[:], in0=i16[:], scalar1=1, scalar2=None,
                                    op0=OP.add)

            A0 = wk.tile([P, H, 256], BF16, tag="A0")
            A1 = wk.tile([P, H, 256], BF16, tag="A1")
            for hb in range(2):
                hs = slice(hb * 4, hb * 4 + 4)
                nc.gpsimd.local_scatter(
                    A0[:, hs, :].rearrange("p h w -> p (h w)"),
                    wf[:, hs, :].rearrange("p h a -> p (h a)"),
                    i16[:, hs, :].rearrange("p h a -> p (h a)"),
                    channels=P, num_elems=4 * 256, num_idxs=4 * 13)
                nc.gpsimd.local_scatter(
                    A1[:, hs, :].rearrange("p h w -> p (h w)"),
                    wcc[:, hs, :].rearrange("p h a -> p (h a)"),
                    i16b[:, hs, :].rearrange("p h a -> p (h a)"),
                    channels=P, num_elems=4 * 256, num_idxs=4 * 13)
            A = wk.tile([P, H, 256], BF16, tag="A")
            nc.vector.tensor_tensor(out=A[:], in0=A0[:], in1=A1[:], op=OP.add)

            # wspecT rows (for the rank-1 edge correction): 8 x [1, P]
            wspb = wk.tile([P, H], BF16, tag="wspb")
            nc.vector.tensor_copy(wspb[:], wspec_s[:])
            wspT = wk.tile([1, H, P], BF16, tag="wspT_sb")
            for h_ in range(H):
                wsp_ps = ps_sm.tile([1, P], BF16, tag="tiny", name="wsp_ps")
                nc.tensor.transpose(wsp_ps[:], wspb[:, h_:h_ + 1], identb[:])
                nc.scalar.copy(wspT[0:1, h_, :], wsp_ps[:])

            # --- transpose A blocks and banded matmul ---
            for cci in range(4):
                po = ps_mm.tile([P, P], F32, tag="mm", name="po")
                for hh in range(2):
                    h = 2 * cci + hh
                    prange = slice(hh * 64, hh * 64 + 64)
                    for blk in range(2):
                        tp = ps_tr.tile([P, P], BF16, tag="tp")
                        nc.tensor.transpose(tp[:], A[:, h, blk * P:(blk + 1) * P], identb[:])
                        at = atp.tile([P, P], BF16, tag="at")
                        if (h + blk) % 2 == 0:
                            nc.scalar.copy(at[:], tp[:])
                        else:
                            nc.vector.tensor_copy(at[:], tp[:])
                        nc.tensor.matmul(po[prange, :],
                                         v_sb[lt + blk][:, h * D:(h + 1) * D],
                                         at[:], start=(blk == 0), stop=False)
                    nc.tensor.matmul(po[prange, :], vv[:, h * D:(h + 1) * D],
                                     wspT[0:1, h, :], start=False, stop=True)
                # SE partial sum + copy out
                red = wk.tile([P, 1], F32, tag="red")
                nc.vector.tensor_reduce(out=red[:], in_=po[:], axis=mybir.AxisListType.X,
                                        op=OP.add)
                nc.vector.tensor_tensor(out=seacc[:, cci:cci + 1], in0=seacc[:, cci:cci + 1],
                                        in1=red[:], op=OP.add)
                nc.scalar.copy(outT_sb[cci][:, lt * P:(lt + 1) * P], po[:])

        # ---------------- AllReduce of SE partial means ----------------
        if skip_cc:
            armean = seacc
        else:
            cci_d = dram.tile([P, 4], F32)
            cco_d = dram.tile([P, 4], F32)
            nc.gpsimd.dma_start(cci_d[:], seacc[:])
            nc.gpsimd.collective_compute(
                "AllReduce", OP.add,
                replica_groups=[[0, 1, 2, 3], [4, 5, 6, 7]],
                ins=[cci_d[:].opt()], outs=[cco_d[:].opt()])
            armean = cst.tile([P, 4], F32)
            nc.gpsimd.dma_start(armean[:], cco_d[:])

        # ---------------- SE MLP ----------------
        hid_ps = ps_sm.tile([1, P], F32, tag="tiny", name="hid_ps")
        for cci in range(4):
            nc.tensor.matmul(hid_ps[:], armean[:, cci:cci + 1], sw1T[cci][:],
                             start=(cci == 0), stop=(cci == 3))
        hid = cst.tile([1, P], BF16)
        hsg = cst.tile([1, P], F32)
        nc.scalar.activation(hsg[:], hid_ps[:], ACT.Sigmoid)
        nc.vector.tensor_tensor(out=hid[:], in0=hid_ps[:], in1=hsg[:], op=OP.mult)
        hidT_ps = ps_sm.tile([P, 1], BF16, tag="tiny", name="hidT_ps")
        nc.tensor.transpose(hidT_ps[:], hid[:], identb[0:1, 0:1])
        hidT = cst.tile([P, 1], BF16)
        nc.scalar.copy(hidT[:], hidT_ps[:])
        sc_ps = ps_sm.tile([1, C], F32, tag="tiny", name="sc_ps")
        nc.tensor.matmul(sc_ps[:], hidT[:], sw2T[:], start=True, stop=True)
        scrow = cst.tile([1, C], BF16)
        nc.scalar.activation(scrow[:], sc_ps[:], ACT.Sigmoid)
        owb = []
        for cci in range(4):
            scT_ps = ps_sm.tile([P, 1], BF16, tag="tiny", name="scT_ps")
            nc.tensor.transpose(scT_ps[:], scrow[:, cci * P:(cci + 1) * P], identb[0:1, 0:1])
            scT = cst.tile([P, 1], F32, tag=f"scT{cci}", name=f"scT{cci}")
            nc.scalar.copy(scT[:], scT_ps[:])
            ow = cst.tile([P, C], BF16, tag=f"owb{cci}", name=f"owb{cci}")
            nc.vector.tensor_scalar(out=ow[:], in0=owT[cci][:], scalar1=scT[:],
                                    scalar2=None, op0=OP.mult)
            owb.append(ow)

        # ---------------- final out_w matmul + silu + DMA out ----------------
        # out[l, cout] = sum_cin out_preT[cin, l] * ow'[cin, cout]  (lhsT = out_preT)
        for lt in range(LT):
            lsl = slice(lt * P, (lt + 1) * P)
            pf_ = ps_mm.tile([P, C], F32, tag="mm", name="pf_")
            for cci in range(4):
                nc.tensor.matmul(pf_[:], outT_sb[cci][:, lsl], owb[cci][:],
                                 start=(cci == 0), stop=(cci == 3))
            fo = wk.tile([P, C], F32, tag="fo")
            nc.scalar.activation(fo[:], pf_[:], ACT.Sigmoid)
            nc.vector.tensor_tensor(out=fo[:], in0=fo[:], in1=pf_[:], op=OP.mult)
            nc.sync.dma_start(dd["out"].ap()[lsl, :], fo[:])


def make_in_maps(inputs, n_cores=8):
    x = np.ascontiguousarray(inputs["x"], dtype=np.float32)
    window_w = inputs["window_w"]; window_b = inputs["window_b"]
    window_gamma = inputs["window_gamma"]
    offset_w = inputs["offset_w"]; offset_b = inputs["offset_b"]
    offset_gamma = inputs["offset_gamma"]
    kernel_w = inputs["kernel_w"]; kernel_b = inputs["kernel_b"]
    kernel_gamma = inputs["kernel_gamma"]
    v_w = inputs["v_w"]; v_b = inputs["v_b"]
    se_w1 = inputs["se_w1"]; se_w2 = inputs["se_w2"]; out_w = inputs["out_w"]

    woT = np.concatenate([window_w, offset_w], 0).T.astype(np.float32)      # (512,16)
    wob = np.concatenate([window_b, offset_b])[None].astype(np.float32)     # (1,16)
    wog = np.tile(np.concatenate([window_gamma, offset_gamma])[None], (P, 1)).astype(np.float32)
    kwT = np.ascontiguousarray(kernel_w.T, np.float32)
    kb = kernel_b[None].astype(np.float32)
    kgm = np.tile(kernel_gamma[None], (P, 1)).astype(np.float32)
    vwT = np.ascontiguousarray(v_w.T, np.float32)
    vbm = v_b[None].astype(np.float32)
    sw1T = np.ascontiguousarray(se_w1.T, np.float32) / np.float32(L)
    sw2T = np.ascontiguousarray(se_w2.T).astype(ml_dtypes.bfloat16)
    owT = np.ascontiguousarray(out_w.T, np.float32)

    in_maps = []
    for i in range(n_cores):
        b, q = divmod(i, 4)
        lo = q * LSH - HALO
        xpad = np.zeros((XROWS, C), np.float32)
        s0, s1 = max(lo, 0), min(lo + XROWS, L)
        xpad[s0 - lo:s1 - lo] = x[b, s0:s1]
        xT = np.ascontiguousarray(xpad.T)
        lpos = (q * LSH + np.arange(LSH, dtype=np.float32)).reshape(LT, P).T.copy()
        in_maps.append(dict(
            xT=xT, lpos=lpos, woT=woT, wob=wob, wog=wog, kwT=kwT, kb=kb,
            kg=kgm, vwT=vwT, vb=vbm, sw1T=sw1T, sw2T=sw2T, owT=owT,
            ones=np.ones((1, P), np.float32),
        ))
    return in_maps


def kernel(**inputs) -> np.ndarray:
    if "graph" not in _GRAPH_CACHE:
        _GRAPH_CACHE["graph"] = build_graph(8)
    nc = _GRAPH_CACHE["graph"]
    in_maps = make_in_maps(inputs, 8)
    res = run_bass_kernel_spmd(nc, in_maps, core_ids=list(range(8)))
    out = np.zeros((B, L, C), np.float32)
    for i in range(8):
        b, q = divmod(i, 4)
        out[b, q * LSH:(q + 1) * LSH] = res.results[i]["out"]
    return out


if __name__ == "__main__":
    import reference
    inputs = {k: np.asarray(v) for k, v in reference.setup_inputs().items()}
    got = kernel(**inputs)
    import jax.numpy as jnp
    exp = np.asarray(reference.reference(**{k: jnp.asarray(v) for k, v in inputs.items()}))
    rel = np.linalg.norm(got - exp) / np.linalg.norm(exp)
    print("Relative error:", rel)


# revision 27
# speedup vs baseline: 18.9084x; 1.1209x over previous
"""AdaptiveLocalConv Trainium2 kernel — 8-core SPMD.

Sharding: (batch, seq-quarter) -> 8 shards of 1024 tokens (+64 halo each side
for the deformable gather, reach <= +-19).

Per-core pipeline:
  - 4 projections from x via PE (f32r), x passed pre-transposed [C, 1152].
  - per-(token,head): 13 deformable taps; kernel-table interpolation via a
    log2 halving-gather on DVE; taps placed into a banded matrix A
    [token, head, 256-slot J-band] with gpsimd local_scatter (bf16).
  - banded matmul out^T[d, l] = sum_J v[J, d] * A^T[J, l] on PE after
    PE-transposing A blocks; sequence-end clamp handled exactly by a rank-1
    correction matmul.
  - squeeze-excite via a 4-core AllReduce of the per-core partial mean (2KB),
    SE scale folded into out_w columns; final out_w matmul in transposed
    layout; silu; DMA out with a transposing access pattern.
"""
import sys
if "/opt/trn_rl_repo" not in sys.path:
    sys.path.insert(0, "/opt/trn_rl_repo")

import numpy as np
import ml_dtypes

import concourse.bass as bass
import concourse.mybir as mybir
from concourse import bacc
from concourse.tile import TileContext
from concourse.bass_utils import run_bass_kernel_spmd
from concourse.masks import make_identity

F32 = mybir.dt.float32
F32R = mybir.dt.float32r
BF16 = mybir.dt.bfloat16
I32 = mybir.dt.int32
I16 = mybir.dt.int16
OP = mybir.AluOpType
ACT = mybir.ActivationFunctionType

B, L, C, H, K, D = 2, 4096, 512, 8, 64, 64
P = 128
HALO = 64
LSH = 1024          # tokens per core
XROWS = LSH + 2 * HALO   # 1152
LT = LSH // P       # 8 own l-tiles
VT = XROWS // P     # 9 v tiles
NB_HI = np.float32(L - 1.001)
EPS = 1e-6

_GRAPH_CACHE = {}
USE_SILU = True  # sim lacks Silu; test_sim sets False


def _bcast(ap, shape):
    return ap.broadcast_to(shape)


def build_graph(n_cores=8, skip_cc=False):
    nc = bacc.Bacc("TRN2", target_bir_lowering=False, debug=False,
                   enable_asserts=False, num_devices=n_cores)

    # ---------------- DRAM parameters ----------------
    xT_d = nc.dram_tensor("xT", [C, XROWS], F32R, kind="ExternalInput")
    lpos_d = nc.dram_tensor("lpos", [P, LT], F32, kind="ExternalInput")
    woT_d = nc.dram_tensor("woT", [C, 16], F32R, kind="ExternalInput")
    wob_d = nc.dram_tensor("wob", [1, 16], F32R, kind="ExternalInput")
    wog_d = nc.dram_tensor("wog", [P, 16], F32, kind="ExternalInput")
    kwT_d = nc.dram_tensor("kwT", [C, C], F32R, kind="ExternalInput")
    kb_d = nc.dram_tensor("kb", [1, C], F32R, kind="ExternalInput")
    kg_d = nc.dram_tensor("kg", [P, C], F32, kind="ExternalInput")
    vwT_d = nc.dram_tensor("vwT", [C, C], F32R, kind="ExternalInput")
    vb_d = nc.dram_tensor("vb", [1, C], F32R, kind="ExternalInput")
    sw1T_d = nc.dram_tensor("sw1T", [C, P], F32, kind="ExternalInput")   # pre-scaled by 1/L
    sw2T_d = nc.dram_tensor("sw2T", [P, C], BF16, kind="ExternalInput")
    owT_d = nc.dram_tensor("owT", [C, C], F32, kind="ExternalInput")
    ones_d = nc.dram_tensor("ones", [1, P], F32R, kind="ExternalInput")
    out_d = nc.dram_tensor("out", [LSH, C], F32, kind="ExternalOutput")

    with TileContext(nc) as tc:
        _build_body(nc, tc, dict(
            xT=xT_d, lpos=lpos_d, woT=woT_d, wob=wob_d, wog=wog_d,
            kwT=kwT_d, kb=kb_d, kg=kg_d, vwT=vwT_d, vb=vb_d,
            sw1T=sw1T_d, sw2T=sw2T_d, owT=owT_d, out=out_d, ones=ones_d,
        ), skip_cc=skip_cc)
    nc.compile()
    return nc


def _build_body(nc, tc, dd, skip_cc=False):
    import contextlib
    ctx = contextlib.ExitStack()
    with ctx:
        cst = ctx.enter_context(tc.tile_pool(name="cst", bufs=1))
        vsb = ctx.enter_context(tc.tile_pool(name="vsb", bufs=VT))
        wk = ctx.enter_context(tc.tile_pool(name="wk", bufs=2))
        atp = ctx.enter_context(tc.tile_pool(name="atp", bufs=4))
        outp = ctx.enter_context(tc.tile_pool(name="outp", bufs=1))
        ps_proj = ctx.enter_context(tc.tile_pool(name="ps_proj", bufs=2, space="PSUM"))
        ps_sm = ctx.enter_context(tc.tile_pool(name="ps_sm", bufs=2, space="PSUM"))
        ps_tr = ctx.enter_context(tc.tile_pool(name="ps_tr", bufs=2, space="PSUM"))
        ps_mm = ctx.enter_context(tc.tile_pool(name="ps_mm", bufs=2, space="PSUM"))
        dram = ctx.enter_context(tc.tile_pool(name="dram", bufs=1, space="DRAM"))

        # ---------------- constants & weights to SBUF ----------------
        xT = [cst.tile([P, XROWS], F32R, tag=f"xT{i}", name=f"xT{i}") for i in range(4)]
        for i in range(4):
            nc.sync.dma_start(xT[i][:], dd["xT"].ap()[i * P:(i + 1) * P, :])
        vwT = [cst.tile([P, C], F32R, tag=f"vwT{i}", name=f"vwT{i}") for i in range(4)]
        kwT = [cst.tile([P, C], F32R, tag=f"kwT{i}", name=f"kwT{i}") for i in range(4)]
        owT = [cst.tile([P, C], F32, tag=f"owT{i}", name=f"owT{i}") for i in range(4)]
        woT = [cst.tile([P, 16], F32R, tag=f"woT{i}", name=f"woT{i}") for i in range(4)]
        sw1T = [cst.tile([P, P], F32, tag=f"sw1T{i}", name=f"sw1T{i}") for i in range(4)]
        for i in range(4):
            sl = slice(i * P, (i + 1) * P)
            nc.sync.dma_start(vwT[i][:], dd["vwT"].ap()[sl, :])
            nc.sync.dma_start(kwT[i][:], dd["kwT"].ap()[sl, :])
            nc.sync.dma_start(owT[i][:], dd["owT"].ap()[sl, :])
            nc.sync.dma_start(woT[i][:], dd["woT"].ap()[sl, :])
            nc.sync.dma_start(sw1T[i][:], dd["sw1T"].ap()[sl, :])
        sw2T = cst.tile([P, C], BF16)
        nc.sync.dma_start(sw2T[:], dd["sw2T"].ap())
        wob = cst.tile([1, 16], F32R)
        kb = cst.tile([1, C], F32R)
        vb = cst.tile([1, C], F32R)
        wog = cst.tile([P, 16], F32)
        kg = cst.tile([P, C], F32)
        lpos = cst.tile([P, LT], F32)
        nc.sync.dma_start(wob[:], dd["wob"].ap())
        nc.sync.dma_start(kb[:], dd["kb"].ap())
        nc.sync.dma_start(vb[:], dd["vb"].ap())
        nc.sync.dma_start(wog[:], dd["wog"].ap())
        nc.sync.dma_start(kg[:], dd["kg"].ap())
        nc.sync.dma_start(lpos[:], dd["lpos"].ap())

        eps_t = cst.tile([P, 1], F32)
        nc.vector.memset(eps_t[:], EPS)
        ones1 = cst.tile([1, P], F32R)
        nc.sync.dma_start(ones1[:], dd["ones"].ap())
        identb = cst.tile([P, P], BF16)
        make_identity(nc, identb[:])

        # iotas
        iotaS = cst.tile([P, H, 13], F32)       # s value -6..6 per head
        it0 = cst.tile([P, H, 13], I32)
        nc.gpsimd.iota(it0[:], pattern=[[0, H], [1, 13]], base=-6, channel_multiplier=0)
        nc.vector.tensor_copy(iotaS[:], it0[:])
        iotaA5 = cst.tile([P, H, 5], F32)       # a = 1..5 per head
        it1 = cst.tile([P, H, 5], I32)
        nc.gpsimd.iota(it1[:], pattern=[[0, H], [1, 5]], base=1, channel_multiplier=0)
        nc.vector.tensor_copy(iotaA5[:], it1[:])
        iotaA7 = cst.tile([P, H, 7], F32)       # a = 0..6 per head
        it2 = cst.tile([P, H, 7], I32)
        nc.gpsimd.iota(it2[:], pattern=[[0, H], [1, 7]], base=0, channel_multiplier=0)
        nc.vector.tensor_copy(iotaA7[:], it2[:])
        # scatter index base: h*256 + lam + 46 + a  (a = s+6: 0..12)
        iotaIDX = cst.tile([P, H, 13], F32)
        it3 = cst.tile([P, H, 13], I32)
        nc.gpsimd.iota(it3[:], pattern=[[0, 2], [256, 4], [1, 13]], base=46, channel_multiplier=1)
        nc.vector.tensor_copy(iotaIDX[:], it3[:])

        # ---------------- v projection over halo (9 tiles) ----------------
        v_sb = []
        for t in range(VT):
            vp = ps_proj.tile([P, C], F32, tag="proj", name="vp")
            col = slice(t * P, (t + 1) * P)
            for kc in range(4):
                nc.tensor.matmul(vp[:], xT[kc][:, col],
                                 vwT[kc][:], start=(kc == 0), stop=False)
            nc.tensor.matmul(vp[:], ones1[:], vb[:],
                             start=False, stop=True)
            vt = vsb.tile([P, C], BF16, tag="v", name=f"v{t}")
            nc.scalar.copy(vt[:], vp[:])
            v_sb.append(vt)

        # vv = 0.001*v[L-2] + 0.999*v[L-1]  (local rows 1086, 1087 = tile 8, parts 62, 63)
        vv = cst.tile([1, C], BF16)
        vvf = cst.tile([1, C], F32)
        vt2 = cst.tile([1, C], BF16)
        vt3 = cst.tile([1, C], BF16)
        nc.sync.dma_start(vt2[:], v_sb[8][62:63, :])
        nc.sync.dma_start(vt3[:], v_sb[8][63:64, :])
        nc.vector.tensor_scalar(out=vvf[:], in0=vt2[:], scalar1=0.001,
                                scalar2=None, op0=OP.mult)
        nc.vector.scalar_tensor_tensor(out=vv[:], in0=vt3[:], scalar=0.999,
                                       in1=vvf[:], op0=OP.mult, op1=OP.add)
        # vv2[cc] = [2,128] block-diag: row0 = [vv_h0 | 0], row1 = [0 | vv_h1]
        vv2 = []
        for cc2 in range(4):
            v2 = cst.tile([2, P], BF16, tag=f"vv2_{cc2}", name=f"vv2_{cc2}")
            nc.vector.memset(v2[:], 0.0)
            nc.vector.tensor_copy(v2[0:1, 0:D], vv[:, (2 * cc2) * D:(2 * cc2 + 1) * D])
            nc.sync.dma_start(v2[1:2, D:2 * D], vv[:, (2 * cc2 + 1) * D:(2 * cc2 + 2) * D])
            vv2.append(v2)

        # squeeze-excite accumulator
        seacc = cst.tile([P, 4], F32)
        nc.vector.memset(seacc[:], 0.0)

        outT_sb = [outp.tile([P, LSH], BF16, tag=f"outT{cc}", name=f"outT{cc}") for cc in range(4)]

        # ---------------- main per-l-tile loop ----------------
        for lt in range(LT):
            xcol = slice(HALO + lt * P, HALO + (lt + 1) * P)

            # window/offset projection [P, 16]
            wop = ps_sm.tile([P, 16], F32, tag="tiny", name="wop")
            for kc in range(4):
                nc.tensor.matmul(wop[:], xT[kc][:, xcol],
                                 woT[kc][:], start=(kc == 0), stop=False)
            nc.tensor.matmul(wop[:], ones1[:], wob[:],
                             start=False, stop=True)
            # kernel projection [P, 512]
            kp = ps_proj.tile([P, C], F32, tag="proj", name="kp")
            for kc in range(4):
                nc.tensor.matmul(kp[:], xT[kc][:, xcol],
                                 kwT[kc][:], start=(kc == 0), stop=False)
            nc.tensor.matmul(kp[:], ones1[:], kb[:],
                             start=False, stop=True)

            # --- rmsnorm window/offset, sigmoid/tanh ---
            wsq = wk.tile([P, 16], F32, tag="wsq")
            nc.scalar.activation(wsq[:], wop[:], ACT.Square)
            wss = wk.tile([P, 2], F32, tag="wss")
            nc.vector.tensor_reduce(out=wss[:], in_=wsq[:].rearrange("p (g h) -> p g h", g=2),
                                    axis=mybir.AxisListType.X, op=OP.add)
            wrstd = wk.tile([P, 2], F32, tag="wrstd")
            nc.scalar.activation(wrstd[:], wss[:], ACT.Sqrt, bias=eps_t[:], scale=1.0 / 8)
            nc.vector.reciprocal(wrstd[:], wrstd[:])
            won = wk.tile([P, 16], F32, tag="won")
            nc.vector.tensor_tensor(
                out=won[:].rearrange("p (g h) -> p g h", g=2),
                in0=wop[:].rearrange("p (g h) -> p g h", g=2),
                in1=_bcast(wrstd[:][:, :, None], [P, 2, 8]), op=OP.mult)
            nc.vector.tensor_tensor(out=won[:], in0=won[:], in1=wog[:], op=OP.mult)
            win_raw = wk.tile([P, H], F32, tag="win_raw")
            nc.scalar.activation(win_raw[:], won[:, 0:8], ACT.Sigmoid)
            cth = wk.tile([P, H], F32, tag="cth")
            nc.scalar.activation(cth[:], won[:, 8:16], ACT.Tanh)
            cc_ = wk.tile([P, H], F32, tag="cc_")
            nc.vector.tensor_scalar(out=cc_[:], in0=cth[:], scalar1=12.0, scalar2=None,
                                    op0=OP.mult)
            hwv = wk.tile([P, H], F32, tag="hwv")
            nc.vector.tensor_scalar(out=hwv[:], in0=win_raw[:], scalar1=5.5, scalar2=0.5,
                                    op0=OP.mult, op1=OP.add)
            tinv = wk.tile([P, H], F32, tag="tinv")
            nc.vector.reciprocal(tinv[:], hwv[:])

            # c0 = floor(c), phi = c - c0  (via +16 trunc with round-fix)
            cp16 = wk.tile([P, H], F32, tag="cp16")
            nc.vector.tensor_scalar(out=cp16[:], in0=cc_[:], scalar1=16.0, scalar2=None,
                                    op0=OP.add)
            ci = wk.tile([P, H], I32, tag="ci")
            nc.vector.tensor_copy(ci[:], cp16[:])
            cf = wk.tile([P, H], F32, tag="cf")
            nc.vector.tensor_copy(cf[:], ci[:])
            cgt = wk.tile([P, H], F32, tag="cgt")
            nc.vector.tensor_tensor(out=cgt[:], in0=cf[:], in1=cp16[:], op=OP.is_gt)
            c0p16 = wk.tile([P, H], F32, tag="c0p16")
            nc.vector.tensor_tensor(out=c0p16[:], in0=cf[:], in1=cgt[:], op=OP.subtract)
            phi = wk.tile([P, H], F32, tag="phi")
            nc.vector.tensor_tensor(out=phi[:], in0=cp16[:], in1=c0p16[:], op=OP.subtract)
            c012 = wk.tile([P, H], F32, tag="c012")   # c0 + 12
            nc.vector.tensor_scalar(out=c012[:], in0=c0p16[:], scalar1=4.0, scalar2=None,
                                    op0=OP.subtract)

            # --- rmsnorm kernel + silu ---
            ksq = wk.tile([P, C], F32, tag="ksq")
            nc.scalar.activation(ksq[:], kp[:], ACT.Square)
            kss = wk.tile([P, 1], F32, tag="kss")
            nc.vector.tensor_reduce(out=kss[:], in_=ksq[:], axis=mybir.AxisListType.X,
                                    op=OP.add)
            krstd = wk.tile([P, 1], F32, tag="krstd")
            nc.scalar.activation(krstd[:], kss[:], ACT.Sqrt, bias=eps_t[:], scale=1.0 / C)
            nc.vector.reciprocal(krstd[:], krstd[:])
            kn = wk.tile([P, C], F32, tag="kn")
            nc.vector.tensor_scalar(out=kn[:], in0=kp[:], scalar1=krstd[:], scalar2=None,
                                    op0=OP.mult)
            nc.vector.tensor_tensor(out=kn[:], in0=kn[:], in1=kg[:], op=OP.mult)
            kern = wk.tile([P, H, K], BF16, tag="kern")
            if USE_SILU:
                nc.scalar.activation(kern[:].rearrange("p h k -> p (h k)"), kn[:], ACT.Silu)
            else:
                ksg = wk.tile([P, C], F32, tag="ksg")
                nc.scalar.activation(ksg[:], kn[:], ACT.Sigmoid)
                nc.vector.tensor_tensor(out=kern[:].rearrange("p h k -> p (h k)"), in0=kn[:],
                                        in1=ksg[:], op=OP.mult)

            # D table: Dt[k] = kern[k+1] - kern[k], Dt[63] = 0
            Dt = wk.tile([P, H, K], BF16, tag="Dt")
            nc.vector.memset(Dt[:, :, 63:64], 0.0)
            nc.vector.tensor_tensor(out=Dt[:, :, 0:63], in0=kern[:, :, 1:64],
                                    in1=kern[:, :, 0:63], op=OP.subtract)

            # --- interpolation indices (a = 1..5; a=6 always clips to slot 63) ---
            A5 = 5
            npos = wk.tile([P, H, A5], F32, tag="npos")
            nc.vector.tensor_tensor(out=npos[:], in0=iotaA5[:],
                                    in1=_bcast(tinv[:][:, :, None], [P, H, A5]), op=OP.mult)
            nc.vector.tensor_scalar(out=npos[:], in0=npos[:], scalar1=1.0, scalar2=float(K - 1),
                                    op0=OP.min, op1=OP.mult)
            ii = wk.tile([P, H, A5], I32, tag="ii")
            nc.vector.tensor_copy(ii[:], npos[:])
            tf = wk.tile([P, H, A5], F32, tag="tf")
            nc.vector.tensor_copy(tf[:], ii[:])
            tgt = wk.tile([P, H, A5], F32, tag="tgt")
            nc.vector.tensor_tensor(out=tgt[:], in0=tf[:], in1=npos[:], op=OP.is_gt)
            idxf = wk.tile([P, H, A5], F32, tag="idxf")
            nc.vector.tensor_tensor(out=idxf[:], in0=tf[:], in1=tgt[:], op=OP.subtract)
            nc.vector.tensor_scalar(out=idxf[:], in0=idxf[:], scalar1=float(K - 2),
                                    scalar2=None, op0=OP.min)
            w_c = wk.tile([P, H, A5], F32, tag="w_c")
            nc.vector.tensor_tensor(out=w_c[:], in0=npos[:], in1=idxf[:], op=OP.subtract)

            # bits of idxf as int16 masks, msb first
            bits = []
            rcur = idxf
            for j, bv in enumerate([32.0, 16.0, 8.0, 4.0, 2.0, 1.0]):
                bi = wk.tile([P, H, A5], I16, tag=f"biti{j}")
                nc.vector.tensor_scalar(out=bi[:], in0=rcur[:], scalar1=bv, scalar2=None,
                                        op0=OP.is_ge)
                bits.append(bi)
                if j < 5:
                    rnew = wk.tile([P, H, A5], F32, tag=f"rem{j}")
                    nc.vector.scalar_tensor_tensor(out=rnew[:], in0=bi[:], scalar=-bv,
                                                   in1=rcur[:], op0=OP.mult, op1=OP.add)
                    rcur = rnew

            # --- halving gather of (kern, Dt) pairs at idxf ---
            st = wk.tile([P, H, A5, 2, 32], BF16, tag="st")
            nc.vector.tensor_copy(st[:, :, :, 0, :],
                                  _bcast(kern[:][:, :, None, 0:32], [P, H, A5, 32]))
            nc.vector.tensor_copy(st[:, :, :, 1, :],
                                  _bcast(Dt[:][:, :, None, 0:32], [P, H, A5, 32]))
            nc.vector.copy_predicated(st[:, :, :, 0, :],
                                      _bcast(bits[0][:][:, :, :, None], [P, H, A5, 32]),
                                      _bcast(kern[:][:, :, None, 32:64], [P, H, A5, 32]))
            nc.vector.copy_predicated(st[:, :, :, 1, :],
                                      _bcast(bits[0][:][:, :, :, None], [P, H, A5, 32]),
                                      _bcast(Dt[:][:, :, None, 32:64], [P, H, A5, 32]))
            w = 16
            for j in range(1, 6):
                nc.vector.copy_predicated(
                    st[:, :, :, :, 0:w],
                    _bcast(bits[j][:][:, :, :, None, None], [P, H, A5, 2, w]),
                    st[:, :, :, :, w:2 * w])
                w //= 2
            # g0 = st[...,0,0], g1 = st[...,1,0]
            lerp = wk.tile([P, H, A5], F32, tag="lerp")
            nc.vector.tensor_tensor(out=lerp[:], in0=w_c[:], in1=st[:, :, :, 1, 0],
                                    op=OP.mult)
            nc.vector.tensor_tensor(out=lerp[:], in0=lerp[:], in1=st[:, :, :, 0, 0],
                                    op=OP.add)
            # ker7 = 1 + max(lerp, 0); col 0 from kern[...,0]; col 6 from kern[...,63]
            ker7 = wk.tile([P, H, 7], F32, tag="ker7")
            nc.vector.tensor_scalar(out=ker7[:, :, 1:6], in0=lerp[:], scalar1=0.0,
                                    scalar2=1.0, op0=OP.max, op1=OP.add)
            nc.vector.tensor_scalar(out=ker7[:, :, 0:1], in0=kern[:, :, 0:1], scalar1=0.0,
                                    scalar2=1.0, op0=OP.max, op1=OP.add)
            nc.vector.tensor_scalar(out=ker7[:, :, 6:7], in0=kern[:, :, 63:64], scalar1=0.0,
                                    scalar2=1.0, op0=OP.max, op1=OP.add)

            # win7 = exp(-(a * tinv)^2)
            rel7 = wk.tile([P, H, 7], F32, tag="rel7")
            nc.vector.tensor_tensor(out=rel7[:], in0=iotaA7[:],
                                    in1=_bcast(tinv[:][:, :, None], [P, H, 7]), op=OP.mult)
            nc.vector.tensor_tensor(out=rel7[:], in0=rel7[:], in1=rel7[:], op=OP.mult)
            win7 = wk.tile([P, H, 7], F32, tag="win7")
            nc.scalar.activation(win7[:], rel7[:], ACT.Exp, scale=-1.0)
            wt7 = wk.tile([P, H, 7], F32, tag="wt7")
            nc.vector.tensor_tensor(out=wt7[:], in0=ker7[:], in1=win7[:], op=OP.mult)

            # expand to 13 taps (s order -6..6)
            w13 = wk.tile([P, H, 13], F32, tag="w13")
            nc.vector.tensor_copy(w13[:, :, 0:7], wt7[:, :, ::-1])
            nc.vector.tensor_copy(w13[:, :, 6:13], wt7[:, :, 0:7])

            # validity / special masks
            nb13 = wk.tile([P, H, 13], F32, tag="nb13")
            lc = wk.tile([P, H], F32, tag="lc")
            nc.vector.tensor_tensor(out=lc[:], in0=cc_[:],
                                    in1=_bcast(lpos[:, lt:lt + 1], [P, H]), op=OP.add)
            nc.vector.tensor_tensor(out=nb13[:], in0=iotaS[:],
                                    in1=_bcast(lc[:][:, :, None], [P, H, 13]), op=OP.add)
            vhi = wk.tile([P, H, 13], F32, tag="vhi")
            nc.vector.tensor_scalar(out=vhi[:], in0=nb13[:], scalar1=float(L), scalar2=None,
                                    op0=OP.is_lt)
            valid = wk.tile([P, H, 13], F32, tag="valid")
            nc.vector.scalar_tensor_tensor(out=valid[:], in0=nb13[:], scalar=0.0,
                                           in1=vhi[:], op0=OP.is_ge, op1=OP.mult)
            spec = wk.tile([P, H, 13], F32, tag="spec")
            nc.vector.scalar_tensor_tensor(out=spec[:], in0=nb13[:], scalar=float(NB_HI),
                                           in1=vhi[:], op0=OP.is_gt, op1=OP.mult)

            wv = wk.tile([P, H, 13], F32, tag="wv")
            nc.vector.tensor_tensor(out=wv[:], in0=w13[:], in1=valid[:], op=OP.mult)
            wsum = wk.tile([P, H], F32, tag="wsum")
            nc.vector.tensor_reduce(out=wsum[:], in_=wv[:], axis=mybir.AxisListType.X,
                                    op=OP.add)
            rw = wk.tile([P, H], F32, tag="rw")
            nc.vector.tensor_scalar(out=rw[:], in0=wsum[:], scalar1=1.0, scalar2=None,
                                    op0=OP.max)
            nc.vector.reciprocal(rw[:], rw[:])

            wsp = wk.tile([P, H, 13], F32, tag="wsp")
            nc.vector.tensor_tensor(out=wsp[:], in0=wv[:], in1=spec[:], op=OP.mult)
            wint = wk.tile([P, H, 13], F32, tag="wint")
            nc.vector.tensor_tensor(out=wint[:], in0=wv[:], in1=wsp[:], op=OP.subtract)
            wspec = wk.tile([P, H], F32, tag="wspec")
            nc.vector.tensor_reduce(out=wspec[:], in_=wsp[:], axis=mybir.AxisListType.X,
                                    op=OP.add)
            wspec_s = wk.tile([P, H], F32, tag="wspec_s")
            nc.vector.tensor_tensor(out=wspec_s[:], in0=wspec[:], in1=rw[:], op=OP.mult)

            om = wk.tile([P, H], F32, tag="om")
            nc.vector.tensor_scalar(out=om[:], in0=phi[:], scalar1=-1.0, scalar2=1.0,
                                    op0=OP.mult, op1=OP.add)
            uf = wk.tile([P, H], F32, tag="uf")
            nc.vector.tensor_tensor(out=uf[:], in0=om[:], in1=rw[:], op=OP.mult)
            uc = wk.tile([P, H], F32, tag="uc")
            nc.vector.tensor_tensor(out=uc[:], in0=phi[:], in1=rw[:], op=OP.mult)

            wf = wk.tile([P, H, 13], BF16, tag="wf")
            nc.vector.tensor_tensor(out=wf[:], in0=wint[:],
                                    in1=_bcast(uf[:][:, :, None], [P, H, 13]), op=OP.mult)
            wcc = wk.tile([P, H, 13], BF16, tag="wcc")
            nc.vector.tensor_tensor(out=wcc[:], in0=wint[:],
                                    in1=_bcast(uc[:][:, :, None], [P, H, 13]), op=OP.mult)

            # scatter indices
            idxf32 = wk.tile([P, H, 13], F32, tag="idxf32")
            nc.vector.tensor_tensor(out=idxf32[:], in0=iotaIDX[:],
                                    in1=_bcast(c012[:][:, :, None], [P, H, 13]), op=OP.add)
            i16 = wk.tile([P, H, 13], I16, tag="i16")
            nc.vector.tensor_copy(i16[:], idxf32[:])
            i16b = wk.tile([P, H, 13], I16, tag="i16b")
            nc.vector.tensor_scalar(out=i16b[:], in0=i16[:], scalar1=1, scalar2=None,
                                    op0=OP.add)

            A0 = wk.tile([P, H, 256], BF16, tag="A0")
            A1 = wk.tile([P, H, 256], BF16, tag="A1")
            for hb in range(2):
                hs = slice(hb * 4, hb * 4 + 4)
                nc.gpsimd.local_scatter(
                    A0[:, hs, :].rearrange("p h w -> p (h w)"),
                    wf[:, hs, :].rearrange("p h a -> p (h a)"),
                    i16[:, hs, :].rearrange("p h a -> p (h a)"),
                    channels=P, num_elems=4 * 256, num_idxs=4 * 13)
                nc.gpsimd.local_scatter(
                    A1[:, hs, :].rearrange("p h w -> p (h w)"),
                    wcc[:, hs, :].rearrange("p h a -> p (h a)"),
                    i16b[:, hs, :].rearrange("p h a -> p (h a)"),
                    channels=P, num_elems=4 * 256, num_idxs=4 * 13)
            A = wk.tile([P, H, 256], BF16, tag="A")
            nc.vector.tensor_tensor(out=A[:], in0=A0[:], in1=A1[:], op=OP.add)

            # wspecT head-pair rows (for the rank-2 edge correction): 4 x [2, P]
            wspb = wk.tile([P, H], BF16, tag="wspb")
            nc.vector.tensor_copy(wspb[:], wspec_s[:])
            wspT2 = []
            for cc2 in range(4):
                wsp_ps = ps_sm.tile([2, P], BF16, tag="tiny", name="wsp_ps")
                nc.tensor.transpose(wsp_ps[:], wspb[:, 2 * cc2:2 * cc2 + 2], identb[:])
                wt_ = wk.tile([2, P], BF16, tag=f"wspT2_{cc2}", name=f"wspT2_{cc2}")
                nc.scalar.copy(wt_[:], wsp_ps[:])
                wspT2.append(wt_)

            # --- transpose A blocks and banded matmul ---
            for cci in range(4):
                po = ps_mm.tile([P, P], F32, tag="mm", name="po")
                for hh in range(2):
                    h = 2 * cci + hh
                    prange = slice(hh * 64, hh * 64 + 64)
                    for blk in range(2):
                        tp = ps_tr.tile([P, P], BF16, tag="tp")
                        nc.tensor.transpose(tp[:], A[:, h, blk * P:(blk + 1) * P], identb[:])
                        at = atp.tile([P, P], BF16, tag="at")
                        nc.scalar.copy(at[:], tp[:])
                        nc.tensor.matmul(po[prange, :],
                                         v_sb[lt + blk][:, h * D:(h + 1) * D],
                                         at[:], start=(blk == 0), stop=(blk == 1))
                nc.tensor.matmul(po[:], vv2[cci][:], wspT2[cci][:],
                                 start=False, stop=True, skip_group_check=True)
                # SE partial sum + copy out
                red = wk.tile([P, 1], F32, tag="red")
                nc.vector.tensor_reduce(out=red[:], in_=po[:], axis=mybir.AxisListType.X,
                                        op=OP.add)
                nc.vector.tensor_tensor(out=seacc[:, cci:cci + 1], in0=seacc[:, cci:cci + 1],
                                        in1=red[:], op=OP.add)
                nc.scalar.copy(outT_sb[cci][:, lt * P:(lt + 1) * P], po[:])

        # ---------------- AllReduce of SE partial means ----------------
        if skip_cc:
            armean = seacc
        else:
            cci_d = dram.tile([P, 4], F32)
            cco_d = dram.tile([P, 4], F32)
            nc.gpsimd.dma_start(cci_d[:], seacc[:])
            nc.gpsimd.collective_compute(
                "AllReduce", OP.add,
                replica_groups=[[0, 1, 2, 3], [4, 5, 6, 7]],
                ins=[cci_d[:].opt()], outs=[cco_d[:].opt()])
            armean = cst.tile([P, 4], F32)
            nc.gpsimd.dma_start(armean[:], cco_d[:])

        # ---------------- SE MLP ----------------
        hid_ps = ps_sm.tile([1, P], F32, tag="tiny", name="hid_ps")
        for cci in range(4):
            nc.tensor.matmul(hid_ps[:], armean[:, cci:cci + 1], sw1T[cci][:],
                             start=(cci == 0), stop=(cci == 3))
        hid = cst.tile([1, P], BF16)
        if USE_SILU:
            nc.scalar.activation(hid[:], hid_ps[:], ACT.Silu)
        else:
            hsg = cst.tile([1, P], F32)
            nc.scalar.activation(hsg[:], hid_ps[:], ACT.Sigmoid)
            nc.vector.tensor_tensor(out=hid[:], in0=hid_ps[:], in1=hsg[:], op=OP.mult)
        hidT_ps = ps_sm.tile([P, 1], BF16, tag="tiny", name="hidT_ps")
        nc.tensor.transpose(hidT_ps[:], hid[:], identb[0:1, 0:1])
        hidT = cst.tile([P, 1], BF16)
        nc.scalar.copy(hidT[:], hidT_ps[:])
        sc_ps = ps_sm.tile([1, C], F32, tag="tiny", name="sc_ps")
        nc.tensor.matmul(sc_ps[:], hidT[:], sw2T[:], start=True, stop=True)
        scrow = cst.tile([1, C], BF16)
        nc.scalar.activation(scrow[:], sc_ps[:], ACT.Sigmoid)
        owb = []
        for cci in range(4):
            scT_ps = ps_sm.tile([P, 1], BF16, tag="tiny", name="scT_ps")
            nc.tensor.transpose(scT_ps[:], scrow[:, cci * P:(cci + 1) * P], identb[0:1, 0:1])
            scT = cst.tile([P, 1], F32, tag=f"scT{cci}", name=f"scT{cci}")
            nc.scalar.copy(scT[:], scT_ps[:])
            ow = cst.tile([P, C], BF16, tag=f"owb{cci}", name=f"owb{cci}")
            nc.vector.tensor_scalar(out=ow[:], in0=owT[cci][:], scalar1=scT[:],
                                    scalar2=None, op0=OP.mult)
            owb.append(ow)

        # ---------------- final out_w matmul + silu + DMA out ----------------
        # out[l, cout] = sum_cin out_preT[cin, l] * ow'[cin, cout]  (lhsT = out_preT)
        for lt in range(LT):
            lsl = slice(lt * P, (lt + 1) * P)
            pf_ = ps_mm.tile([P, C], F32, tag="mm", name="pf_")
            for cci in range(4):
                nc.tensor.matmul(pf_[:], outT_sb[cci][:, lsl], owb[cci][:],
                                 start=(cci == 0), stop=(cci == 3))
            fo = wk.tile([P, C], F32, tag="fo")
            if USE_SILU:
                nc.scalar.activation(fo[:], pf_[:], ACT.Silu)
            else:
                nc.scalar.activation(fo[:], pf_[:], ACT.Sigmoid)
                nc.vector.tensor_tensor(out=fo[:], in0=fo[:], in1=pf_[:], op=OP.mult)
            nc.sync.dma_start(dd["out"].ap()[lsl, :], fo[:])


def make_in_maps(inputs, n_cores=8):
    x = np.ascontiguousarray(inputs["x"], dtype=np.float32)
    window_w = inputs["window_w"]; window_b = inputs["window_b"]
    window_gamma = inputs["window_gamma"]
    offset_w = inputs["offset_w"]; offset_b = inputs["offset_b"]
    offset_gamma = inputs["offset_gamma"]
    kernel_w = inputs["kernel_w"]; kernel_b = inputs["kernel_b"]
    kernel_gamma = inputs["kernel_gamma"]
    v_w = inputs["v_w"]; v_b = inputs["v_b"]
    se_w1 = inputs["se_w1"]; se_w2 = inputs["se_w2"]; out_w = inputs["out_w"]

    woT = np.concatenate([window_w, offset_w], 0).T.astype(np.float32)      # (512,16)
    wob = np.concatenate([window_b, offset_b])[None].astype(np.float32)     # (1,16)
    wog = np.tile(np.concatenate([window_gamma, offset_gamma])[None], (P, 1)).astype(np.float32)
    kwT = np.ascontiguousarray(kernel_w.T, np.float32)
    kb = kernel_b[None].astype(np.float32)
    kgm = np.tile(kernel_gamma[None], (P, 1)).astype(np.float32)
    vwT = np.ascontiguousarray(v_w.T, np.float32)
    vbm = v_b[None].astype(np.float32)
    sw1T = np.ascontiguousarray(se_w1.T, np.float32) / np.float32(L)
    sw2T = np.ascontiguousarray(se_w2.T).astype(ml_dtypes.bfloat16)
    owT = np.ascontiguousarray(out_w.T, np.float32)

    in_maps = []
    for i in range(n_cores):
        b, q = divmod(i, 4)
        lo = q * LSH - HALO
        xpad = np.zeros((XROWS, C), np.float32)
        s0, s1 = max(lo, 0), min(lo + XROWS, L)
        xpad[s0 - lo:s1 - lo] = x[b, s0:s1]
        xT = np.ascontiguousarray(xpad.T)
        lpos = (q * LSH + np.arange(LSH, dtype=np.float32)).reshape(LT, P).T.copy()
        in_maps.append(dict(
            xT=xT, lpos=lpos, woT=woT, wob=wob, wog=wog, kwT=kwT, kb=kb,
            kg=kgm, vwT=vwT, vb=vbm, sw1T=sw1T, sw2T=sw2T, owT=owT,
            ones=np.ones((1, P), np.float32),
        ))
    return in_maps


def kernel(**inputs) -> np.ndarray:
    if "graph" not in _GRAPH_CACHE:
        _GRAPH_CACHE["graph"] = build_graph(8)
    nc = _GRAPH_CACHE["graph"]
    in_maps = make_in_maps(inputs, 8)
    res = run_bass_kernel_spmd(nc, in_maps, core_ids=list(range(8)))
    out = np.zeros((B, L, C), np.float32)
    for i in range(8):
        b, q = divmod(i, 4)
        out[b, q * LSH:(q + 1) * LSH] = res.results[i]["out"]
    return out


if __name__ == "__main__":
    import reference
    inputs = {k: np.asarray(v) for k, v in reference.setup_inputs().items()}
    got = kernel(**inputs)
    import jax.numpy as jnp
    exp = np.asarray(reference.reference(**{k: jnp.asarray(v) for k, v in inputs.items()}))
    rel = np.linalg.norm(got - exp) / np.linalg.norm(exp)
    print("Relative error:", rel)


# revision 31
# speedup vs baseline: 19.7951x; 1.0469x over previous
"""AdaptiveLocalConv Trainium2 kernel — 8-core SPMD.

Sharding: (batch, seq-quarter) -> 8 shards of 1024 tokens (+64 halo each side
for the deformable gather, reach <= +-19).

Per-core pipeline:
  - 4 projections from x via PE (f32r), x passed pre-transposed [C, 1152].
  - per-(token,head): 13 deformable taps; kernel-table interpolation via a
    log2 halving-gather on DVE; taps placed into a banded matrix A
    [token, head, 256-slot J-band] with gpsimd local_scatter (bf16).
  - banded matmul out^T[d, l] = sum_J v[J, d] * A^T[J, l] on PE after
    PE-transposing A blocks; sequence-end clamp handled exactly by a rank-1
    correction matmul.
  - squeeze-excite via a 4-core AllReduce of the per-core partial mean (2KB),
    SE scale folded into out_w columns; final out_w matmul in transposed
    layout; silu; DMA out with a transposing access pattern.
"""
import sys
if "/opt/trn_rl_repo" not in sys.path:
    sys.path.insert(0, "/opt/trn_rl_repo")

import numpy as np
import ml_dtypes

import concourse.bass as bass
import concourse.mybir as mybir
from concourse import bacc
from concourse.tile import TileContext
from concourse.bass_utils import run_bass_kernel_spmd
from concourse.masks import make_identity

F32 = mybir.dt.float32
F32R = mybir.dt.float32r
BF16 = mybir.dt.bfloat16
I32 = mybir.dt.int32
I16 = mybir.dt.int16
OP = mybir.AluOpType
ACT = mybir.ActivationFunctionType

B, L, C, H, K, D = 2, 4096, 512, 8, 64, 64
P = 128
HALO = 64
LSH = 1024          # tokens per core
XROWS = LSH + 2 * HALO   # 1152
LT = LSH // P       # 8 own l-tiles
VT = XROWS // P     # 9 v tiles
NB_HI = np.float32(L - 1.001)
EPS = 1e-6

_GRAPH_CACHE = {}
USE_SILU = True  # sim lacks Silu; test_sim sets False


def _bcast(ap, shape):
    return ap.broadcast_to(shape)


def build_graph(n_cores=8, skip_cc=False):
    nc = bacc.Bacc("TRN2", target_bir_lowering=False, debug=False,
                   enable_asserts=False, num_devices=n_cores)

    # ---------------- DRAM parameters ----------------
    xT_d = nc.dram_tensor("xT", [C, XROWS], F32R, kind="ExternalInput")
    lpos_d = nc.dram_tensor("lpos", [P, LT], F32, kind="ExternalInput")
    woT_d = nc.dram_tensor("woT", [C, 16], F32R, kind="ExternalInput")
    wob_d = nc.dram_tensor("wob", [1, 16], F32R, kind="ExternalInput")
    wog_d = nc.dram_tensor("wog", [P, 16], F32, kind="ExternalInput")
    kwT_d = nc.dram_tensor("kwT", [C, C], F32R, kind="ExternalInput")
    kb_d = nc.dram_tensor("kb", [1, C], F32R, kind="ExternalInput")
    kg_d = nc.dram_tensor("kg", [P, C], F32, kind="ExternalInput")
    vwT_d = nc.dram_tensor("vwT", [C, C], F32R, kind="ExternalInput")
    vb_d = nc.dram_tensor("vb", [1, C], F32R, kind="ExternalInput")
    sw1T_d = nc.dram_tensor("sw1T", [C, P], F32, kind="ExternalInput")   # pre-scaled by 1/L
    sw2T_d = nc.dram_tensor("sw2T", [P, C], BF16, kind="ExternalInput")
    owT_d = nc.dram_tensor("owT", [C, C], F32, kind="ExternalInput")
    ones_d = nc.dram_tensor("ones", [1, P], F32R, kind="ExternalInput")
    out_d = nc.dram_tensor("out", [LSH, C], F32, kind="ExternalOutput")

    with TileContext(nc) as tc:
        _build_body(nc, tc, dict(
            xT=xT_d, lpos=lpos_d, woT=woT_d, wob=wob_d, wog=wog_d,
            kwT=kwT_d, kb=kb_d, kg=kg_d, vwT=vwT_d, vb=vb_d,
            sw1T=sw1T_d, sw2T=sw2T_d, owT=owT_d, out=out_d, ones=ones_d,
        ), skip_cc=skip_cc)
    nc.compile()
    return nc


def _build_body(nc, tc, dd, skip_cc=False):
    import contextlib
    ctx = contextlib.ExitStack()
    with ctx:
        cst = ctx.enter_context(tc.tile_pool(name="cst", bufs=1))
        vsb = ctx.enter_context(tc.tile_pool(name="vsb", bufs=VT))
        wk = ctx.enter_context(tc.tile_pool(name="wk", bufs=2))
        atp = ctx.enter_context(tc.tile_pool(name="atp", bufs=4))
        outp = ctx.enter_context(tc.tile_pool(name="outp", bufs=1))
        ps_proj = ctx.enter_context(tc.tile_pool(name="ps_proj", bufs=2, space="PSUM"))
        ps_sm = ctx.enter_context(tc.tile_pool(name="ps_sm", bufs=2, space="PSUM"))
        ps_tr = ctx.enter_context(tc.tile_pool(name="ps_tr", bufs=2, space="PSUM"))
        ps_mm = ctx.enter_context(tc.tile_pool(name="ps_mm", bufs=2, space="PSUM"))
        dram = ctx.enter_context(tc.tile_pool(name="dram", bufs=1, space="DRAM"))

        # ---------------- constants & weights to SBUF ----------------
        xT = [cst.tile([P, XROWS], F32R, tag=f"xT{i}", name=f"xT{i}") for i in range(4)]
        for i in range(4):
            nc.sync.dma_start(xT[i][:], dd["xT"].ap()[i * P:(i + 1) * P, :])
        vwT = [cst.tile([P, C], F32R, tag=f"vwT{i}", name=f"vwT{i}") for i in range(4)]
        kwT = [cst.tile([P, C], F32R, tag=f"kwT{i}", name=f"kwT{i}") for i in range(4)]
        owT = [cst.tile([P, C], F32, tag=f"owT{i}", name=f"owT{i}") for i in range(4)]
        woT = [cst.tile([P, 16], F32R, tag=f"woT{i}", name=f"woT{i}") for i in range(4)]
        sw1T = [cst.tile([P, P], F32, tag=f"sw1T{i}", name=f"sw1T{i}") for i in range(4)]
        for i in range(4):
            sl = slice(i * P, (i + 1) * P)
            nc.sync.dma_start(vwT[i][:], dd["vwT"].ap()[sl, :])
            nc.sync.dma_start(kwT[i][:], dd["kwT"].ap()[sl, :])
            nc.sync.dma_start(owT[i][:], dd["owT"].ap()[sl, :])
            nc.sync.dma_start(woT[i][:], dd["woT"].ap()[sl, :])
            nc.sync.dma_start(sw1T[i][:], dd["sw1T"].ap()[sl, :])
        sw2T = cst.tile([P, C], BF16)
        nc.sync.dma_start(sw2T[:], dd["sw2T"].ap())
        wob = cst.tile([1, 16], F32R)
        kb = cst.tile([1, C], F32R)
        vb = cst.tile([1, C], F32R)
        wog = cst.tile([P, 16], F32)
        kg = cst.tile([P, C], F32)
        lpos = cst.tile([P, LT], F32)
        nc.sync.dma_start(wob[:], dd["wob"].ap())
        nc.sync.dma_start(kb[:], dd["kb"].ap())
        nc.sync.dma_start(vb[:], dd["vb"].ap())
        nc.sync.dma_start(wog[:], dd["wog"].ap())
        nc.sync.dma_start(kg[:], dd["kg"].ap())
        nc.sync.dma_start(lpos[:], dd["lpos"].ap())

        eps_t = cst.tile([P, 1], F32)
        nc.vector.memset(eps_t[:], EPS)
        ones1 = cst.tile([1, P], F32R)
        nc.sync.dma_start(ones1[:], dd["ones"].ap())
        identb = cst.tile([P, P], BF16)
        make_identity(nc, identb[:])

        # iotas
        iotaS = cst.tile([P, H, 13], F32)       # s value -6..6 per head
        it0 = cst.tile([P, H, 13], I32)
        nc.gpsimd.iota(it0[:], pattern=[[0, H], [1, 13]], base=-6, channel_multiplier=0)
        nc.vector.tensor_copy(iotaS[:], it0[:])
        iotaA5 = cst.tile([P, H, 5], F32)       # a = 1..5 per head
        it1 = cst.tile([P, H, 5], I32)
        nc.gpsimd.iota(it1[:], pattern=[[0, H], [1, 5]], base=1, channel_multiplier=0)
        nc.vector.tensor_copy(iotaA5[:], it1[:])
        iotaA7 = cst.tile([P, H, 7], F32)       # a = 0..6 per head
        it2 = cst.tile([P, H, 7], I32)
        nc.gpsimd.iota(it2[:], pattern=[[0, H], [1, 7]], base=0, channel_multiplier=0)
        nc.vector.tensor_copy(iotaA7[:], it2[:])
        # scatter index base: h*256 + lam + 46 + a  (a = s+6: 0..12)
        iotaIDX = cst.tile([P, H, 13], F32)
        it3 = cst.tile([P, H, 13], I32)
        nc.gpsimd.iota(it3[:], pattern=[[0, 2], [256, 4], [1, 13]], base=46, channel_multiplier=1)
        nc.vector.tensor_copy(iotaIDX[:], it3[:])

        # ---------------- v projection over halo (9 tiles, interleaved) ----------------
        v_sb = {}

        def emit_vproj(t):
            vp = ps_proj.tile([P, C], F32, tag="proj", name="vp")
            col = slice(t * P, (t + 1) * P)
            for kc in range(4):
                nc.tensor.matmul(vp[:], xT[kc][:, col],
                                 vwT[kc][:], start=(kc == 0), stop=False)
            nc.tensor.matmul(vp[:], ones1[:], vb[:],
                             start=False, stop=True)
            vt = vsb.tile([P, C], BF16, tag="v", name=f"v{t}")
            nc.scalar.copy(vt[:], vp[:])
            v_sb[t] = vt

        for t in range(2):
            emit_vproj(t)
        emit_vproj(VT - 1)

        outT_sb = [outp.tile([P, LSH], BF16, tag=f"outT{cc}", name=f"outT{cc}") for cc in range(4)]

        # ---------------- main per-l-tile loop ----------------
        vv2 = []

        def emit_vv():
            # vv = 0.001*v[L-2] + 0.999*v[L-1]  (local rows 1086/1087 = tile 8 parts 62/63)
            vv = cst.tile([1, C], BF16, name="vv")
            vvf = cst.tile([1, C], F32, name="vvf")
            vt2 = cst.tile([1, C], BF16, name="vt2")
            vt3 = cst.tile([1, C], BF16, name="vt3")
            nc.sync.dma_start(vt2[:], v_sb[8][62:63, :])
            nc.sync.dma_start(vt3[:], v_sb[8][63:64, :])
            nc.vector.tensor_scalar(out=vvf[:], in0=vt2[:], scalar1=0.001,
                                    scalar2=None, op0=OP.mult)
            nc.vector.scalar_tensor_tensor(out=vv[:], in0=vt3[:], scalar=0.999,
                                           in1=vvf[:], op0=OP.mult, op1=OP.add)
            for cc2 in range(4):
                v2 = cst.tile([2, P], BF16, tag=f"vv2_{cc2}", name=f"vv2_{cc2}")
                nc.vector.memset(v2[:], 0.0)
                nc.vector.tensor_copy(v2[0:1, 0:D], vv[:, (2 * cc2) * D:(2 * cc2 + 1) * D])
                nc.sync.dma_start(v2[1:2, D:2 * D], vv[:, (2 * cc2 + 1) * D:(2 * cc2 + 2) * D])
                vv2.append(v2)

        emit_vv()
        for lt in range(LT):
            if lt + 2 < VT - 1:
                emit_vproj(lt + 2)
            xcol = slice(HALO + lt * P, HALO + (lt + 1) * P)

            # window/offset projection [P, 16]
            wop = ps_sm.tile([P, 16], F32, tag="tiny", name="wop")
            for kc in range(4):
                nc.tensor.matmul(wop[:], xT[kc][:, xcol],
                                 woT[kc][:], start=(kc == 0), stop=False)
            nc.tensor.matmul(wop[:], ones1[:], wob[:],
                             start=False, stop=True)
            # kernel projection [P, 512]
            kp = ps_proj.tile([P, C], F32, tag="proj", name="kp")
            for kc in range(4):
                nc.tensor.matmul(kp[:], xT[kc][:, xcol],
                                 kwT[kc][:], start=(kc == 0), stop=False)
            nc.tensor.matmul(kp[:], ones1[:], kb[:],
                             start=False, stop=True)

            # --- rmsnorm window/offset, sigmoid/tanh ---
            wsq = wk.tile([P, 16], F32, tag="wsq")
            nc.scalar.activation(wsq[:], wop[:], ACT.Square)
            wss = wk.tile([P, 2], F32, tag="wss")
            nc.vector.tensor_reduce(out=wss[:], in_=wsq[:].rearrange("p (g h) -> p g h", g=2),
                                    axis=mybir.AxisListType.X, op=OP.add)
            wrstd = wk.tile([P, 2], F32, tag="wrstd")
            nc.scalar.activation(wrstd[:], wss[:], ACT.Sqrt, bias=eps_t[:], scale=1.0 / 8)
            nc.vector.reciprocal(wrstd[:], wrstd[:])
            won = wk.tile([P, 16], F32, tag="won")
            nc.vector.tensor_tensor(
                out=won[:].rearrange("p (g h) -> p g h", g=2),
                in0=wop[:].rearrange("p (g h) -> p g h", g=2),
                in1=_bcast(wrstd[:][:, :, None], [P, 2, 8]), op=OP.mult)
            nc.vector.tensor_tensor(out=won[:], in0=won[:], in1=wog[:], op=OP.mult)
            win_raw = wk.tile([P, H], F32, tag="win_raw")
            nc.scalar.activation(win_raw[:], won[:, 0:8], ACT.Sigmoid)
            cth = wk.tile([P, H], F32, tag="cth")
            nc.scalar.activation(cth[:], won[:, 8:16], ACT.Tanh)
            cc_ = wk.tile([P, H], F32, tag="cc_")
            nc.vector.tensor_scalar(out=cc_[:], in0=cth[:], scalar1=12.0, scalar2=None,
                                    op0=OP.mult)
            hwv = wk.tile([P, H], F32, tag="hwv")
            nc.vector.tensor_scalar(out=hwv[:], in0=win_raw[:], scalar1=5.5, scalar2=0.5,
                                    op0=OP.mult, op1=OP.add)
            tinv = wk.tile([P, H], F32, tag="tinv")
            nc.vector.reciprocal(tinv[:], hwv[:])

            # c0 = floor(c), phi = c - c0  (via +16 trunc with round-fix)
            cp16 = wk.tile([P, H], F32, tag="cp16")
            nc.vector.tensor_scalar(out=cp16[:], in0=cc_[:], scalar1=16.0, scalar2=None,
                                    op0=OP.add)
            ci = wk.tile([P, H], I32, tag="ci")
            nc.vector.tensor_copy(ci[:], cp16[:])
            cf = wk.tile([P, H], F32, tag="cf")
            nc.vector.tensor_copy(cf[:], ci[:])
            cgt = wk.tile([P, H], F32, tag="cgt")
            nc.vector.tensor_tensor(out=cgt[:], in0=cf[:], in1=cp16[:], op=OP.is_gt)
            c0p16 = wk.tile([P, H], F32, tag="c0p16")
            nc.vector.tensor_tensor(out=c0p16[:], in0=cf[:], in1=cgt[:], op=OP.subtract)
            phi = wk.tile([P, H], F32, tag="phi")
            nc.vector.tensor_tensor(out=phi[:], in0=cp16[:], in1=c0p16[:], op=OP.subtract)
            c012 = wk.tile([P, H], F32, tag="c012")   # c0 + 12
            nc.vector.tensor_scalar(out=c012[:], in0=c0p16[:], scalar1=4.0, scalar2=None,
                                    op0=OP.subtract)

            # --- rmsnorm kernel + silu ---
            ksq = wk.tile([P, C], F32, tag="ksq")
            nc.scalar.activation(ksq[:], kp[:], ACT.Square)
            kss = wk.tile([P, 1], F32, tag="kss")
            nc.vector.tensor_reduce(out=kss[:], in_=ksq[:], axis=mybir.AxisListType.X,
                                    op=OP.add)
            krstd = wk.tile([P, 1], F32, tag="krstd")
            nc.scalar.activation(krstd[:], kss[:], ACT.Sqrt, bias=eps_t[:], scale=1.0 / C)
            nc.vector.reciprocal(krstd[:], krstd[:])
            kn = wk.tile([P, C], F32, tag="kn")
            nc.vector.tensor_scalar(out=kn[:], in0=kp[:], scalar1=krstd[:], scalar2=None,
                                    op0=OP.mult)
            nc.vector.tensor_tensor(out=kn[:], in0=kn[:], in1=kg[:], op=OP.mult)
            kern = wk.tile([P, H, K], BF16, tag="kern")
            if USE_SILU:
                nc.scalar.activation(kern[:].rearrange("p h k -> p (h k)"), kn[:], ACT.Silu)
            else:
                ksg = wk.tile([P, C], F32, tag="ksg")
                nc.scalar.activation(ksg[:], kn[:], ACT.Sigmoid)
                nc.vector.tensor_tensor(out=kern[:].rearrange("p h k -> p (h k)"), in0=kn[:],
                                        in1=ksg[:], op=OP.mult)

            # D table: Dt[k] = kern[k+1] - kern[k], Dt[63] = 0
            Dt = wk.tile([P, H, K], BF16, tag="Dt")
            nc.vector.memset(Dt[:, :, 63:64], 0.0)
            nc.vector.tensor_tensor(out=Dt[:, :, 0:63], in0=kern[:, :, 1:64],
                                    in1=kern[:, :, 0:63], op=OP.subtract)

            # --- interpolation indices (a = 1..5; a=6 always clips to slot 63) ---
            A5 = 5
            npos = wk.tile([P, H, A5], F32, tag="npos")
            nc.vector.tensor_tensor(out=npos[:], in0=iotaA5[:],
                                    in1=_bcast(tinv[:][:, :, None], [P, H, A5]), op=OP.mult)
            nc.vector.tensor_scalar(out=npos[:], in0=npos[:], scalar1=1.0, scalar2=float(K - 1),
                                    op0=OP.min, op1=OP.mult)
            ii = wk.tile([P, H, A5], I32, tag="ii")
            nc.vector.tensor_copy(ii[:], npos[:])
            tf = wk.tile([P, H, A5], F32, tag="tf")
            nc.vector.tensor_copy(tf[:], ii[:])
            tgt = wk.tile([P, H, A5], F32, tag="tgt")
            nc.vector.tensor_tensor(out=tgt[:], in0=tf[:], in1=npos[:], op=OP.is_gt)
            idxf = wk.tile([P, H, A5], F32, tag="idxf")
            nc.vector.tensor_tensor(out=idxf[:], in0=tf[:], in1=tgt[:], op=OP.subtract)
            nc.vector.tensor_scalar(out=idxf[:], in0=idxf[:], scalar1=float(K - 2),
                                    scalar2=None, op0=OP.min)
            w_c = wk.tile([P, H, A5], F32, tag="w_c")
            nc.vector.tensor_tensor(out=w_c[:], in0=npos[:], in1=idxf[:], op=OP.subtract)

            # bits of idxf as int16 masks, msb first
            bits = []
            rcur = idxf
            for j, bv in enumerate([32.0, 16.0, 8.0, 4.0, 2.0, 1.0]):
                bi = wk.tile([P, H, A5], I16, tag=f"biti{j}")
                nc.vector.tensor_scalar(out=bi[:], in0=rcur[:], scalar1=bv, scalar2=None,
                                        op0=OP.is_ge)
                bits.append(bi)
                if j < 5:
                    rnew = wk.tile([P, H, A5], F32, tag=f"rem{j}")
                    nc.vector.scalar_tensor_tensor(out=rnew[:], in0=bi[:], scalar=-bv,
                                                   in1=rcur[:], op0=OP.mult, op1=OP.add)
                    rcur = rnew

            # --- halving gather of (kern, Dt) pairs at idxf ---
            st = wk.tile([P, H, A5, 2, 32], BF16, tag="st")
            nc.vector.tensor_copy(st[:, :, :, 0, :],
                                  _bcast(kern[:][:, :, None, 0:32], [P, H, A5, 32]))
            nc.vector.tensor_copy(st[:, :, :, 1, :],
                                  _bcast(Dt[:][:, :, None, 0:32], [P, H, A5, 32]))
            nc.vector.copy_predicated(st[:, :, :, 0, :],
                                      _bcast(bits[0][:][:, :, :, None], [P, H, A5, 32]),
                                      _bcast(kern[:][:, :, None, 32:64], [P, H, A5, 32]))
            nc.vector.copy_predicated(st[:, :, :, 1, :],
                                      _bcast(bits[0][:][:, :, :, None], [P, H, A5, 32]),
                                      _bcast(Dt[:][:, :, None, 32:64], [P, H, A5, 32]))
            w = 16
            for j in range(1, 6):
                nc.vector.copy_predicated(
                    st[:, :, :, :, 0:w],
                    _bcast(bits[j][:][:, :, :, None, None], [P, H, A5, 2, w]),
                    st[:, :, :, :, w:2 * w])
                w //= 2
            # g0 = st[...,0,0], g1 = st[...,1,0]
            lerp = wk.tile([P, H, A5], F32, tag="lerp")
            nc.vector.tensor_tensor(out=lerp[:], in0=w_c[:], in1=st[:, :, :, 1, 0],
                                    op=OP.mult)
            nc.vector.tensor_tensor(out=lerp[:], in0=lerp[:], in1=st[:, :, :, 0, 0],
                                    op=OP.add)
            # ker7 = 1 + max(lerp, 0); col 0 from kern[...,0]; col 6 from kern[...,63]
            ker7 = wk.tile([P, H, 7], F32, tag="ker7")
            nc.vector.tensor_scalar(out=ker7[:, :, 1:6], in0=lerp[:], scalar1=0.0,
                                    scalar2=1.0, op0=OP.max, op1=OP.add)
            nc.vector.tensor_scalar(out=ker7[:, :, 0:1], in0=kern[:, :, 0:1], scalar1=0.0,
                                    scalar2=1.0, op0=OP.max, op1=OP.add)
            nc.vector.tensor_scalar(out=ker7[:, :, 6:7], in0=kern[:, :, 63:64], scalar1=0.0,
                                    scalar2=1.0, op0=OP.max, op1=OP.add)

            # win7 = exp(-(a * tinv)^2)
            rel7 = wk.tile([P, H, 7], F32, tag="rel7")
            nc.vector.tensor_tensor(out=rel7[:], in0=iotaA7[:],
                                    in1=_bcast(tinv[:][:, :, None], [P, H, 7]), op=OP.mult)
            nc.vector.tensor_tensor(out=rel7[:], in0=rel7[:], in1=rel7[:], op=OP.mult)
            win7 = wk.tile([P, H, 7], F32, tag="win7")
            nc.scalar.activation(win7[:], rel7[:], ACT.Exp, scale=-1.0)
            wt7 = wk.tile([P, H, 7], F32, tag="wt7")
            nc.vector.tensor_tensor(out=wt7[:], in0=ker7[:], in1=win7[:], op=OP.mult)

            # expand to 13 taps (s order -6..6)
            w13 = wk.tile([P, H, 13], F32, tag="w13")
            nc.vector.tensor_copy(w13[:, :, 0:7], wt7[:, :, ::-1])
            nc.vector.tensor_copy(w13[:, :, 6:13], wt7[:, :, 0:7])

            # validity / special masks
            nb13 = wk.tile([P, H, 13], F32, tag="nb13")
            lc = wk.tile([P, H], F32, tag="lc")
            nc.vector.tensor_tensor(out=lc[:], in0=cc_[:],
                                    in1=_bcast(lpos[:, lt:lt + 1], [P, H]), op=OP.add)
            nc.vector.tensor_tensor(out=nb13[:], in0=iotaS[:],
                                    in1=_bcast(lc[:][:, :, None], [P, H, 13]), op=OP.add)
            vhi = wk.tile([P, H, 13], F32, tag="vhi")
            nc.vector.tensor_scalar(out=vhi[:], in0=nb13[:], scalar1=float(L), scalar2=None,
                                    op0=OP.is_lt)
            valid = wk.tile([P, H, 13], F32, tag="valid")
            nc.vector.scalar_tensor_tensor(out=valid[:], in0=nb13[:], scalar=0.0,
                                           in1=vhi[:], op0=OP.is_ge, op1=OP.mult)
            spec = wk.tile([P, H, 13], F32, tag="spec")
            nc.vector.scalar_tensor_tensor(out=spec[:], in0=nb13[:], scalar=float(NB_HI),
                                           in1=vhi[:], op0=OP.is_gt, op1=OP.mult)

            wv = wk.tile([P, H, 13], F32, tag="wv")
            nc.vector.tensor_tensor(out=wv[:], in0=w13[:], in1=valid[:], op=OP.mult)
            wsum = wk.tile([P, H], F32, tag="wsum")
            nc.vector.tensor_reduce(out=wsum[:], in_=wv[:], axis=mybir.AxisListType.X,
                                    op=OP.add)
            rw = wk.tile([P, H], F32, tag="rw")
            nc.vector.tensor_scalar(out=rw[:], in0=wsum[:], scalar1=1.0, scalar2=None,
                                    op0=OP.max)
            nc.vector.reciprocal(rw[:], rw[:])

            wsp = wk.tile([P, H, 13], F32, tag="wsp")
            nc.vector.tensor_tensor(out=wsp[:], in0=wv[:], in1=spec[:], op=OP.mult)
            wint = wk.tile([P, H, 13], F32, tag="wint")
            nc.vector.tensor_tensor(out=wint[:], in0=wv[:], in1=wsp[:], op=OP.subtract)
            wspec = wk.tile([P, H], F32, tag="wspec")
            nc.vector.tensor_reduce(out=wspec[:], in_=wsp[:], axis=mybir.AxisListType.X,
                                    op=OP.add)
            wspec_s = wk.tile([P, H], F32, tag="wspec_s")
            nc.vector.tensor_tensor(out=wspec_s[:], in0=wspec[:], in1=rw[:], op=OP.mult)

            om = wk.tile([P, H], F32, tag="om")
            nc.vector.tensor_scalar(out=om[:], in0=phi[:], scalar1=-1.0, scalar2=1.0,
                                    op0=OP.mult, op1=OP.add)
            uf = wk.tile([P, H], F32, tag="uf")
            nc.vector.tensor_tensor(out=uf[:], in0=om[:], in1=rw[:], op=OP.mult)
            uc = wk.tile([P, H], F32, tag="uc")
            nc.vector.tensor_tensor(out=uc[:], in0=phi[:], in1=rw[:], op=OP.mult)

            wf = wk.tile([P, H, 13], BF16, tag="wf")
            nc.vector.tensor_tensor(out=wf[:], in0=wint[:],
                                    in1=_bcast(uf[:][:, :, None], [P, H, 13]), op=OP.mult)
            wcc = wk.tile([P, H, 13], BF16, tag="wcc")
            nc.vector.tensor_tensor(out=wcc[:], in0=wint[:],
                                    in1=_bcast(uc[:][:, :, None], [P, H, 13]), op=OP.mult)

            # scatter indices
            idxf32 = wk.tile([P, H, 13], F32, tag="idxf32")
            nc.vector.tensor_tensor(out=idxf32[:], in0=iotaIDX[:],
                                    in1=_bcast(c012[:][:, :, None], [P, H, 13]), op=OP.add)
            i16 = wk.tile([P, H, 13], I16, tag="i16")
            nc.vector.tensor_copy(i16[:], idxf32[:])
            i16b = wk.tile([P, H, 13], I16, tag="i16b")
            nc.vector.tensor_scalar(out=i16b[:], in0=i16[:], scalar1=1, scalar2=None,
                                    op0=OP.add)

            A0 = wk.tile([P, H, 256], BF16, tag="A0")
            A1 = wk.tile([P, H, 256], BF16, tag="A1")
            for hb in range(2):
                hs = slice(hb * 4, hb * 4 + 4)
                nc.gpsimd.local_scatter(
                    A0[:, hs, :].rearrange("p h w -> p (h w)"),
                    wf[:, hs, :].rearrange("p h a -> p (h a)"),
                    i16[:, hs, :].rearrange("p h a -> p (h a)"),
                    channels=P, num_elems=4 * 256, num_idxs=4 * 13)
                nc.gpsimd.local_scatter(
                    A1[:, hs, :].rearrange("p h w -> p (h w)"),
                    wcc[:, hs, :].rearrange("p h a -> p (h a)"),
                    i16b[:, hs, :].rearrange("p h a -> p (h a)"),
                    channels=P, num_elems=4 * 256, num_idxs=4 * 13)
            A = wk.tile([P, H, 256], BF16, tag="A")
            nc.vector.tensor_tensor(out=A[:], in0=A0[:], in1=A1[:], op=OP.add)

            # wspecT head-pair rows (for the rank-2 edge correction): 4 x [2, P]
            wspb = wk.tile([P, H], BF16, tag="wspb")
            nc.vector.tensor_copy(wspb[:], wspec_s[:])
            wspT2 = []
            for cc2 in range(4):
                wsp_ps = ps_sm.tile([2, P], BF16, tag="tiny", name="wsp_ps")
                nc.tensor.transpose(wsp_ps[:], wspb[:, 2 * cc2:2 * cc2 + 2], identb[:])
                wt_ = wk.tile([2, P], BF16, tag=f"wspT2_{cc2}", name=f"wspT2_{cc2}")
                nc.scalar.copy(wt_[:], wsp_ps[:])
                wspT2.append(wt_)

            # --- transpose A blocks and banded matmul ---
            for cci in range(4):
                po = ps_mm.tile([P, P], F32, tag="mm", name="po")
                tp = ps_tr.tile([P, 4, P], BF16, tag="tp")
                at = atp.tile([P, 4, P], BF16, tag="at")
                for hh in range(2):
                    h = 2 * cci + hh
                    for blk in range(2):
                        nc.tensor.transpose(tp[:, 2 * hh + blk, :],
                                            A[:, h, blk * P:(blk + 1) * P], identb[:])
                nc.scalar.copy(at[:], tp[:])
                for hh in range(2):
                    h = 2 * cci + hh
                    prange = slice(hh * 64, hh * 64 + 64)
                    for blk in range(2):
                        nc.tensor.matmul(po[prange, :],
                                         v_sb[lt + blk][:, h * D:(h + 1) * D],
                                         at[:, 2 * hh + blk, :], start=(blk == 0),
                                         stop=(blk == 1))
                nc.tensor.matmul(po[:], vv2[cci][:], wspT2[cci][:],
                                 start=False, stop=True, skip_group_check=True)
                nc.scalar.copy(outT_sb[cci][:, lt * P:(lt + 1) * P], po[:])

        # ---------------- SE partial sums + AllGather + local sum ----------------
        seacc = cst.tile([P, 4], F32)
        for cci in range(4):
            nc.vector.tensor_reduce(out=seacc[:, cci:cci + 1],
                                    in_=outT_sb[cci][:],
                                    axis=mybir.AxisListType.X, op=OP.add)
        if skip_cc:
            armean = seacc
        else:
            cci_d = dram.tile([P, 4], F32)
            cco_d = dram.tile([4, P, 4], F32)
            nc.gpsimd.dma_start(cci_d[:], seacc[:])
            nc.gpsimd.collective_compute(
                "AllGather", OP.bypass,
                replica_groups=[[0, 1, 2, 3], [4, 5, 6, 7]],
                ins=[cci_d[:].opt()], outs=[cco_d[:].opt()])
            # gathered along partition axis: [4 ranks, 128, 4] -> partitions 0..511?
            ag = cst.tile([P, 4, 4], F32)
            nc.sync.dma_start(ag[:], cco_d[:].rearrange("r p f -> p r f"))
            armean = cst.tile([P, 4], F32)
            nc.vector.tensor_tensor(out=armean[:], in0=ag[:, 0, :], in1=ag[:, 1, :],
                                    op=OP.add)
            nc.vector.tensor_tensor(out=armean[:], in0=armean[:], in1=ag[:, 2, :],
                                    op=OP.add)
            nc.vector.tensor_tensor(out=armean[:], in0=armean[:], in1=ag[:, 3, :],
                                    op=OP.add)

        # ---------------- SE MLP (transposed throughout) ----------------
        hidT_ps = ps_sm.tile([P, 1], F32, tag="tiny", name="hidT_ps")
        for cci in range(4):
            nc.tensor.matmul(hidT_ps[:], sw1T[cci][:], armean[:, cci:cci + 1],
                             start=(cci == 0), stop=(cci == 3))
        hidT = cst.tile([P, 1], BF16)
        if USE_SILU:
            nc.scalar.activation(hidT[:], hidT_ps[:], ACT.Silu)
        else:
            hsg = cst.tile([P, 1], F32)
            nc.scalar.activation(hsg[:], hidT_ps[:], ACT.Sigmoid)
            nc.vector.tensor_tensor(out=hidT[:], in0=hidT_ps[:], in1=hsg[:], op=OP.mult)
        owb = []
        for cci in range(4):
            scT_ps = ps_sm.tile([P, 1], F32, tag="tiny", name="scT_ps")
            nc.tensor.matmul(scT_ps[:], sw2T[:, cci * P:(cci + 1) * P], hidT[:],
                             start=True, stop=True)
            scT = cst.tile([P, 1], F32, tag=f"scT{cci}", name=f"scT{cci}")
            nc.scalar.activation(scT[:], scT_ps[:], ACT.Sigmoid)
            ow = cst.tile([P, C], BF16, tag=f"owb{cci}", name=f"owb{cci}")
            nc.vector.tensor_scalar(out=ow[:], in0=owT[cci][:], scalar1=scT[:],
                                    scalar2=None, op0=OP.mult)
            owb.append(ow)

        # ---------------- final out_w matmul + silu + DMA out ----------------
        # out[l, cout] = sum_cin out_preT[cin, l] * ow'[cin, cout]  (lhsT = out_preT)
        for lt in range(LT):
            lsl = slice(lt * P, (lt + 1) * P)
            pf_ = ps_mm.tile([P, C], F32, tag="mm", name="pf_")
            for cci in range(4):
                nc.tensor.matmul(pf_[:], outT_sb[cci][:, lsl], owb[cci][:],
                                 start=(cci == 0), stop=(cci == 3))
            fo = wk.tile([P, C], F32, tag="fo")
            if USE_SILU:
                nc.scalar.activation(fo[:], pf_[:], ACT.Silu)
            else:
                nc.scalar.activation(fo[:], pf_[:], ACT.Sigmoid)
                nc.vector.tensor_tensor(out=fo[:], in0=fo[:], in1=pf_[:], op=OP.mult)
            nc.sync.dma_start(dd["out"].ap()[lsl, :], fo[:])


def make_in_maps(inputs, n_cores=8):
    x = np.ascontiguousarray(inputs["x"], dtype=np.float32)
    window_w = inputs["window_w"]; window_b = inputs["window_b"]
    window_gamma = inputs["window_gamma"]
    offset_w = inputs["offset_w"]; offset_b = inputs["offset_b"]
    offset_gamma = inputs["offset_gamma"]
    kernel_w = inputs["kernel_w"]; kernel_b = inputs["kernel_b"]
    kernel_gamma = inputs["kernel_gamma"]
    v_w = inputs["v_w"]; v_b = inputs["v_b"]
    se_w1 = inputs["se_w1"]; se_w2 = inputs["se_w2"]; out_w = inputs["out_w"]

    woT = np.concatenate([window_w, offset_w], 0).T.astype(np.float32)      # (512,16)
    wob = np.concatenate([window_b, offset_b])[None].astype(np.float32)     # (1,16)
    wog = np.tile(np.concatenate([window_gamma, offset_gamma])[None], (P, 1)).astype(np.float32)
    kwT = np.ascontiguousarray(kernel_w.T, np.float32)
    kb = kernel_b[None].astype(np.float32)
    kgm = np.tile(kernel_gamma[None], (P, 1)).astype(np.float32)
    vwT = np.ascontiguousarray(v_w.T, np.float32)
    vbm = v_b[None].astype(np.float32)
    sw1T = np.ascontiguousarray(se_w1.T, np.float32) / np.float32(L)
    sw2T = np.ascontiguousarray(se_w2.T).astype(ml_dtypes.bfloat16)
    owT = np.ascontiguousarray(out_w.T, np.float32)

    in_maps = []
    for i in range(n_cores):
        b, q = divmod(i, 4)
        lo = q * LSH - HALO
        xpad = np.zeros((XROWS, C), np.float32)
        s0, s1 = max(lo, 0), min(lo + XROWS, L)
        xpad[s0 - lo:s1 - lo] = x[b, s0:s1]
        xT = np.ascontiguousarray(xpad.T)
        lpos = (q * LSH + np.arange(LSH, dtype=np.float32)).reshape(LT, P).T.copy()
        in_maps.append(dict(
            xT=xT, lpos=lpos, woT=woT, wob=wob, wog=wog, kwT=kwT, kb=kb,
            kg=kgm, vwT=vwT, vb=vbm, sw1T=sw1T, sw2T=sw2T, owT=owT,
            ones=np.ones((1, P), np.float32),
        ))
    return in_maps


def kernel(**inputs) -> np.ndarray:
    if "graph" not in _GRAPH_CACHE:
        _GRAPH_CACHE["graph"] = build_graph(8)
    nc = _GRAPH_CACHE["graph"]
    in_maps = make_in_maps(inputs, 8)
    res = run_bass_kernel_spmd(nc, in_maps, core_ids=list(range(8)))
    out = np.zeros((B, L, C), np.float32)
    for i in range(8):
        b, q = divmod(i, 4)
        out[b, q * LSH:(q + 1) * LSH] = res.results[i]["out"]
    return out


if __name__ == "__main__":
    import reference
    inputs = {k: np.asarray(v) for k, v in reference.setup_inputs().items()}
    got = kernel(**inputs)
    import jax.numpy as jnp
    exp = np.asarray(reference.reference(**{k: jnp.asarray(v) for k, v in inputs.items()}))
    rel = np.linalg.norm(got - exp) / np.linalg.norm(exp)
    print("Relative error:", rel)


# revision 35
# speedup vs baseline: 20.9734x; 1.0595x over previous
"""AdaptiveLocalConv Trainium2 kernel — 8-core SPMD.

Sharding: (batch, seq-quarter) -> 8 shards of 1024 tokens (+64 halo each side
for the deformable gather, reach <= +-19).

Per-core pipeline:
  - 4 projections from x via PE (f32r), x passed pre-transposed [C, 1152].
  - per-(token,head): 13 deformable taps; kernel-table interpolation via a
    log2 halving-gather on DVE; taps placed into a banded matrix A
    [token, head, 256-slot J-band] with gpsimd local_scatter (bf16).
  - banded matmul out^T[d, l] = sum_J v[J, d] * A^T[J, l] on PE after
    PE-transposing A blocks; sequence-end clamp handled exactly by a rank-1
    correction matmul.
  - squeeze-excite via a 4-core AllReduce of the per-core partial mean (2KB),
    SE scale folded into out_w columns; final out_w matmul in transposed
    layout; silu; DMA out with a transposing access pattern.
"""
import sys
if "/opt/trn_rl_repo" not in sys.path:
    sys.path.insert(0, "/opt/trn_rl_repo")

import numpy as np
import ml_dtypes

import concourse.bass as bass
import concourse.mybir as mybir
from concourse import bacc
from concourse.tile import TileContext
from concourse.bass_utils import run_bass_kernel_spmd
from concourse.masks import make_identity

F32 = mybir.dt.float32
F32R = mybir.dt.float32r
BF16 = mybir.dt.bfloat16
I32 = mybir.dt.int32
I16 = mybir.dt.int16
OP = mybir.AluOpType
ACT = mybir.ActivationFunctionType

B, L, C, H, K, D = 2, 4096, 512, 8, 64, 64
P = 128
HALO = 64
LSH = 1024          # tokens per core
XROWS = LSH + 2 * HALO   # 1152
LT = LSH // P       # 8 own l-tiles
VT = XROWS // P     # 9 v tiles
NB_HI = np.float32(L - 1.001)
EPS = 1e-6

_GRAPH_CACHE = {}
USE_SILU = True  # sim lacks Silu; test_sim sets False


def _bcast(ap, shape):
    return ap.broadcast_to(shape)


def build_graph(n_cores=8, skip_cc=False):
    nc = bacc.Bacc("TRN2", target_bir_lowering=False, debug=False,
                   enable_asserts=False, num_devices=n_cores)

    # ---------------- DRAM parameters ----------------
    xT_d = nc.dram_tensor("xT", [C, XROWS], F32R, kind="ExternalInput")
    lpos_d = nc.dram_tensor("lpos", [P, LT], F32, kind="ExternalInput")
    woT_d = nc.dram_tensor("woT", [C, 16], F32R, kind="ExternalInput")
    wob_d = nc.dram_tensor("wob", [1, 16], F32R, kind="ExternalInput")
    wog_d = nc.dram_tensor("wog", [P, 16], F32, kind="ExternalInput")
    kwT_d = nc.dram_tensor("kwT", [C, C], F32R, kind="ExternalInput")
    kb_d = nc.dram_tensor("kb", [1, C], F32R, kind="ExternalInput")
    kg_d = nc.dram_tensor("kg", [P, C], F32, kind="ExternalInput")
    vwT_d = nc.dram_tensor("vwT", [C, C], F32R, kind="ExternalInput")
    vb_d = nc.dram_tensor("vb", [1, C], F32R, kind="ExternalInput")
    sw1T_d = nc.dram_tensor("sw1T", [C, P], F32, kind="ExternalInput")   # pre-scaled by 1/L
    sw2T_d = nc.dram_tensor("sw2T", [P, C], BF16, kind="ExternalInput")
    owT_d = nc.dram_tensor("owT", [C, C], F32, kind="ExternalInput")
    ones_d = nc.dram_tensor("ones", [1, P], F32R, kind="ExternalInput")
    out_d = nc.dram_tensor("out", [LSH, C], F32, kind="ExternalOutput")

    with TileContext(nc) as tc:
        _build_body(nc, tc, dict(
            xT=xT_d, lpos=lpos_d, woT=woT_d, wob=wob_d, wog=wog_d,
            kwT=kwT_d, kb=kb_d, kg=kg_d, vwT=vwT_d, vb=vb_d,
            sw1T=sw1T_d, sw2T=sw2T_d, owT=owT_d, out=out_d, ones=ones_d,
        ), skip_cc=skip_cc)
    nc.compile()
    return nc


def _build_body(nc, tc, dd, skip_cc=False):
    import contextlib
    ctx = contextlib.ExitStack()
    with ctx:
        cst = ctx.enter_context(tc.tile_pool(name="cst", bufs=1))
        vsb = ctx.enter_context(tc.tile_pool(name="vsb", bufs=VT))
        wk = ctx.enter_context(tc.tile_pool(name="wk", bufs=2))
        atp = ctx.enter_context(tc.tile_pool(name="atp", bufs=4))
        outp = ctx.enter_context(tc.tile_pool(name="outp", bufs=1))
        ps_proj = ctx.enter_context(tc.tile_pool(name="ps_proj", bufs=2, space="PSUM"))
        ps_sm = ctx.enter_context(tc.tile_pool(name="ps_sm", bufs=2, space="PSUM"))
        ps_tr = ctx.enter_context(tc.tile_pool(name="ps_tr", bufs=2, space="PSUM"))
        ps_mm = ctx.enter_context(tc.tile_pool(name="ps_mm", bufs=2, space="PSUM"))
        dram = ctx.enter_context(tc.tile_pool(name="dram", bufs=1, space="DRAM"))

        # ---------------- constants & weights to SBUF ----------------
        xT = [cst.tile([P, XROWS], F32R, tag=f"xT{i}", name=f"xT{i}") for i in range(4)]
        for i in range(4):
            nc.sync.dma_start(xT[i][:], dd["xT"].ap()[i * P:(i + 1) * P, :])
        vwT = [cst.tile([P, C], F32R, tag=f"vwT{i}", name=f"vwT{i}") for i in range(4)]
        kwT = [cst.tile([P, C], F32R, tag=f"kwT{i}", name=f"kwT{i}") for i in range(4)]
        owT = [cst.tile([P, C], F32, tag=f"owT{i}", name=f"owT{i}") for i in range(4)]
        woT = [cst.tile([P, 16], F32R, tag=f"woT{i}", name=f"woT{i}") for i in range(4)]
        sw1T = [cst.tile([P, P], F32, tag=f"sw1T{i}", name=f"sw1T{i}") for i in range(4)]
        for i in range(4):
            sl = slice(i * P, (i + 1) * P)
            nc.scalar.dma_start(vwT[i][:], dd["vwT"].ap()[sl, :])
            nc.gpsimd.dma_start(kwT[i][:], dd["kwT"].ap()[sl, :])
            nc.scalar.dma_start(owT[i][:], dd["owT"].ap()[sl, :])
            nc.sync.dma_start(woT[i][:], dd["woT"].ap()[sl, :])
            nc.gpsimd.dma_start(sw1T[i][:], dd["sw1T"].ap()[sl, :])
        sw2T = cst.tile([P, C], BF16)
        nc.sync.dma_start(sw2T[:], dd["sw2T"].ap())
        wob = cst.tile([1, 16], F32R)
        kb = cst.tile([1, C], F32R)
        vb = cst.tile([1, C], F32R)
        wog = cst.tile([P, 16], F32)
        kg = cst.tile([P, C], F32)
        lpos = cst.tile([P, LT], F32)
        nc.sync.dma_start(wob[:], dd["wob"].ap())
        nc.sync.dma_start(kb[:], dd["kb"].ap())
        nc.sync.dma_start(vb[:], dd["vb"].ap())
        nc.sync.dma_start(wog[:], dd["wog"].ap())
        nc.sync.dma_start(kg[:], dd["kg"].ap())
        nc.sync.dma_start(lpos[:], dd["lpos"].ap())

        eps_t = cst.tile([P, 1], F32)
        nc.vector.memset(eps_t[:], EPS)
        ones1 = cst.tile([1, P], F32R)
        nc.sync.dma_start(ones1[:], dd["ones"].ap())
        identb = cst.tile([P, P], BF16)
        make_identity(nc, identb[:])

        # iotas
        iotaS = cst.tile([P, H, 13], F32)       # s value -6..6 per head
        it0 = cst.tile([P, H, 13], I32)
        nc.gpsimd.iota(it0[:], pattern=[[0, H], [1, 13]], base=-6, channel_multiplier=0)
        nc.vector.tensor_copy(iotaS[:], it0[:])
        iotaA5 = cst.tile([P, H, 5], F32)       # a = 1..5 per head
        it1 = cst.tile([P, H, 5], I32)
        nc.gpsimd.iota(it1[:], pattern=[[0, H], [1, 5]], base=1, channel_multiplier=0)
        nc.vector.tensor_copy(iotaA5[:], it1[:])
        iotaA7 = cst.tile([P, H, 7], F32)       # a = 0..6 per head
        it2 = cst.tile([P, H, 7], I32)
        nc.gpsimd.iota(it2[:], pattern=[[0, H], [1, 7]], base=0, channel_multiplier=0)
        nc.vector.tensor_copy(iotaA7[:], it2[:])
        # scatter index base: h*256 + lam + 46 + a  (a = s+6: 0..12)
        iotaC = cst.tile([P, H, 2], I16)   # h%4*256 + 190 + {0,1}
        itc = cst.tile([P, H, 2], I32)
        nc.gpsimd.iota(itc[:], pattern=[[0, 2], [256, 4], [1, 2]], base=190, channel_multiplier=0)
        nc.vector.tensor_copy(iotaC[:], itc[:])
        iotaIDX = cst.tile([P, H, 13], F32)
        it3 = cst.tile([P, H, 13], I32)
        nc.gpsimd.iota(it3[:], pattern=[[0, 2], [256, 4], [1, 13]], base=46, channel_multiplier=1)
        nc.vector.tensor_copy(iotaIDX[:], it3[:])

        # ---------------- v projection over halo (9 tiles, interleaved) ----------------
        v_sb = {}

        def emit_vproj(t):
            vp = ps_proj.tile([P, C], F32, tag="proj", name="vp")
            col = slice(t * P, (t + 1) * P)
            for kc in range(4):
                nc.tensor.matmul(vp[:], xT[kc][:, col],
                                 vwT[kc][:], start=(kc == 0), stop=False)
            nc.tensor.matmul(vp[:], ones1[:], vb[:],
                             start=False, stop=True)
            vt = vsb.tile([P, C], BF16, tag="v", name=f"v{t}")
            nc.scalar.copy(vt[:], vp[:])
            v_sb[t] = vt

        for t in range(2):
            emit_vproj(t)

        outT_sb = [outp.tile([P, LSH], BF16, tag=f"outT{cc}", name=f"outT{cc}") for cc in range(4)]

        # ---------------- main per-l-tile loop ----------------
        for lt in range(LT):
            if lt + 2 < VT:
                emit_vproj(lt + 2)
            xcol = slice(HALO + lt * P, HALO + (lt + 1) * P)

            # window/offset projection [P, 16]
            wop = ps_sm.tile([P, 16], F32, tag="tiny", name="wop")
            for kc in range(4):
                nc.tensor.matmul(wop[:], xT[kc][:, xcol],
                                 woT[kc][:], start=(kc == 0), stop=False)
            nc.tensor.matmul(wop[:], ones1[:], wob[:],
                             start=False, stop=True)
            # kernel projection [P, 512]
            kp = ps_proj.tile([P, C], F32, tag="proj", name="kp")
            for kc in range(4):
                nc.tensor.matmul(kp[:], xT[kc][:, xcol],
                                 kwT[kc][:], start=(kc == 0), stop=False)
            nc.tensor.matmul(kp[:], ones1[:], kb[:],
                             start=False, stop=True)

            # --- rmsnorm window/offset, sigmoid/tanh (Squares/Sqrts grouped) ---
            wsq = wk.tile([P, 16], F32, tag="wsq")
            nc.scalar.activation(wsq[:], wop[:], ACT.Square)
            ksq = wk.tile([P, C], F32, tag="ksq")
            nc.scalar.activation(ksq[:], kp[:], ACT.Square)
            wss = wk.tile([P, 2], F32, tag="wss")
            nc.vector.tensor_reduce(out=wss[:], in_=wsq[:].rearrange("p (g h) -> p g h", g=2),
                                    axis=mybir.AxisListType.X, op=OP.add)
            kss = wk.tile([P, 1], F32, tag="kss")
            nc.vector.tensor_reduce(out=kss[:], in_=ksq[:], axis=mybir.AxisListType.X,
                                    op=OP.add)
            wrstd = wk.tile([P, 2], F32, tag="wrstd")
            nc.scalar.activation(wrstd[:], wss[:], ACT.Sqrt, bias=eps_t[:], scale=1.0 / 8)
            krstd = wk.tile([P, 1], F32, tag="krstd")
            nc.scalar.activation(krstd[:], kss[:], ACT.Sqrt, bias=eps_t[:], scale=1.0 / C)
            nc.vector.reciprocal(wrstd[:], wrstd[:])
            nc.vector.reciprocal(krstd[:], krstd[:])
            won = wk.tile([P, 16], F32, tag="won")
            nc.vector.tensor_tensor(
                out=won[:].rearrange("p (g h) -> p g h", g=2),
                in0=wop[:].rearrange("p (g h) -> p g h", g=2),
                in1=_bcast(wrstd[:][:, :, None], [P, 2, 8]), op=OP.mult)
            nc.vector.tensor_tensor(out=won[:], in0=won[:], in1=wog[:], op=OP.mult)
            win_raw = wk.tile([P, H], F32, tag="win_raw")
            nc.scalar.activation(win_raw[:], won[:, 0:8], ACT.Sigmoid)
            cth = wk.tile([P, H], F32, tag="cth")
            nc.scalar.activation(cth[:], won[:, 8:16], ACT.Tanh)
            cc_ = wk.tile([P, H], F32, tag="cc_")
            nc.vector.tensor_scalar(out=cc_[:], in0=cth[:], scalar1=12.0, scalar2=None,
                                    op0=OP.mult)
            hwv = wk.tile([P, H], F32, tag="hwv")
            nc.vector.tensor_scalar(out=hwv[:], in0=win_raw[:], scalar1=5.5, scalar2=0.5,
                                    op0=OP.mult, op1=OP.add)
            tinv = wk.tile([P, H], F32, tag="tinv")
            nc.vector.reciprocal(tinv[:], hwv[:])

            # c0 = floor(c), phi = c - c0  (via +16 trunc with round-fix)
            cp16 = wk.tile([P, H], F32, tag="cp16")
            nc.vector.tensor_scalar(out=cp16[:], in0=cc_[:], scalar1=16.0, scalar2=None,
                                    op0=OP.add)
            ci = wk.tile([P, H], I32, tag="ci")
            nc.vector.tensor_copy(ci[:], cp16[:])
            cf = wk.tile([P, H], F32, tag="cf")
            nc.vector.tensor_copy(cf[:], ci[:])
            cgt = wk.tile([P, H], F32, tag="cgt")
            nc.vector.tensor_tensor(out=cgt[:], in0=cf[:], in1=cp16[:], op=OP.is_gt)
            c0p16 = wk.tile([P, H], F32, tag="c0p16")
            nc.vector.tensor_tensor(out=c0p16[:], in0=cf[:], in1=cgt[:], op=OP.subtract)
            phi = wk.tile([P, H], F32, tag="phi")
            nc.vector.tensor_tensor(out=phi[:], in0=cp16[:], in1=c0p16[:], op=OP.subtract)
            c012 = wk.tile([P, H], F32, tag="c012")   # c0 + 12
            nc.vector.tensor_scalar(out=c012[:], in0=c0p16[:], scalar1=4.0, scalar2=None,
                                    op0=OP.subtract)

            # --- rmsnorm kernel + silu ---
            kn = wk.tile([P, C], F32, tag="kn")
            nc.vector.tensor_scalar(out=kn[:], in0=kp[:], scalar1=krstd[:], scalar2=None,
                                    op0=OP.mult)
            nc.vector.tensor_tensor(out=kn[:], in0=kn[:], in1=kg[:], op=OP.mult)
            kern = wk.tile([P, H, K], BF16, tag="kern")
            if USE_SILU:
                nc.scalar.activation(kern[:].rearrange("p h k -> p (h k)"), kn[:], ACT.Silu)
            else:
                ksg = wk.tile([P, C], F32, tag="ksg")
                nc.scalar.activation(ksg[:], kn[:], ACT.Sigmoid)
                nc.vector.tensor_tensor(out=kern[:].rearrange("p h k -> p (h k)"), in0=kn[:],
                                        in1=ksg[:], op=OP.mult)

            # D table: Dt[k] = kern[k+1] - kern[k], Dt[63] = 0
            Dt = wk.tile([P, H, K], BF16, tag="Dt")
            nc.vector.memset(Dt[:, :, 63:64], 0.0)
            nc.vector.tensor_tensor(out=Dt[:, :, 0:63], in0=kern[:, :, 1:64],
                                    in1=kern[:, :, 0:63], op=OP.subtract)

            # --- interpolation indices (a = 1..5; a=6 always clips to slot 63) ---
            A5 = 5
            npos = wk.tile([P, H, A5], F32, tag="npos")
            nc.vector.tensor_tensor(out=npos[:], in0=iotaA5[:],
                                    in1=_bcast(tinv[:][:, :, None], [P, H, A5]), op=OP.mult)
            nc.vector.tensor_scalar(out=npos[:], in0=npos[:], scalar1=1.0, scalar2=float(K - 1),
                                    op0=OP.min, op1=OP.mult)
            ii = wk.tile([P, H, A5], I32, tag="ii")
            nc.vector.tensor_copy(ii[:], npos[:])
            tf = wk.tile([P, H, A5], F32, tag="tf")
            nc.vector.tensor_copy(tf[:], ii[:])
            tgt = wk.tile([P, H, A5], F32, tag="tgt")
            nc.vector.tensor_tensor(out=tgt[:], in0=tf[:], in1=npos[:], op=OP.is_gt)
            idxf = wk.tile([P, H, A5], F32, tag="idxf")
            nc.vector.tensor_tensor(out=idxf[:], in0=tf[:], in1=tgt[:], op=OP.subtract)
            nc.vector.tensor_scalar(out=idxf[:], in0=idxf[:], scalar1=float(K - 2),
                                    scalar2=None, op0=OP.min)
            w_c = wk.tile([P, H, A5], F32, tag="w_c")
            nc.vector.tensor_tensor(out=w_c[:], in0=npos[:], in1=idxf[:], op=OP.subtract)

            # bits of idxf as int16 masks, msb first
            bits = []
            rcur = idxf
            for j, bv in enumerate([32.0, 16.0, 8.0, 4.0, 2.0, 1.0]):
                bi = wk.tile([P, H, A5], I16, tag=f"biti{j}")
                nc.vector.tensor_scalar(out=bi[:], in0=rcur[:], scalar1=bv, scalar2=None,
                                        op0=OP.is_ge)
                bits.append(bi)
                if j < 5:
                    rnew = wk.tile([P, H, A5], F32, tag=f"rem{j}")
                    nc.vector.scalar_tensor_tensor(out=rnew[:], in0=bi[:], scalar=-bv,
                                                   in1=rcur[:], op0=OP.mult, op1=OP.add)
                    rcur = rnew

            # --- halving gather of (kern, Dt) pairs at idxf ---
            st = wk.tile([P, H, A5, 2, 32], BF16, tag="st")
            nc.vector.tensor_copy(st[:, :, :, 0, :],
                                  _bcast(kern[:][:, :, None, 0:32], [P, H, A5, 32]))
            nc.vector.tensor_copy(st[:, :, :, 1, :],
                                  _bcast(Dt[:][:, :, None, 0:32], [P, H, A5, 32]))
            nc.vector.copy_predicated(st[:, :, :, 0, :],
                                      _bcast(bits[0][:][:, :, :, None], [P, H, A5, 32]),
                                      _bcast(kern[:][:, :, None, 32:64], [P, H, A5, 32]))
            nc.vector.copy_predicated(st[:, :, :, 1, :],
                                      _bcast(bits[0][:][:, :, :, None], [P, H, A5, 32]),
                                      _bcast(Dt[:][:, :, None, 32:64], [P, H, A5, 32]))
            w = 16
            for j in range(1, 6):
                nc.vector.copy_predicated(
                    st[:, :, :, :, 0:w],
                    _bcast(bits[j][:][:, :, :, None, None], [P, H, A5, 2, w]),
                    st[:, :, :, :, w:2 * w])
                w //= 2
            # g0 = st[...,0,0], g1 = st[...,1,0]
            lerp = wk.tile([P, H, A5], F32, tag="lerp")
            nc.vector.tensor_tensor(out=lerp[:], in0=w_c[:], in1=st[:, :, :, 1, 0],
                                    op=OP.mult)
            nc.vector.tensor_tensor(out=lerp[:], in0=lerp[:], in1=st[:, :, :, 0, 0],
                                    op=OP.add)
            # ker7 = 1 + max(lerp, 0); col 0 from kern[...,0]; col 6 from kern[...,63]
            ker7 = wk.tile([P, H, 7], F32, tag="ker7")
            nc.vector.tensor_scalar(out=ker7[:, :, 1:6], in0=lerp[:], scalar1=0.0,
                                    scalar2=1.0, op0=OP.max, op1=OP.add)
            nc.vector.tensor_scalar(out=ker7[:, :, 0:1], in0=kern[:, :, 0:1], scalar1=0.0,
                                    scalar2=1.0, op0=OP.max, op1=OP.add)
            nc.vector.tensor_scalar(out=ker7[:, :, 6:7], in0=kern[:, :, 63:64], scalar1=0.0,
                                    scalar2=1.0, op0=OP.max, op1=OP.add)

            # win7 = exp(-(a * tinv)^2)
            rel7 = wk.tile([P, H, 7], F32, tag="rel7")
            nc.vector.tensor_tensor(out=rel7[:], in0=iotaA7[:],
                                    in1=_bcast(tinv[:][:, :, None], [P, H, 7]), op=OP.mult)
            nc.vector.tensor_tensor(out=rel7[:], in0=rel7[:], in1=rel7[:], op=OP.mult)
            win7 = wk.tile([P, H, 7], F32, tag="win7")
            nc.scalar.activation(win7[:], rel7[:], ACT.Exp, scale=-1.0)
            wt7 = wk.tile([P, H, 7], F32, tag="wt7")
            nc.vector.tensor_tensor(out=wt7[:], in0=ker7[:], in1=win7[:], op=OP.mult)

            # expand to 13 taps (s order -6..6)
            w13 = wk.tile([P, H, 13], F32, tag="w13")
            nc.vector.tensor_copy(w13[:, :, 0:7], wt7[:, :, ::-1])
            nc.vector.tensor_copy(w13[:, :, 6:13], wt7[:, :, 0:7])

            # validity / special masks
            nb13 = wk.tile([P, H, 13], F32, tag="nb13")
            lc = wk.tile([P, H], F32, tag="lc")
            nc.vector.tensor_tensor(out=lc[:], in0=cc_[:],
                                    in1=_bcast(lpos[:, lt:lt + 1], [P, H]), op=OP.add)
            nc.vector.tensor_tensor(out=nb13[:], in0=iotaS[:],
                                    in1=_bcast(lc[:][:, :, None], [P, H, 13]), op=OP.add)
            vhi = wk.tile([P, H, 13], F32, tag="vhi")
            nc.vector.tensor_scalar(out=vhi[:], in0=nb13[:], scalar1=float(L), scalar2=None,
                                    op0=OP.is_lt)
            valid = wk.tile([P, H, 13], F32, tag="valid")
            nc.vector.scalar_tensor_tensor(out=valid[:], in0=nb13[:], scalar=0.0,
                                           in1=vhi[:], op0=OP.is_ge, op1=OP.mult)
            spec = wk.tile([P, H, 13], F32, tag="spec")
            nc.vector.scalar_tensor_tensor(out=spec[:], in0=nb13[:], scalar=float(NB_HI),
                                           in1=vhi[:], op0=OP.is_gt, op1=OP.mult)

            wv = wk.tile([P, H, 13], F32, tag="wv")
            nc.vector.tensor_tensor(out=wv[:], in0=w13[:], in1=valid[:], op=OP.mult)
            wsum = wk.tile([P, H], F32, tag="wsum")
            nc.vector.tensor_reduce(out=wsum[:], in_=wv[:], axis=mybir.AxisListType.X,
                                    op=OP.add)
            rw = wk.tile([P, H], F32, tag="rw")
            nc.vector.tensor_scalar(out=rw[:], in0=wsum[:], scalar1=1.0, scalar2=None,
                                    op0=OP.max)
            nc.vector.reciprocal(rw[:], rw[:])

            wsp = wk.tile([P, H, 13], F32, tag="wsp")
            nc.vector.tensor_tensor(out=wsp[:], in0=wv[:], in1=spec[:], op=OP.mult)
            wint = wk.tile([P, H, 13], F32, tag="wint")
            nc.vector.tensor_tensor(out=wint[:], in0=wv[:], in1=wsp[:], op=OP.subtract)
            wspec = wk.tile([P, H], F32, tag="wspec")
            nc.vector.tensor_reduce(out=wspec[:], in_=wsp[:], axis=mybir.AxisListType.X,
                                    op=OP.add)
            wspec_s = wk.tile([P, H], F32, tag="wspec_s")
            nc.vector.tensor_tensor(out=wspec_s[:], in0=wspec[:], in1=rw[:], op=OP.mult)
            if lt == LT - 1:
                wcd = wk.tile([P, H, 2], BF16, tag="wcd")
                nc.vector.tensor_scalar(out=wcd[:, :, 0:1], in0=wspec_s[:][:, :, None],
                                        scalar1=0.001, scalar2=None, op0=OP.mult)
                nc.vector.tensor_scalar(out=wcd[:, :, 1:2], in0=wspec_s[:][:, :, None],
                                        scalar1=0.999, scalar2=None, op0=OP.mult)

            om = wk.tile([P, H], F32, tag="om")
            nc.vector.tensor_scalar(out=om[:], in0=phi[:], scalar1=-1.0, scalar2=1.0,
                                    op0=OP.mult, op1=OP.add)
            uf = wk.tile([P, H], F32, tag="uf")
            nc.vector.tensor_tensor(out=uf[:], in0=om[:], in1=rw[:], op=OP.mult)
            uc = wk.tile([P, H], F32, tag="uc")
            nc.vector.tensor_tensor(out=uc[:], in0=phi[:], in1=rw[:], op=OP.mult)

            wf = wk.tile([P, H, 13], BF16, tag="wf")
            nc.vector.tensor_tensor(out=wf[:], in0=wint[:],
                                    in1=_bcast(uf[:][:, :, None], [P, H, 13]), op=OP.mult)
            wcc = wk.tile([P, H, 13], BF16, tag="wcc")
            nc.vector.tensor_tensor(out=wcc[:], in0=wint[:],
                                    in1=_bcast(uc[:][:, :, None], [P, H, 13]), op=OP.mult)

            # scatter indices
            idxf32 = wk.tile([P, H, 13], F32, tag="idxf32")
            nc.vector.tensor_tensor(out=idxf32[:], in0=iotaIDX[:],
                                    in1=_bcast(c012[:][:, :, None], [P, H, 13]), op=OP.add)
            i16 = wk.tile([P, H, 13], I16, tag="i16")
            nc.vector.tensor_copy(i16[:], idxf32[:])
            i16b = wk.tile([P, H, 13], I16, tag="i16b")
            nc.vector.tensor_scalar(out=i16b[:], in0=i16[:], scalar1=1, scalar2=None,
                                    op0=OP.add)

            A0 = wk.tile([P, H, 256], BF16, tag="A0")
            A1 = wk.tile([P, H, 256], BF16, tag="A1")
            for hb in range(2):
                hs = slice(hb * 4, hb * 4 + 4)
                nc.gpsimd.local_scatter(
                    A0[:, hs, :].rearrange("p h w -> p (h w)"),
                    wf[:, hs, :].rearrange("p h a -> p (h a)"),
                    i16[:, hs, :].rearrange("p h a -> p (h a)"),
                    channels=P, num_elems=4 * 256, num_idxs=4 * 13)
                nc.gpsimd.local_scatter(
                    A1[:, hs, :].rearrange("p h w -> p (h w)"),
                    wcc[:, hs, :].rearrange("p h a -> p (h a)"),
                    i16b[:, hs, :].rearrange("p h a -> p (h a)"),
                    channels=P, num_elems=4 * 256, num_idxs=4 * 13)
            A = wk.tile([P, H, 256], BF16, tag="A")
            if lt == LT - 1:
                A2 = wk.tile([P, H, 256], BF16, tag="A2")
                for hb in range(2):
                    hs = slice(hb * 4, hb * 4 + 4)
                    nc.gpsimd.local_scatter(
                        A2[:, hs, :].rearrange("p h w -> p (h w)"),
                        wcd[:, hs, :].rearrange("p h a -> p (h a)"),
                        iotaC[:, hs, :].rearrange("p h a -> p (h a)"),
                        channels=P, num_elems=4 * 256, num_idxs=4 * 2)
                nc.vector.tensor_tensor(out=A0[:], in0=A0[:], in1=A2[:], op=OP.add)
            nc.vector.tensor_tensor(out=A[:], in0=A0[:], in1=A1[:], op=OP.add)


            # --- transpose A blocks and banded matmul ---
            for cci in range(4):
                po = ps_mm.tile([P, P], F32, tag="mm", name="po")
                tp = ps_tr.tile([P, 4, P], BF16, tag="tp")
                at = atp.tile([P, 4, P], BF16, tag="at")
                for hh in range(2):
                    h = 2 * cci + hh
                    for blk in range(2):
                        nc.tensor.transpose(tp[:, 2 * hh + blk, :],
                                            A[:, h, blk * P:(blk + 1) * P], identb[:])
                nc.scalar.copy(at[:], tp[:])
                for hh in range(2):
                    h = 2 * cci + hh
                    prange = slice(hh * 64, hh * 64 + 64)
                    for blk in range(2):
                        nc.tensor.matmul(po[prange, :],
                                         v_sb[lt + blk][:, h * D:(h + 1) * D],
                                         at[:, 2 * hh + blk, :], start=(blk == 0),
                                         stop=(blk == 1))
                nc.scalar.copy(outT_sb[cci][:, lt * P:(lt + 1) * P], po[:])

        # ---------------- SE partial sums + AllGather + local sum ----------------
        seacc = cst.tile([P, 4], F32)
        for cci in range(4):
            nc.vector.tensor_reduce(out=seacc[:, cci:cci + 1],
                                    in_=outT_sb[cci][:],
                                    axis=mybir.AxisListType.X, op=OP.add)
        if skip_cc:
            armean = seacc
        else:
            cci_d = dram.tile([P, 4], F32)
            cco_d = dram.tile([4, P, 4], F32)
            nc.gpsimd.dma_start(cci_d[:], seacc[:])
            nc.gpsimd.collective_compute(
                "AllGather", OP.bypass,
                replica_groups=[[0, 1, 2, 3], [4, 5, 6, 7]],
                ins=[cci_d[:].opt()], outs=[cco_d[:].opt()])
            # gathered along partition axis: [4 ranks, 128, 4] -> partitions 0..511?
            ag = cst.tile([P, 4, 4], F32)
            nc.sync.dma_start(ag[:], cco_d[:].rearrange("r p f -> p r f"))
            armean = cst.tile([P, 4], F32)
            nc.vector.tensor_tensor(out=armean[:], in0=ag[:, 0, :], in1=ag[:, 1, :],
                                    op=OP.add)
            nc.vector.tensor_tensor(out=armean[:], in0=armean[:], in1=ag[:, 2, :],
                                    op=OP.add)
            nc.vector.tensor_tensor(out=armean[:], in0=armean[:], in1=ag[:, 3, :],
                                    op=OP.add)

        # ---------------- SE MLP (transposed throughout) ----------------
        hidT_ps = ps_sm.tile([P, 1], F32, tag="tiny", name="hidT_ps")
        for cci in range(4):
            nc.tensor.matmul(hidT_ps[:], sw1T[cci][:], armean[:, cci:cci + 1],
                             start=(cci == 0), stop=(cci == 3))
        hidT = cst.tile([P, 1], BF16)
        if USE_SILU:
            nc.scalar.activation(hidT[:], hidT_ps[:], ACT.Silu)
        else:
            hsg = cst.tile([P, 1], F32)
            nc.scalar.activation(hsg[:], hidT_ps[:], ACT.Sigmoid)
            nc.vector.tensor_tensor(out=hidT[:], in0=hidT_ps[:], in1=hsg[:], op=OP.mult)
        owb = []
        for cci in range(4):
            scT_ps = ps_sm.tile([P, 1], F32, tag="tiny", name="scT_ps")
            nc.tensor.matmul(scT_ps[:], sw2T[:, cci * P:(cci + 1) * P], hidT[:],
                             start=True, stop=True)
            scT = cst.tile([P, 1], F32, tag=f"scT{cci}", name=f"scT{cci}")
            nc.scalar.activation(scT[:], scT_ps[:], ACT.Sigmoid)
            ow = cst.tile([P, C], BF16, tag=f"owb{cci}", name=f"owb{cci}")
            nc.vector.tensor_scalar(out=ow[:], in0=owT[cci][:], scalar1=scT[:],
                                    scalar2=None, op0=OP.mult)
            owb.append(ow)

        # ---------------- final out_w matmul + silu + DMA out ----------------
        # out[l, cout] = sum_cin out_preT[cin, l] * ow'[cin, cout]  (lhsT = out_preT)
        for lt in range(LT):
            lsl = slice(lt * P, (lt + 1) * P)
            pf_ = ps_mm.tile([P, C], F32, tag="mm", name="pf_")
            for cci in range(4):
                nc.tensor.matmul(pf_[:], outT_sb[cci][:, lsl], owb[cci][:],
                                 start=(cci == 0), stop=(cci == 3))
            fo = wk.tile([P, C], F32, tag="fo")
            if USE_SILU:
                nc.scalar.activation(fo[:], pf_[:], ACT.Silu)
            else:
                nc.scalar.activation(fo[:], pf_[:], ACT.Sigmoid)
                nc.vector.tensor_tensor(out=fo[:], in0=fo[:], in1=pf_[:], op=OP.mult)
            nc.sync.dma_start(dd["out"].ap()[lsl, :], fo[:])


def make_in_maps(inputs, n_cores=8):
    x = np.ascontiguousarray(inputs["x"], dtype=np.float32)
    window_w = inputs["window_w"]; window_b = inputs["window_b"]
    window_gamma = inputs["window_gamma"]
    offset_w = inputs["offset_w"]; offset_b = inputs["offset_b"]
    offset_gamma = inputs["offset_gamma"]
    kernel_w = inputs["kernel_w"]; kernel_b = inputs["kernel_b"]
    kernel_gamma = inputs["kernel_gamma"]
    v_w = inputs["v_w"]; v_b = inputs["v_b"]
    se_w1 = inputs["se_w1"]; se_w2 = inputs["se_w2"]; out_w = inputs["out_w"]

    woT = np.concatenate([window_w, offset_w], 0).T.astype(np.float32)      # (512,16)
    wob = np.concatenate([window_b, offset_b])[None].astype(np.float32)     # (1,16)
    wog = np.tile(np.concatenate([window_gamma, offset_gamma])[None], (P, 1)).astype(np.float32)
    kwT = np.ascontiguousarray(kernel_w.T, np.float32)
    kb = kernel_b[None].astype(np.float32)
    kgm = np.tile(kernel_gamma[None], (P, 1)).astype(np.float32)
    vwT = np.ascontiguousarray(v_w.T, np.float32)
    vbm = v_b[None].astype(np.float32)
    sw1T = np.ascontiguousarray(se_w1.T, np.float32) / np.float32(L)
    sw2T = np.ascontiguousarray(se_w2.T).astype(ml_dtypes.bfloat16)
    owT = np.ascontiguousarray(out_w.T, np.float32)

    in_maps = []
    for i in range(n_cores):
        b, q = divmod(i, 4)
        lo = q * LSH - HALO
        xpad = np.zeros((XROWS, C), np.float32)
        s0, s1 = max(lo, 0), min(lo + XROWS, L)
        xpad[s0 - lo:s1 - lo] = x[b, s0:s1]
        xT = np.ascontiguousarray(xpad.T)
        lpos = (q * LSH + np.arange(LSH, dtype=np.float32)).reshape(LT, P).T.copy()
        in_maps.append(dict(
            xT=xT, lpos=lpos, woT=woT, wob=wob, wog=wog, kwT=kwT, kb=kb,
            kg=kgm, vwT=vwT, vb=vbm, sw1T=sw1T, sw2T=sw2T, owT=owT,
            ones=np.ones((1, P), np.float32),
        ))
    return in_maps


def kernel(**inputs) -> np.ndarray:
    if "graph" not in _GRAPH_CACHE:
        _GRAPH_CACHE["graph"] = build_graph(8)
    nc = _GRAPH_CACHE["graph"]
    in_maps = make_in_maps(inputs, 8)
    res = run_bass_kernel_spmd(nc, in_maps, core_ids=list(range(8)))
    out = np.zeros((B, L, C), np.float32)
    for i in range(8):
        b, q = divmod(i, 4)
        out[b, q * LSH:(q + 1) * LSH] = res.results[i]["out"]
    return out


if __name__ == "__main__":
    import reference
    inputs = {k: np.asarray(v) for k, v in reference.setup_inputs().items()}
    got = kernel(**inputs)
    import jax.numpy as jnp
    exp = np.asarray(reference.reference(**{k: jnp.asarray(v) for k, v in inputs.items()}))
    rel = np.linalg.norm(got - exp) / np.linalg.norm(exp)
    print("Relative error:", rel)


# revision 36
# speedup vs baseline: 25.6030x; 1.2207x over previous
"""AdaptiveLocalConv Trainium2 kernel — 8-core SPMD.

Sharding: (batch, seq-quarter) -> 8 shards of 1024 tokens (+64 halo each side
for the deformable gather, reach <= +-19).

Per-core pipeline:
  - 4 projections from x via PE (f32r), x passed pre-transposed [C, 1152].
  - per-(token,head): 13 deformable taps; kernel-table interpolation via a
    log2 halving-gather on DVE; taps placed into a banded matrix A
    [token, head, 256-slot J-band] with gpsimd local_scatter (bf16).
  - banded matmul out^T[d, l] = sum_J v[J, d] * A^T[J, l] on PE after
    PE-transposing A blocks; sequence-end clamp handled exactly by a rank-1
    correction matmul.
  - squeeze-excite via a 4-core AllReduce of the per-core partial mean (2KB),
    SE scale folded into out_w columns; final out_w matmul in transposed
    layout; silu; DMA out with a transposing access pattern.
"""
import sys
if "/opt/trn_rl_repo" not in sys.path:
    sys.path.insert(0, "/opt/trn_rl_repo")

import numpy as np
import ml_dtypes

import concourse.bass as bass
import concourse.mybir as mybir
from concourse import bacc
from concourse.tile import TileContext
from concourse.bass_utils import run_bass_kernel_spmd
from concourse.masks import make_identity

F32 = mybir.dt.float32
F32R = mybir.dt.float32r
BF16 = mybir.dt.bfloat16
I32 = mybir.dt.int32
I16 = mybir.dt.int16
OP = mybir.AluOpType
ACT = mybir.ActivationFunctionType

B, L, C, H, K, D = 2, 4096, 512, 8, 64, 64
P = 128
HALO = 64
LSH = 1024          # tokens per core
XROWS = LSH + 2 * HALO   # 1152
LT = LSH // P       # 8 own l-tiles
VT = XROWS // P     # 9 v tiles
NB_HI = np.float32(L - 1.001)
EPS = 1e-6

_GRAPH_CACHE = {}
USE_SILU = True  # sim lacks Silu; test_sim sets False


def _bcast(ap, shape):
    return ap.broadcast_to(shape)


def build_graph(n_cores=8, skip_cc=False):
    nc = bacc.Bacc("TRN2", target_bir_lowering=False, debug=False,
                   enable_asserts=False, num_devices=n_cores)

    # ---------------- DRAM parameters ----------------
    xT_d = nc.dram_tensor("xT", [C, XROWS], F32R, kind="ExternalInput")
    lpos_d = nc.dram_tensor("lpos", [P, LT], F32, kind="ExternalInput")
    woT_d = nc.dram_tensor("woT", [C, 16], F32R, kind="ExternalInput")
    wob_d = nc.dram_tensor("wob", [1, 16], F32R, kind="ExternalInput")
    wog_d = nc.dram_tensor("wog", [P, 16], F32, kind="ExternalInput")
    kwT_d = nc.dram_tensor("kwT", [C, C], F32R, kind="ExternalInput")
    kb_d = nc.dram_tensor("kb", [1, C], F32R, kind="ExternalInput")
    kg_d = nc.dram_tensor("kg", [P, C], F32, kind="ExternalInput")
    vwT_d = nc.dram_tensor("vwT", [C, C], F32R, kind="ExternalInput")
    vb_d = nc.dram_tensor("vb", [1, C], F32R, kind="ExternalInput")
    sw1T_d = nc.dram_tensor("sw1T", [C, P], F32, kind="ExternalInput")   # pre-scaled by 1/L
    sw2T_d = nc.dram_tensor("sw2T", [P, C], BF16, kind="ExternalInput")
    owT_d = nc.dram_tensor("owT", [C, C], F32, kind="ExternalInput")
    ones_d = nc.dram_tensor("ones", [1, P], F32R, kind="ExternalInput")
    out_d = nc.dram_tensor("out", [LSH, C], F32, kind="ExternalOutput")

    with TileContext(nc) as tc:
        _build_body(nc, tc, dict(
            xT=xT_d, lpos=lpos_d, woT=woT_d, wob=wob_d, wog=wog_d,
            kwT=kwT_d, kb=kb_d, kg=kg_d, vwT=vwT_d, vb=vb_d,
            sw1T=sw1T_d, sw2T=sw2T_d, owT=owT_d, out=out_d, ones=ones_d,
        ), skip_cc=skip_cc)
    nc.compile()
    return nc


def _build_body(nc, tc, dd, skip_cc=False):
    import contextlib
    ctx = contextlib.ExitStack()
    with ctx:
        cst = ctx.enter_context(tc.tile_pool(name="cst", bufs=1))
        vsb = ctx.enter_context(tc.tile_pool(name="vsb", bufs=VT))
        wk = ctx.enter_context(tc.tile_pool(name="wk", bufs=3))
        atp = ctx.enter_context(tc.tile_pool(name="atp", bufs=3))
        outp = ctx.enter_context(tc.tile_pool(name="outp", bufs=1))
        ps_proj = ctx.enter_context(tc.tile_pool(name="ps_proj", bufs=2, space="PSUM"))
        ps_sm = ctx.enter_context(tc.tile_pool(name="ps_sm", bufs=2, space="PSUM"))
        ps_tr = ctx.enter_context(tc.tile_pool(name="ps_tr", bufs=2, space="PSUM"))
        ps_mm = ctx.enter_context(tc.tile_pool(name="ps_mm", bufs=2, space="PSUM"))
        dram = ctx.enter_context(tc.tile_pool(name="dram", bufs=1, space="DRAM"))

        # ---------------- constants & weights to SBUF ----------------
        xT = [cst.tile([P, XROWS], F32R, tag=f"xT{i}", name=f"xT{i}") for i in range(4)]
        for i in range(4):
            nc.sync.dma_start(xT[i][:], dd["xT"].ap()[i * P:(i + 1) * P, :])
        vwT = [cst.tile([P, C], F32R, tag=f"vwT{i}", name=f"vwT{i}") for i in range(4)]
        kwT = [cst.tile([P, C], F32R, tag=f"kwT{i}", name=f"kwT{i}") for i in range(4)]
        owT = [cst.tile([P, C], F32, tag=f"owT{i}", name=f"owT{i}") for i in range(4)]
        woT = [cst.tile([P, 16], F32R, tag=f"woT{i}", name=f"woT{i}") for i in range(4)]
        sw1T = [cst.tile([P, P], F32, tag=f"sw1T{i}", name=f"sw1T{i}") for i in range(4)]
        for i in range(4):
            sl = slice(i * P, (i + 1) * P)
            nc.scalar.dma_start(vwT[i][:], dd["vwT"].ap()[sl, :])
            nc.gpsimd.dma_start(kwT[i][:], dd["kwT"].ap()[sl, :])
            nc.scalar.dma_start(owT[i][:], dd["owT"].ap()[sl, :])
            nc.sync.dma_start(woT[i][:], dd["woT"].ap()[sl, :])
            nc.gpsimd.dma_start(sw1T[i][:], dd["sw1T"].ap()[sl, :])
        sw2T = cst.tile([P, C], BF16)
        nc.sync.dma_start(sw2T[:], dd["sw2T"].ap())
        wob = cst.tile([1, 16], F32R)
        kb = cst.tile([1, C], F32R)
        vb = cst.tile([1, C], F32R)
        wog = cst.tile([P, 16], F32)
        kg = cst.tile([P, C], F32)
        lpos = cst.tile([P, LT], F32)
        nc.sync.dma_start(wob[:], dd["wob"].ap())
        nc.sync.dma_start(kb[:], dd["kb"].ap())
        nc.sync.dma_start(vb[:], dd["vb"].ap())
        nc.sync.dma_start(wog[:], dd["wog"].ap())
        nc.sync.dma_start(kg[:], dd["kg"].ap())
        nc.sync.dma_start(lpos[:], dd["lpos"].ap())

        eps_t = cst.tile([P, 1], F32)
        nc.vector.memset(eps_t[:], EPS)
        ones1 = cst.tile([1, P], F32R)
        nc.sync.dma_start(ones1[:], dd["ones"].ap())
        identb = cst.tile([P, P], BF16)
        make_identity(nc, identb[:])

        # iotas
        iotaS = cst.tile([P, H, 13], F32)       # s value -6..6 per head
        it0 = cst.tile([P, H, 13], I32)
        nc.gpsimd.iota(it0[:], pattern=[[0, H], [1, 13]], base=-6, channel_multiplier=0)
        nc.vector.tensor_copy(iotaS[:], it0[:])
        iotaA5 = cst.tile([P, H, 5], F32)       # a = 1..5 per head
        it1 = cst.tile([P, H, 5], I32)
        nc.gpsimd.iota(it1[:], pattern=[[0, H], [1, 5]], base=1, channel_multiplier=0)
        nc.vector.tensor_copy(iotaA5[:], it1[:])
        iotaA7 = cst.tile([P, H, 7], F32)       # a = 0..6 per head
        it2 = cst.tile([P, H, 7], I32)
        nc.gpsimd.iota(it2[:], pattern=[[0, H], [1, 7]], base=0, channel_multiplier=0)
        nc.vector.tensor_copy(iotaA7[:], it2[:])
        # scatter index base: h*256 + lam + 46 + a  (a = s+6: 0..12)
        iotaC = cst.tile([P, H, 2], I16)   # h%4*256 + 190 + {0,1}
        itc = cst.tile([P, H, 2], I32)
        nc.gpsimd.iota(itc[:], pattern=[[0, 2], [256, 4], [1, 2]], base=190, channel_multiplier=0)
        nc.vector.tensor_copy(iotaC[:], itc[:])
        iotaIDX = cst.tile([P, H, 13], F32)
        it3 = cst.tile([P, H, 13], I32)
        nc.gpsimd.iota(it3[:], pattern=[[0, 2], [256, 4], [1, 13]], base=46, channel_multiplier=1)
        nc.vector.tensor_copy(iotaIDX[:], it3[:])

        # ---------------- v projection over halo (9 tiles, interleaved) ----------------
        v_sb = {}

        def emit_vproj(t):
            vp = ps_proj.tile([P, C], F32, tag="proj", name="vp")
            col = slice(t * P, (t + 1) * P)
            for kc in range(4):
                nc.tensor.matmul(vp[:], xT[kc][:, col],
                                 vwT[kc][:], start=(kc == 0), stop=False)
            nc.tensor.matmul(vp[:], ones1[:], vb[:],
                             start=False, stop=True)
            vt = vsb.tile([P, C], BF16, tag="v", name=f"v{t}")
            nc.scalar.copy(vt[:], vp[:])
            v_sb[t] = vt

        for t in range(2):
            emit_vproj(t)

        outT_sb = [outp.tile([P, LSH], BF16, tag=f"outT{cc}", name=f"outT{cc}") for cc in range(4)]

        # ---------------- main per-l-tile loop ----------------
        for lt in range(LT):
            if lt + 2 < VT:
                emit_vproj(lt + 2)
            xcol = slice(HALO + lt * P, HALO + (lt + 1) * P)

            # window/offset projection [P, 16]
            wop = ps_sm.tile([P, 16], F32, tag="tiny", name="wop")
            for kc in range(4):
                nc.tensor.matmul(wop[:], xT[kc][:, xcol],
                                 woT[kc][:], start=(kc == 0), stop=False)
            nc.tensor.matmul(wop[:], ones1[:], wob[:],
                             start=False, stop=True)
            # kernel projection [P, 512]
            kp = ps_proj.tile([P, C], F32, tag="proj", name="kp")
            for kc in range(4):
                nc.tensor.matmul(kp[:], xT[kc][:, xcol],
                                 kwT[kc][:], start=(kc == 0), stop=False)
            nc.tensor.matmul(kp[:], ones1[:], kb[:],
                             start=False, stop=True)

            # --- rmsnorm window/offset, sigmoid/tanh (Squares/Sqrts grouped) ---
            wsq = wk.tile([P, 16], F32, tag="wsq")
            nc.scalar.activation(wsq[:], wop[:], ACT.Square)
            ksq = wk.tile([P, C], F32, tag="ksq")
            nc.scalar.activation(ksq[:], kp[:], ACT.Square)
            wss = wk.tile([P, 2], F32, tag="wss")
            nc.vector.tensor_reduce(out=wss[:], in_=wsq[:].rearrange("p (g h) -> p g h", g=2),
                                    axis=mybir.AxisListType.X, op=OP.add)
            kss = wk.tile([P, 1], F32, tag="kss")
            nc.vector.tensor_reduce(out=kss[:], in_=ksq[:], axis=mybir.AxisListType.X,
                                    op=OP.add)
            wrstd = wk.tile([P, 2], F32, tag="wrstd")
            nc.scalar.activation(wrstd[:], wss[:], ACT.Sqrt, bias=eps_t[:], scale=1.0 / 8)
            krstd = wk.tile([P, 1], F32, tag="krstd")
            nc.scalar.activation(krstd[:], kss[:], ACT.Sqrt, bias=eps_t[:], scale=1.0 / C)
            nc.vector.reciprocal(wrstd[:], wrstd[:])
            nc.vector.reciprocal(krstd[:], krstd[:])
            won = wk.tile([P, 16], F32, tag="won")
            nc.vector.tensor_tensor(
                out=won[:].rearrange("p (g h) -> p g h", g=2),
                in0=wop[:].rearrange("p (g h) -> p g h", g=2),
                in1=_bcast(wrstd[:][:, :, None], [P, 2, 8]), op=OP.mult)
            nc.vector.tensor_tensor(out=won[:], in0=won[:], in1=wog[:], op=OP.mult)
            win_raw = wk.tile([P, H], F32, tag="win_raw")
            nc.scalar.activation(win_raw[:], won[:, 0:8], ACT.Sigmoid)
            cth = wk.tile([P, H], F32, tag="cth")
            nc.scalar.activation(cth[:], won[:, 8:16], ACT.Tanh)
            cc_ = wk.tile([P, H], F32, tag="cc_")
            nc.vector.tensor_scalar(out=cc_[:], in0=cth[:], scalar1=12.0, scalar2=None,
                                    op0=OP.mult)
            hwv = wk.tile([P, H], F32, tag="hwv")
            nc.vector.tensor_scalar(out=hwv[:], in0=win_raw[:], scalar1=5.5, scalar2=0.5,
                                    op0=OP.mult, op1=OP.add)
            tinv = wk.tile([P, H], F32, tag="tinv")
            nc.vector.reciprocal(tinv[:], hwv[:])

            # c0 = floor(c), phi = c - c0  (via +16 trunc with round-fix)
            cp16 = wk.tile([P, H], F32, tag="cp16")
            nc.vector.tensor_scalar(out=cp16[:], in0=cc_[:], scalar1=16.0, scalar2=None,
                                    op0=OP.add)
            ci = wk.tile([P, H], I32, tag="ci")
            nc.vector.tensor_copy(ci[:], cp16[:])
            cf = wk.tile([P, H], F32, tag="cf")
            nc.vector.tensor_copy(cf[:], ci[:])
            cgt = wk.tile([P, H], F32, tag="cgt")
            nc.vector.tensor_tensor(out=cgt[:], in0=cf[:], in1=cp16[:], op=OP.is_gt)
            c0p16 = wk.tile([P, H], F32, tag="c0p16")
            nc.vector.tensor_tensor(out=c0p16[:], in0=cf[:], in1=cgt[:], op=OP.subtract)
            phi = wk.tile([P, H], F32, tag="phi")
            nc.vector.tensor_tensor(out=phi[:], in0=cp16[:], in1=c0p16[:], op=OP.subtract)
            c012 = wk.tile([P, H], F32, tag="c012")   # c0 + 12
            nc.vector.tensor_scalar(out=c012[:], in0=c0p16[:], scalar1=4.0, scalar2=None,
                                    op0=OP.subtract)

            # --- rmsnorm kernel + silu ---
            kn = wk.tile([P, C], F32, tag="kn")
            nc.scalar.activation(kn[:], kp[:], ACT.Copy, scale=krstd[:])
            nc.vector.tensor_tensor(out=kn[:], in0=kn[:], in1=kg[:], op=OP.mult)
            kern = wk.tile([P, H, K], BF16, tag="kern")
            if USE_SILU:
                nc.scalar.activation(kern[:].rearrange("p h k -> p (h k)"), kn[:], ACT.Silu)
            else:
                ksg = wk.tile([P, C], F32, tag="ksg")
                nc.scalar.activation(ksg[:], kn[:], ACT.Sigmoid)
                nc.vector.tensor_tensor(out=kern[:].rearrange("p h k -> p (h k)"), in0=kn[:],
                                        in1=ksg[:], op=OP.mult)

            # D table: Dt[k] = kern[k+1] - kern[k], Dt[63] = 0
            Dt = wk.tile([P, H, K], BF16, tag="Dt")
            nc.vector.memset(Dt[:, :, 63:64], 0.0)
            nc.vector.tensor_tensor(out=Dt[:, :, 0:63], in0=kern[:, :, 1:64],
                                    in1=kern[:, :, 0:63], op=OP.subtract)

            # --- interpolation indices (a = 1..5; a=6 always clips to slot 63) ---
            A5 = 5
            npos = wk.tile([P, H, A5], F32, tag="npos")
            nc.vector.tensor_tensor(out=npos[:], in0=iotaA5[:],
                                    in1=_bcast(tinv[:][:, :, None], [P, H, A5]), op=OP.mult)
            nc.vector.tensor_scalar(out=npos[:], in0=npos[:], scalar1=1.0, scalar2=float(K - 1),
                                    op0=OP.min, op1=OP.mult)
            ii = wk.tile([P, H, A5], I32, tag="ii")
            nc.vector.tensor_copy(ii[:], npos[:])
            tf = wk.tile([P, H, A5], F32, tag="tf")
            nc.vector.tensor_copy(tf[:], ii[:])
            tgt = wk.tile([P, H, A5], F32, tag="tgt")
            nc.vector.tensor_tensor(out=tgt[:], in0=tf[:], in1=npos[:], op=OP.is_gt)
            idxf = wk.tile([P, H, A5], F32, tag="idxf")
            nc.vector.tensor_tensor(out=idxf[:], in0=tf[:], in1=tgt[:], op=OP.subtract)
            nc.vector.tensor_scalar(out=idxf[:], in0=idxf[:], scalar1=float(K - 2),
                                    scalar2=None, op0=OP.min)
            w_c = wk.tile([P, H, A5], F32, tag="w_c")
            nc.vector.tensor_tensor(out=w_c[:], in0=npos[:], in1=idxf[:], op=OP.subtract)

            # bits of idxf as int16 masks, msb first
            bits = []
            rcur = idxf
            for j, bv in enumerate([32.0, 16.0, 8.0, 4.0, 2.0, 1.0]):
                bi = wk.tile([P, H, A5], I16, tag=f"biti{j}")
                nc.vector.tensor_scalar(out=bi[:], in0=rcur[:], scalar1=bv, scalar2=None,
                                        op0=OP.is_ge)
                bits.append(bi)
                if j < 5:
                    rnew = wk.tile([P, H, A5], F32, tag=f"rem{j}")
                    nc.vector.scalar_tensor_tensor(out=rnew[:], in0=bi[:], scalar=-bv,
                                                   in1=rcur[:], op0=OP.mult, op1=OP.add)
                    rcur = rnew

            # --- halving gather of (kern, Dt) pairs at idxf ---
            st = wk.tile([P, H, A5, 2, 32], BF16, tag="st")
            nc.vector.tensor_copy(st[:, :, :, 0, :],
                                  _bcast(kern[:][:, :, None, 0:32], [P, H, A5, 32]))
            nc.vector.tensor_copy(st[:, :, :, 1, :],
                                  _bcast(Dt[:][:, :, None, 0:32], [P, H, A5, 32]))
            nc.vector.copy_predicated(st[:, :, :, 0, :],
                                      _bcast(bits[0][:][:, :, :, None], [P, H, A5, 32]),
                                      _bcast(kern[:][:, :, None, 32:64], [P, H, A5, 32]))
            nc.vector.copy_predicated(st[:, :, :, 1, :],
                                      _bcast(bits[0][:][:, :, :, None], [P, H, A5, 32]),
                                      _bcast(Dt[:][:, :, None, 32:64], [P, H, A5, 32]))
            w = 16
            for j in range(1, 6):
                nc.vector.copy_predicated(
                    st[:, :, :, :, 0:w],
                    _bcast(bits[j][:][:, :, :, None, None], [P, H, A5, 2, w]),
                    st[:, :, :, :, w:2 * w])
                w //= 2
            # g0 = st[...,0,0], g1 = st[...,1,0]
            lerp = wk.tile([P, H, A5], F32, tag="lerp")
            nc.vector.tensor_tensor(out=lerp[:], in0=w_c[:], in1=st[:, :, :, 1, 0],
                                    op=OP.mult)
            nc.vector.tensor_tensor(out=lerp[:], in0=lerp[:], in1=st[:, :, :, 0, 0],
                                    op=OP.add)
            # ker7 = 1 + max(lerp, 0); col 0 from kern[...,0]; col 6 from kern[...,63]
            ker7 = wk.tile([P, H, 7], F32, tag="ker7")
            nc.vector.tensor_scalar(out=ker7[:, :, 1:6], in0=lerp[:], scalar1=0.0,
                                    scalar2=1.0, op0=OP.max, op1=OP.add)
            nc.vector.tensor_scalar(out=ker7[:, :, 0:1], in0=kern[:, :, 0:1], scalar1=0.0,
                                    scalar2=1.0, op0=OP.max, op1=OP.add)
            nc.vector.tensor_scalar(out=ker7[:, :, 6:7], in0=kern[:, :, 63:64], scalar1=0.0,
                                    scalar2=1.0, op0=OP.max, op1=OP.add)

            # win7 = exp(-(a * tinv)^2)
            rel7 = wk.tile([P, H, 7], F32, tag="rel7")
            nc.vector.tensor_tensor(out=rel7[:], in0=iotaA7[:],
                                    in1=_bcast(tinv[:][:, :, None], [P, H, 7]), op=OP.mult)
            nc.vector.tensor_tensor(out=rel7[:], in0=rel7[:], in1=rel7[:], op=OP.mult)
            win7 = wk.tile([P, H, 7], F32, tag="win7")
            nc.scalar.activation(win7[:], rel7[:], ACT.Exp, scale=-1.0)
            wt7 = wk.tile([P, H, 7], F32, tag="wt7")
            nc.vector.tensor_tensor(out=wt7[:], in0=ker7[:], in1=win7[:], op=OP.mult)

            # expand to 13 taps (s order -6..6)
            w13 = wk.tile([P, H, 13], F32, tag="w13")
            nc.vector.tensor_copy(w13[:, :, 0:7], wt7[:, :, ::-1])
            nc.vector.tensor_copy(w13[:, :, 6:13], wt7[:, :, 0:7])

            # validity / special masks
            nb13 = wk.tile([P, H, 13], F32, tag="nb13")
            lc = wk.tile([P, H], F32, tag="lc")
            nc.vector.tensor_tensor(out=lc[:], in0=cc_[:],
                                    in1=_bcast(lpos[:, lt:lt + 1], [P, H]), op=OP.add)
            nc.vector.tensor_tensor(out=nb13[:], in0=iotaS[:],
                                    in1=_bcast(lc[:][:, :, None], [P, H, 13]), op=OP.add)
            vhi = wk.tile([P, H, 13], F32, tag="vhi")
            nc.vector.tensor_scalar(out=vhi[:], in0=nb13[:], scalar1=float(L), scalar2=None,
                                    op0=OP.is_lt)
            valid = wk.tile([P, H, 13], F32, tag="valid")
            nc.vector.scalar_tensor_tensor(out=valid[:], in0=nb13[:], scalar=0.0,
                                           in1=vhi[:], op0=OP.is_ge, op1=OP.mult)
            spec = wk.tile([P, H, 13], F32, tag="spec")
            nc.vector.scalar_tensor_tensor(out=spec[:], in0=nb13[:], scalar=float(NB_HI),
                                           in1=vhi[:], op0=OP.is_gt, op1=OP.mult)

            wv = wk.tile([P, H, 13], F32, tag="wv")
            nc.vector.tensor_tensor(out=wv[:], in0=w13[:], in1=valid[:], op=OP.mult)
            wsum = wk.tile([P, H], F32, tag="wsum")
            nc.vector.tensor_reduce(out=wsum[:], in_=wv[:], axis=mybir.AxisListType.X,
                                    op=OP.add)
            rw = wk.tile([P, H], F32, tag="rw")
            nc.vector.tensor_scalar(out=rw[:], in0=wsum[:], scalar1=1.0, scalar2=None,
                                    op0=OP.max)
            nc.vector.reciprocal(rw[:], rw[:])

            wsp = wk.tile([P, H, 13], F32, tag="wsp")
            nc.vector.tensor_tensor(out=wsp[:], in0=wv[:], in1=spec[:], op=OP.mult)
            wint = wk.tile([P, H, 13], F32, tag="wint")
            nc.vector.tensor_tensor(out=wint[:], in0=wv[:], in1=wsp[:], op=OP.subtract)
            wspec = wk.tile([P, H], F32, tag="wspec")
            nc.vector.tensor_reduce(out=wspec[:], in_=wsp[:], axis=mybir.AxisListType.X,
                                    op=OP.add)
            wspec_s = wk.tile([P, H], F32, tag="wspec_s")
            nc.vector.tensor_tensor(out=wspec_s[:], in0=wspec[:], in1=rw[:], op=OP.mult)
            if lt == LT - 1:
                wcd = wk.tile([P, H, 2], BF16, tag="wcd")
                nc.vector.tensor_scalar(out=wcd[:, :, 0:1], in0=wspec_s[:][:, :, None],
                                        scalar1=0.001, scalar2=None, op0=OP.mult)
                nc.vector.tensor_scalar(out=wcd[:, :, 1:2], in0=wspec_s[:][:, :, None],
                                        scalar1=0.999, scalar2=None, op0=OP.mult)

            om = wk.tile([P, H], F32, tag="om")
            nc.vector.tensor_scalar(out=om[:], in0=phi[:], scalar1=-1.0, scalar2=1.0,
                                    op0=OP.mult, op1=OP.add)
            uf = wk.tile([P, H], F32, tag="uf")
            nc.vector.tensor_tensor(out=uf[:], in0=om[:], in1=rw[:], op=OP.mult)
            uc = wk.tile([P, H], F32, tag="uc")
            nc.vector.tensor_tensor(out=uc[:], in0=phi[:], in1=rw[:], op=OP.mult)

            wf = wk.tile([P, H, 13], BF16, tag="wf")
            nc.vector.tensor_tensor(out=wf[:], in0=wint[:],
                                    in1=_bcast(uf[:][:, :, None], [P, H, 13]), op=OP.mult)
            wcc = wk.tile([P, H, 13], BF16, tag="wcc")
            nc.vector.tensor_tensor(out=wcc[:], in0=wint[:],
                                    in1=_bcast(uc[:][:, :, None], [P, H, 13]), op=OP.mult)

            # scatter indices
            idxf32 = wk.tile([P, H, 13], F32, tag="idxf32")
            nc.vector.tensor_tensor(out=idxf32[:], in0=iotaIDX[:],
                                    in1=_bcast(c012[:][:, :, None], [P, H, 13]), op=OP.add)
            i16 = wk.tile([P, H, 13], I16, tag="i16")
            nc.vector.tensor_copy(i16[:], idxf32[:])
            i16b = wk.tile([P, H, 13], I16, tag="i16b")
            nc.vector.tensor_scalar(out=i16b[:], in0=i16[:], scalar1=1, scalar2=None,
                                    op0=OP.add)

            A0 = wk.tile([P, H, 256], BF16, tag="A0")
            A1 = wk.tile([P, H, 256], BF16, tag="A1")
            for hb in range(2):
                hs = slice(hb * 4, hb * 4 + 4)
                nc.gpsimd.local_scatter(
                    A0[:, hs, :].rearrange("p h w -> p (h w)"),
                    wf[:, hs, :].rearrange("p h a -> p (h a)"),
                    i16[:, hs, :].rearrange("p h a -> p (h a)"),
                    channels=P, num_elems=4 * 256, num_idxs=4 * 13)
                nc.gpsimd.local_scatter(
                    A1[:, hs, :].rearrange("p h w -> p (h w)"),
                    wcc[:, hs, :].rearrange("p h a -> p (h a)"),
                    i16b[:, hs, :].rearrange("p h a -> p (h a)"),
                    channels=P, num_elems=4 * 256, num_idxs=4 * 13)
            A = wk.tile([P, H, 256], BF16, tag="A")
            if lt == LT - 1:
                A2 = wk.tile([P, H, 256], BF16, tag="A2")
                for hb in range(2):
                    hs = slice(hb * 4, hb * 4 + 4)
                    nc.gpsimd.local_scatter(
                        A2[:, hs, :].rearrange("p h w -> p (h w)"),
                        wcd[:, hs, :].rearrange("p h a -> p (h a)"),
                        iotaC[:, hs, :].rearrange("p h a -> p (h a)"),
                        channels=P, num_elems=4 * 256, num_idxs=4 * 2)
                nc.vector.tensor_tensor(out=A0[:], in0=A0[:], in1=A2[:], op=OP.add)
            nc.vector.tensor_tensor(out=A[:], in0=A0[:], in1=A1[:], op=OP.add)


            # --- transpose A blocks and banded matmul ---
            for cci in range(4):
                po = ps_mm.tile([P, P], F32, tag="mm", name="po")
                tp = ps_tr.tile([P, 4, P], BF16, tag="tp")
                at = atp.tile([P, 4, P], BF16, tag="at")
                for hh in range(2):
                    h = 2 * cci + hh
                    for blk in range(2):
                        nc.tensor.transpose(tp[:, 2 * hh + blk, :],
                                            A[:, h, blk * P:(blk + 1) * P], identb[:])
                nc.scalar.copy(at[:], tp[:])
                for hh in range(2):
                    h = 2 * cci + hh
                    prange = slice(hh * 64, hh * 64 + 64)
                    for blk in range(2):
                        nc.tensor.matmul(po[prange, :],
                                         v_sb[lt + blk][:, h * D:(h + 1) * D],
                                         at[:, 2 * hh + blk, :], start=(blk == 0),
                                         stop=(blk == 1))
                nc.scalar.copy(outT_sb[cci][:, lt * P:(lt + 1) * P], po[:])

        # ---------------- SE partial sums + AllGather + local sum ----------------
        seacc = cst.tile([P, 4], F32)
        for cci in range(4):
            nc.vector.tensor_reduce(out=seacc[:, cci:cci + 1],
                                    in_=outT_sb[cci][:],
                                    axis=mybir.AxisListType.X, op=OP.add)
        if skip_cc:
            armean = seacc
        else:
            cci_d = dram.tile([P, 4], F32)
            cco_d = dram.tile([4, P, 4], F32)
            nc.gpsimd.dma_start(cci_d[:], seacc[:])
            nc.gpsimd.collective_compute(
                "AllGather", OP.bypass,
                replica_groups=[[0, 1, 2, 3], [4, 5, 6, 7]],
                ins=[cci_d[:].opt()], outs=[cco_d[:].opt()])
            # gathered along partition axis: [4 ranks, 128, 4] -> partitions 0..511?
            ag = cst.tile([P, 4, 4], F32)
            nc.sync.dma_start(ag[:], cco_d[:].rearrange("r p f -> p r f"))
            armean = cst.tile([P, 4], F32)
            nc.vector.tensor_tensor(out=armean[:], in0=ag[:, 0, :], in1=ag[:, 1, :],
                                    op=OP.add)
            nc.vector.tensor_tensor(out=armean[:], in0=armean[:], in1=ag[:, 2, :],
                                    op=OP.add)
            nc.vector.tensor_tensor(out=armean[:], in0=armean[:], in1=ag[:, 3, :],
                                    op=OP.add)

        # ---------------- SE MLP (transposed throughout) ----------------
        hidT_ps = ps_sm.tile([P, 1], F32, tag="tiny", name="hidT_ps")
        for cci in range(4):
            nc.tensor.matmul(hidT_ps[:], sw1T[cci][:], armean[:, cci:cci + 1],
                             start=(cci == 0), stop=(cci == 3))
        hidT = cst.tile([P, 1], BF16)
        if USE_SILU:
            nc.scalar.activation(hidT[:], hidT_ps[:], ACT.Silu)
        else:
            hsg = cst.tile([P, 1], F32)
            nc.scalar.activation(hsg[:], hidT_ps[:], ACT.Sigmoid)
            nc.vector.tensor_tensor(out=hidT[:], in0=hidT_ps[:], in1=hsg[:], op=OP.mult)
        owb = []
        for cci in range(4):
            scT_ps = ps_sm.tile([P, 1], F32, tag="tiny", name="scT_ps")
            nc.tensor.matmul(scT_ps[:], sw2T[:, cci * P:(cci + 1) * P], hidT[:],
                             start=True, stop=True)
            scT = cst.tile([P, 1], F32, tag=f"scT{cci}", name=f"scT{cci}")
            nc.scalar.activation(scT[:], scT_ps[:], ACT.Sigmoid)
            ow = cst.tile([P, C], BF16, tag=f"owb{cci}", name=f"owb{cci}")
            nc.vector.tensor_scalar(out=ow[:], in0=owT[cci][:], scalar1=scT[:],
                                    scalar2=None, op0=OP.mult)
            owb.append(ow)

        # ---------------- final out_w matmul + silu + DMA out ----------------
        # out[l, cout] = sum_cin out_preT[cin, l] * ow'[cin, cout]  (lhsT = out_preT)
        for lt in range(LT):
            lsl = slice(lt * P, (lt + 1) * P)
            pf_ = ps_mm.tile([P, C], F32, tag="mm", name="pf_")
            for cci in range(4):
                nc.tensor.matmul(pf_[:], outT_sb[cci][:, lsl], owb[cci][:],
                                 start=(cci == 0), stop=(cci == 3))
            fo = wk.tile([P, C], F32, tag="fo")
            if USE_SILU:
                nc.scalar.activation(fo[:], pf_[:], ACT.Silu)
            else:
                nc.scalar.activation(fo[:], pf_[:], ACT.Sigmoid)
                nc.vector.tensor_tensor(out=fo[:], in0=fo[:], in1=pf_[:], op=OP.mult)
            nc.sync.dma_start(dd["out"].ap()[lsl, :], fo[:])


def make_in_maps(inputs, n_cores=8):
    x = np.ascontiguousarray(inputs["x"], dtype=np.float32)
    window_w = inputs["window_w"]; window_b = inputs["window_b"]
    window_gamma = inputs["window_gamma"]
    offset_w = inputs["offset_w"]; offset_b = inputs["offset_b"]
    offset_gamma = inputs["offset_gamma"]
    kernel_w = inputs["kernel_w"]; kernel_b = inputs["kernel_b"]
    kernel_gamma = inputs["kernel_gamma"]
    v_w = inputs["v_w"]; v_b = inputs["v_b"]
    se_w1 = inputs["se_w1"]; se_w2 = inputs["se_w2"]; out_w = inputs["out_w"]

    woT = np.concatenate([window_w, offset_w], 0).T.astype(np.float32)      # (512,16)
    wob = np.concatenate([window_b, offset_b])[None].astype(np.float32)     # (1,16)
    wog = np.tile(np.concatenate([window_gamma, offset_gamma])[None], (P, 1)).astype(np.float32)
    kwT = np.ascontiguousarray(kernel_w.T, np.float32)
    kb = kernel_b[None].astype(np.float32)
    kgm = np.tile(kernel_gamma[None], (P, 1)).astype(np.float32)
    vwT = np.ascontiguousarray(v_w.T, np.float32)
    vbm = v_b[None].astype(np.float32)
    sw1T = np.ascontiguousarray(se_w1.T, np.float32) / np.float32(L)
    sw2T = np.ascontiguousarray(se_w2.T).astype(ml_dtypes.bfloat16)
    owT = np.ascontiguousarray(out_w.T, np.float32)

    in_maps = []
    for i in range(n_cores):
        b, q = divmod(i, 4)
        lo = q * LSH - HALO
        xpad = np.zeros((XROWS, C), np.float32)
        s0, s1 = max(lo, 0), min(lo + XROWS, L)
        xpad[s0 - lo:s1 - lo] = x[b, s0:s1]
        xT = np.ascontiguousarray(xpad.T)
        lpos = (q * LSH + np.arange(LSH, dtype=np.float32)).reshape(LT, P).T.copy()
        in_maps.append(dict(
            xT=xT, lpos=lpos, woT=woT, wob=wob, wog=wog, kwT=kwT, kb=kb,
            kg=kgm, vwT=vwT, vb=vbm, sw1T=sw1T, sw2T=sw2T, owT=owT,
            ones=np.ones((1, P), np.float32),
        ))
    return in_maps


def kernel(**inputs) -> np.ndarray:
    if "graph" not in _GRAPH_CACHE:
        _GRAPH_CACHE["graph"] = build_graph(8)
    nc = _GRAPH_CACHE["graph"]
    in_maps = make_in_maps(inputs, 8)
    res = run_bass_kernel_spmd(nc, in_maps, core_ids=list(range(8)))
    out = np.zeros((B, L, C), np.float32)
    for i in range(8):
        b, q = divmod(i, 4)
        out[b, q * LSH:(q + 1) * LSH] = res.results[i]["out"]
    return out


if __name__ == "__main__":
    import reference
    inputs = {k: np.asarray(v) for k, v in reference.setup_inputs().items()}
    got = kernel(**inputs)
    import jax.numpy as jnp
    exp = np.asarray(reference.reference(**{k: jnp.asarray(v) for k, v in inputs.items()}))
    rel = np.linalg.norm(got - exp) / np.linalg.norm(exp)
    print("Relative error:", rel)
